# revision 80
# baseline (speedup 1.0000x reference)
"""Trainium2 Bass kernel for nn_HCMGNNBasedMetaPathModel (v4).

Bacteria rows sharded over 8 cores (3750 -> padded 3840); trait side and
weights replicated. Edge segment ops are dense adjacency matmuls with
EXACT fp8e4m3 edge counts; the 1/max(deg,1) mean normalization is
applied post-matmul in f32.

v4 over v3 (957us -> ~805us):
 - phases B and C use fp8 DoubleRow matmuls (both operands fp8): the
   aggregation features (tb, lin_l-transformed tt) are cast to fp8e4m3,
   paired along the contraction dim with the fp8 adjacency tiles.
   Numerically validated: adds <0.1% to the final max-rel error.
 - layer pipeline reordered: C(i), A1b(i+1), A2b(i+1), D(i),
   A1t(i+1), A2t(i+1), B(i+1)+AR trigger.  The AllReduce of layer i
   (~40-55us fixed latency) completes under C(i)+A1b+A2b; D's
   vector/scalar epilogues overlap A1t/A2t and B's DMA-bound stretch.
 - epilogues use LN/l2 per-row scale invariance twice: the deg-scaled
   lin_r term is matmul-accumulated into the aggregation PSUM via the
   identity (so no cbv STT), and LN(u/||u||+res) == LN(u+||u||*res)
   (so no reciprocal).  Per-group batching of the [128,1]-wide stat
   ops (sqrt/recip/mean*inv) cuts small-op overhead ~4x.
 - final phase fully pipelined: F1b bacteria head overlaps D(2); F2
   metapath groups fused with the F3 p1-head stage and the F1c trait
   head; F3 p2-head keeps hb UNnormalized and applies 1/||hb|| on the
   sim output copy; sim matmuls+writeout lag one pair so the 15MB
   output DMA spreads across the whole tail.
 - Abt stream: fp8 pair tiles split into halves on two DMA queues
   (sync+gpsimd) with a 5-deep pool; fp8 casts round-robin over
   gpsimd/vector/scalar.
"""
import contextlib
import sys

for _p in ("/opt/trn_rl_repo",):
    if _p not in sys.path:
        sys.path.insert(0, _p)

import numpy as np
import ml_dtypes

import concourse.bass as bass
import concourse.tile as tile
from concourse import bacc, mybir
from concourse.bass_utils import run_bass_kernel_spmd

BF16 = ml_dtypes.bfloat16
FP8 = ml_dtypes.float8_e4m3
F32 = mybir.dt.float32
BF = mybir.dt.bfloat16
F8 = mybir.dt.float8e4
AF = mybir.ActivationFunctionType
ALU = mybir.AluOpType
DR = mybir.MatmulPerfMode.DoubleRow
ts, ds = bass.ts, bass.ds

N_B, N_T, D, L, M = 30000, 2000, 256, 3, 2
NC = 8
B_SH = 3750          # real bacteria rows per core
BP = 3840            # padded bacteria rows per core
NBT = BP // 128      # 30 node tiles
NBP = NBT // 2       # 15 node-tile pairs
TP = 2048            # padded trait rows
NTT = TP // 128      # 16 trait tiles
GRP = 4              # bacteria tiles per aggregation group
NBG = 8              # ceil(30/4) groups
GW = GRP * 128       # 512 group width (last group: 2 real tiles + pad)
LN_EPS = 1e-5
NW = 12 * 3 + 10     # wc entries

# ---------------------------------------------------------------------------
# Host-side preprocessing
# ---------------------------------------------------------------------------


def _counts(src, dst, n_dst, n_src):
    """A[d, s] = #edges (s->d) as float32; plus per-dst degree."""
    idx = dst.astype(np.int64) * n_src + src.astype(np.int64)
    A = np.bincount(idx, minlength=n_dst * n_src).astype(np.float32)
    A = A.reshape(n_dst, n_src)
    deg = np.bincount(dst.astype(np.int64), minlength=n_dst).astype(np.float32)
    return A, deg


def _to_fp8_exact(A, what):
    A8 = A.astype(FP8)
    assert np.array_equal(A8.astype(np.float32), A), f"{what} not fp8-exact"
    return A8


def _prep(inp):
    f32 = np.float32
    emb_b = np.asarray(inp["emb_b"], f32)
    emb_t = np.asarray(inp["emb_t"], f32)

    A_tb, deg_b = _counts(np.asarray(inp["src_tb"]), np.asarray(inp["dst_tb"]),
                          N_B, N_T)
    A_bt, deg_t = _counts(np.asarray(inp["src_bt"]), np.asarray(inp["dst_bt"]),
                          N_T, N_B)
    mpw = np.asarray(inp["mp_w"], np.float64)
    e = np.exp(mpw - mpw.max())
    w = e / e.sum()
    sw = float(w.sum())
    mp_adj = np.asarray(inp["mp_adj"], f32)
    A_mp = (w[0] * mp_adj[0].astype(np.float64) +
            w[1] * mp_adj[1].astype(np.float64)).astype(f32)

    xb0 = np.zeros((NC, NBT, 128, D), BF16)
    xb0.reshape(NC, BP, D)[:, :B_SH] = emb_b.reshape(NC, B_SH, D).astype(BF16)
    xt0 = np.zeros((NTT, 128, D), BF16)
    xt0.reshape(TP, D)[:N_T] = emb_t.astype(BF16)

    BPG = NBG * GW  # 4096 padded for group layout

    def shard_T(A):  # [N_B, N_T] -> per-core [NBG, 128, NTT, GW] trait-major
        out = np.zeros((NC, NBG, 128, NTT, GW), f32)
        for c in range(NC):
            blk = np.zeros((TP, BPG), f32)
            blk[:N_T, :B_SH] = A[c * B_SH:(c + 1) * B_SH].T
            out[c] = blk.reshape(NTT, 128, NBG, GW).transpose(2, 1, 0, 3)
        return out

    At8 = _to_fp8_exact(shard_T(A_tb), "A_tb counts")
    Amp8 = _to_fp8_exact(shard_T(A_mp), "A_mp")
    # Abt in node-tile PAIRS for DoubleRow: [NBP, 128, 2, TP]
    Abt8 = np.zeros((NC, NBP, 128, 2, TP), FP8)
    for c in range(NC):
        blk = np.zeros((BP, TP), f32)
        blk[:B_SH, :N_T] = A_bt[:, c * B_SH:(c + 1) * B_SH].T
        Abt8[c] = _to_fp8_exact(
            blk.reshape(NBP, 2, 128, TP).transpose(0, 2, 1, 3), "A_bt counts")

    # degree normalizers: deg_real = max(deg, 1) (exact f32 ints).
    # The kernel exploits LN/l2 per-row scale invariance: instead of
    # l2(agg/deg + lr) it computes l2(agg + deg*lr), so only deg_real
    # is needed.
    degR = np.maximum(deg_b, 1.0)
    degbR = np.zeros((NC, 128, NBT), f32)
    for c in range(NC):
        v = np.ones(BP, f32)
        v[:B_SH] = degR[c * B_SH:(c + 1) * B_SH]
        degbR[c] = v.reshape(NBT, 128).T
    dtr = np.ones(TP, f32)
    dtr[:N_T] = np.maximum(deg_t, 1.0)
    degtR = np.ascontiguousarray(dtr.reshape(NTT, 128).T)

    # ---- weights: gamma folds; all biases must be zero ----
    for nm in ("bt_b", "bt_t", "bl_b", "bl_t", "mpb", "bp1b", "bp1t",
               "bp2b", "bp2t", "lnb_b", "lnb_t", "mplnb", "plnbb", "plnbt"):
        assert not np.any(np.asarray(inp[nm])), f"{nm} must be zero"
    plngb = np.asarray(inp["plngb"], f32)
    plngt = np.asarray(inp["plngt"], f32)
    assert (plngb > 0).all() and (plngt > 0).all()

    Wt_b = np.asarray(inp["Wt_b"], f32)
    Wt_t = np.asarray(inp["Wt_t"], f32)
    Wl_b = np.asarray(inp["Wl_b"], f32)
    Wr_b = np.asarray(inp["Wr_b"], f32)
    Wl_t = np.asarray(inp["Wl_t"], f32)
    Wr_t = np.asarray(inp["Wr_t"], f32)
    lng_b = np.asarray(inp["lng_b"], f32)
    lng_t = np.asarray(inp["lng_t"], f32)

    wlist = []

    def addW(*WTs):
        """for kc in (0,1): for each WT: append WT[kc*128:(kc+1)*128]."""
        base = len(wlist)
        for kc in range(2):
            for WT in WTs:
                wlist.append(np.ascontiguousarray(
                    WT[kc * 128:(kc + 1) * 128]).astype(BF16))
        return base

    wi = {}
    g_b = np.ones(D, f32)
    g_t = np.ones(D, f32)
    for i in range(L):
        WtTb = Wt_b[i].T * g_b[:, None]
        WcTb = (Wr_b[i] @ Wt_b[i]).T * g_b[:, None]
        WtTt = Wt_t[i].T * g_t[:, None]
        WctT = (Wl_b[i] @ Wt_t[i]).T * g_t[:, None]
        WcTt = (Wr_t[i] @ Wt_t[i]).T * g_t[:, None]
        wi[("b", i)] = addW(WtTb, WcTb)          # stride 2 per kc
        wi[("t", i)] = addW(WtTt, WcTt, WctT)    # [tt, lrt, ttl] per kc
        wi[("wl", i)] = addW(Wl_t[i].T)          # stride 1 per kc
        g_b, g_t = lng_b[i], lng_t[i]

    mpW = np.asarray(inp["mpW"], f32)
    g_mp = np.asarray(inp["mplng"], f32)
    Wp1b = np.asarray(inp["Wp1b"], f32)
    Wp1t = np.asarray(inp["Wp1t"], f32)
    wi["fb"] = addW(sw * (mpW.T * g_b[:, None]), Wp1b[:, :D].T * g_b[:, None])
    wi["ft"] = addW(mpW.T * g_t[:, None], Wp1t.T * g_t[:, None])
    wi["p1bb"] = addW(Wp1b[:, D:].T * g_mp[:, None])
    Wc = np.stack(wlist)
    assert Wc.shape[0] == NW, Wc.shape

    W2b = (np.asarray(inp["Wp2b"], f32) * plngb).T
    W2t = (np.asarray(inp["Wp2t"], f32) * plngt).T
    w128 = np.stack([
        np.ascontiguousarray(W2b[:128]).astype(BF16),
        np.ascontiguousarray(W2b[128:]).astype(BF16),
        np.ascontiguousarray(W2t[:128]).astype(BF16),
        np.ascontiguousarray(W2t[128:]).astype(BF16),
    ])

    ident = np.eye(128, dtype=f32).astype(BF16)
    temp = float(np.asarray(inp["temperature"]).reshape(-1)[0])
    simb = float(np.asarray(inp["sim_bias"]).reshape(-1)[0])

    shared = dict(xt0=xt0, Wc=Wc, W128=w128, degt=degtR, ident=ident)
    in_maps = []
    for c in range(NC):
        m = dict(shared)
        m["xb0"] = np.ascontiguousarray(xb0[c])
        m["At8"] = np.ascontiguousarray(At8[c])
        m["Abt8"] = np.ascontiguousarray(Abt8[c])
        m["Amp8"] = np.ascontiguousarray(Amp8[c])
        m["degb"] = np.ascontiguousarray(degbR[c])
        in_maps.append(m)
    meta = dict(wi=wi, temp=temp, simb=simb)
    return in_maps, meta


# ---------------------------------------------------------------------------
# Device program
# ---------------------------------------------------------------------------


def build_program(meta):
    nc = bacc.Bacc("TRN2", target_bir_lowering=False, debug=False,
                   num_devices=NC)
    wi = meta["wi"]
    temp = meta["temp"]

    xb0_d = nc.dram_tensor("xb0", [NBT, 128, D], BF, kind="ExternalInput")
    xt0_d = nc.dram_tensor("xt0", [NTT, 128, D], BF, kind="ExternalInput")
    At_d = nc.dram_tensor("At8", [NBG, 128, NTT, GW], F8, kind="ExternalInput")
    Abt_d = nc.dram_tensor("Abt8", [NBP, 128, 2, TP], F8,
                           kind="ExternalInput")
    Amp_d = nc.dram_tensor("Amp8", [NBG, 128, NTT, GW], F8,
                           kind="ExternalInput")
    Wc_d = nc.dram_tensor("Wc", [NW, 128, D], BF, kind="ExternalInput")
    W128_d = nc.dram_tensor("W128", [4, 128, 128], BF, kind="ExternalInput")
    degb_d = nc.dram_tensor("degb", [128, NBT], F32, kind="ExternalInput")
    degt_d = nc.dram_tensor("degt", [128, NTT], F32, kind="ExternalInput")
    id_d = nc.dram_tensor("ident", [128, 128], BF, kind="ExternalInput")
    sim_d = nc.dram_tensor("simO", [NBT, 128, TP], BF, kind="ExternalOutput")

    with tile.TileContext(nc) as tc, contextlib.ExitStack() as ctx:
        cpool = ctx.enter_context(tc.tile_pool(name="const", bufs=1))
        fpool = ctx.enter_context(tc.tile_pool(name="feat", bufs=1))
        spool = ctx.enter_context(tc.tile_pool(name="at_stream", bufs=3))
        epool = ctx.enter_context(tc.tile_pool(name="epi", bufs=3))
        hpool = ctx.enter_context(tc.tile_pool(name="hbf", bufs=3))
        qpool = ctx.enter_context(tc.tile_pool(name="sq", bufs=1))
        tpool = ctx.enter_context(tc.tile_pool(name="tiny", bufs=24))
        ppool = ctx.enter_context(tc.tile_pool(name="pscr", bufs=2))
        dpool = ctx.enter_context(tc.tile_pool(name="dram", bufs=2,
                                               space="DRAM"))

        # ---- persistent features first (layer-0 transposes need them);
        # chunked so the first transposes start almost immediately ----
        ident = cpool.tile([128, 128], BF)
        nc.sync.dma_start(ident[:], id_d[:])
        wc = cpool.tile([128, NW, D], BF)
        nc.sync.dma_start(wc[:, 0:12, :],
                          Wc_d[0:12].rearrange("n p d -> p n d"))
        xb = fpool.tile([128, NBT, D], BF, tag="xb")
        for t0 in range(0, NBT, 2):
            nc.sync.dma_start(xb[:, t0:t0 + 2, :],
                              xb0_d[t0:t0 + 2].rearrange("n p d -> p n d"))
        xt = fpool.tile([128, NTT, D], BF, tag="xt")
        for t0 in range(0, NTT, 4):
            nc.gpsimd.dma_start(xt[:, t0:t0 + 4, :],
                                xt0_d[t0:t0 + 4].rearrange("n p d -> p n d"))

        # ---- constants (stream under the first transposes) ----
        for j0 in range(12, NW, 12):
            j1 = min(j0 + 12, NW)
            nc.sync.dma_start(wc[:, j0:j1, :],
                              Wc_d[j0:j1].rearrange("n p d -> p n d"))
        w128 = cpool.tile([128, 4, 128], BF)
        nc.sync.dma_start(w128[:], W128_d.rearrange("n p d -> p n d"))
        degb = cpool.tile([128, NBT], F32)
        nc.sync.dma_start(degb[:], degb_d[:])
        degt = cpool.tile([128, NTT], F32)
        nc.sync.dma_start(degt[:], degt_d[:])
        epsb = cpool.tile([128, 1], F32, name="epsb")
        nc.gpsimd.memset(epsb[:], LN_EPS)
        eps24 = cpool.tile([128, 1], F32, name="eps24")
        nc.gpsimd.memset(eps24[:], 1e-24)

        # feature tiles: [tb|lrb], [tt|lrt], fp8 copies for DoubleRow
        tl_b = fpool.tile([128, NBT, 2, D], BF, tag="tl_b")
        tl_t = fpool.tile([128, NTT, 2, D], BF, tag="tl_t")
        tb8 = fpool.tile([128, NBT, 2, 128], F8, tag="tb8")
        tt8 = fpool.tile([128, NTT, D], F8, tag="tt8")
        xbT = fpool.tile([128, 2, NBT, 128], BF, tag="xbT")
        xtT = fpool.tile([128, 2, NTT, 128], BF, tag="xtT")

        # engine alternation for plain psum->sbuf copies
        _alt = [0]

        def cpy(dst, src):
            _alt[0] ^= 1
            (nc.vector.tensor_copy if _alt[0] else nc.scalar.copy)(dst, src)

        # 3-way rotation for sbuf->sbuf fp8 casts (gpsimd is slow alone)
        _c3 = [0]

        def cast3(dst, src):
            _c3[0] = (_c3[0] + 1) % 3
            eng = (nc.gpsimd, nc.vector, nc.scalar)[_c3[0]]
            (eng.copy if eng is nc.scalar else eng.tensor_copy)(dst, src)

        def scpy(dst, src, scale_ap):
            """psum->sbuf copy with per-partition scale, alternating."""
            _alt[0] ^= 1
            if _alt[0]:
                nc.vector.tensor_scalar_mul(dst, src, scale_ap)
            else:
                nc.scalar.activation(dst, src, AF.Copy, scale=scale_ap)

        def transpose_into(dst, src_tile, n_tiles, trp):
            """dst [128, 2, n_tiles, 128] <- per-tile transposes of
            src_tile [128, n_tiles, 256]; two node tiles per psum buf,
            one merged copy per pair."""
            for nt0 in range(0, n_tiles, 2):
                ps = trp.tile([128, 2, 2, 128], BF, tag="tr")
                for k in range(2):
                    for kcc in range(2):
                        nc.tensor.transpose(ps[:, kcc, k, :],
                                            src_tile[:, nt0 + k, ts(kcc, 128)],
                                            ident[:])
                cpy(dst[:, :, nt0:nt0 + 2, :], ps[:])

        def ln_z(out_ap, s1_ap):
            """out = normalize(s1) along free dim (gamma folded downstream).
            The 256-wide apply alternates between vector and scalar."""
            st6 = tpool.tile([128, 6], F32, tag="st6")
            nc.vector.bn_stats(st6[:], s1_ap)
            mv = tpool.tile([128, 2], F32, tag="mv")
            nc.vector.bn_aggr(mv[:], st6[:])
            std = tpool.tile([128, 1], F32, tag="std")
            nc.scalar.activation(std[:], mv[:, 1:2], AF.Sqrt, bias=epsb[:])
            inv = tpool.tile([128, 1], F32, tag="inv")
            nc.vector.reciprocal(inv[:], std[:])
            _alt[0] ^= 1
            if _alt[0]:
                mi = tpool.tile([128, 1], F32, tag="mi")
                nc.scalar.activation(mi[:], mv[:, 0:1], AF.Copy, scale=inv[:])
                nc.vector.tensor_scalar(out_ap, s1_ap, inv[:], mi[:],
                                        ALU.mult, ALU.subtract)
            else:
                nmi = tpool.tile([128, 1], F32, tag="mi")
                nc.vector.tensor_scalar(nmi[:], mv[:, 0:1], inv[:], -1.0,
                                        ALU.mult, ALU.mult)
                nc.scalar.activation(out_ap, s1_ap, AF.Identity, bias=nmi[:],
                                     scale=inv[:])

        def l2_rec(v_ap, scale=None):
            """[128,1] 1/sqrt(||v||^2+1e-24) per row, optionally * scale."""
            ssq = tpool.tile([128, 1], F32, tag="ssq")
            scr = qpool.tile([128, D], F32, tag="sq")
            nc.scalar.activation(scr[:, :v_ap.shape[-1]], v_ap, AF.Square,
                                 accum_out=ssq[:])
            nrm = tpool.tile([128, 1], F32, tag="l2n")
            nc.scalar.activation(nrm[:], ssq[:], AF.Sqrt, bias=eps24[:])
            rec = tpool.tile([128, 1], F32, tag="l2r")
            nc.vector.reciprocal(rec[:], nrm[:])
            if scale is not None:
                nc.scalar.mul(rec[:], rec[:], scale)
            return rec

        def ln_z_group(s1g, n, out_aps):
            """Batched LN over n windows s1g[:, k, :]: the [128,1]-ish
            stat ops run once per group instead of once per tile."""
            st6g = tpool.tile([128, GRP, 6], F32, tag="st6g", bufs=4)
            for k in range(n):
                nc.vector.bn_stats(st6g[:, k, :], s1g[:, k, :])
            mv4 = tpool.tile([128, GRP, 2], F32, tag="mv4", bufs=4)
            for k in range(n):
                nc.vector.bn_aggr(mv4[:, k, :], st6g[:, k, :])
            stdg = tpool.tile([128, GRP], F32, tag="stdg", bufs=4)
            nc.scalar.activation(stdg[:, 0:n], mv4[:, 0:n, 1], AF.Sqrt,
                                 bias=epsb[:])
            invg = tpool.tile([128, GRP], F32, tag="invg", bufs=4)
            nc.vector.reciprocal(invg[:, 0:n], stdg[:, 0:n])
            mig = tpool.tile([128, GRP], F32, tag="mig", bufs=4)
            nc.vector.tensor_tensor(mig[:, 0:n], mv4[:, 0:n, 0],
                                    invg[:, 0:n], ALU.mult)
            nmig = tpool.tile([128, GRP], F32, tag="nmig", bufs=4)
            nc.vector.tensor_scalar_mul(nmig[:, 0:n], mig[:, 0:n], -1.0)
            for k in range(n):
                _alt[0] ^= 1
                if _alt[0]:
                    nc.vector.tensor_scalar(out_aps[k], s1g[:, k, :],
                                            invg[:, k:k + 1], mig[:, k:k + 1],
                                            ALU.mult, ALU.subtract)
                else:
                    nc.scalar.activation(out_aps[k], s1g[:, k, :],
                                         AF.Identity, bias=nmig[:, k:k + 1],
                                         scale=invg[:, k:k + 1])

        def sage_epi_group(pss, res_aps, out_aps):
            """Batched: out = LN(l2(u) + res) per tile, u in PSUM
            (deg-scaled lr was matmul-accumulated via the identity).
            Uses LN's per-row scale invariance: LN(u/||u|| + res) ==
            LN(u + ||u||*res), so no reciprocal on the l2 side."""
            n = len(pss)
            ssqg = tpool.tile([128, GRP], F32, tag="ssqg", bufs=4)
            for k in range(n):
                scr = qpool.tile([128, D], F32, tag="sq")
                nc.scalar.activation(scr[:], pss[k], AF.Square,
                                     accum_out=ssqg[:, k:k + 1])
            nrmg = tpool.tile([128, GRP], F32, tag="nrmg", bufs=4)
            nc.scalar.activation(nrmg[:, 0:n], ssqg[:, 0:n], AF.Sqrt,
                                 bias=eps24[:])
            s1g = epool.tile([128, GRP, D], F32, tag="s1g", bufs=2)
            for k in range(n):
                nc.vector.scalar_tensor_tensor(s1g[:, k, :], res_aps[k],
                                               nrmg[:, k:k + 1], pss[k],
                                               ALU.mult, ALU.add)
            ln_z_group(s1g, n, out_aps)

        def ln_relu(out_ap, h_ap):
            """out = relu(normalize(h)) (gamma>0 folded downstream)."""
            st6 = tpool.tile([128, 6], F32, tag="st6")
            nc.vector.bn_stats(st6[:], h_ap)
            mv = tpool.tile([128, 2], F32, tag="mv")
            nc.vector.bn_aggr(mv[:], st6[:])
            std = tpool.tile([128, 1], F32, tag="std")
            nc.scalar.activation(std[:], mv[:, 1:2], AF.Sqrt, bias=epsb[:])
            inv = tpool.tile([128, 1], F32, tag="inv")
            nc.vector.reciprocal(inv[:], std[:])
            nmi = tpool.tile([128, 1], F32, tag="nmi")
            nc.vector.tensor_scalar(nmi[:], mv[:, 0:1], inv[:], -1.0,
                                    ALU.mult, ALU.mult)
            nc.scalar.activation(out_ap, h_ap, AF.Relu, bias=nmi[:],
                                 scale=inv[:])

        # ---------------- phase builders ----------------
        def phase_A1b(i):
            with tc.tile_pool(name=f"psAtb{i}", bufs=2, space="PSUM") as trA:
                transpose_into(xbT, xb, NBT, trA)

        def phase_A1t(i):
            with tc.tile_pool(name=f"psAtt{i}", bufs=2, space="PSUM") as trA:
                transpose_into(xtT, xt, NTT, trA)

        def phase_A2b(i):
            jb = wi[("b", i)]
            with tc.tile_pool(name=f"psA{i}", bufs=3, space="PSUM") as mmA:
                for nt in range(NBT):
                    ps = mmA.tile([128, 2, 256], F32, tag="mmb")
                    for kc in range(2):
                        nc.tensor.matmul(ps[:], xbT[:, kc, nt, :],
                                         wc[:, ds(jb + 2 * kc, 2), :],
                                         start=kc == 0, stop=kc == 1)
                    cpy(tl_b[:, nt, 0, :], ps[:, 0, :])
                    scpy(tl_b[:, nt, 1, :], ps[:, 1, :], degb[:, nt:nt + 1])
                    cast3(tb8[:, nt, :, :], tl_b[:, nt, 0, :])

        def phase_A2t(i):
            jt = wi[("t", i)]
            with tc.tile_pool(name=f"psAt{i}", bufs=3, space="PSUM") as mmA:
                for tt_ in range(NTT):
                    ps = mmA.tile([128, 2, 256], F32, tag="mmt", bufs=2)
                    ps2 = mmA.tile([128, 256], F32, tag="mmt2", bufs=2)
                    for kc in range(2):
                        # [tt|lrt] one 512-wide group; ttl separate bank
                        nc.tensor.matmul(ps[:], xtT[:, kc, tt_, :],
                                         wc[:, ds(jt + 3 * kc, 2), :],
                                         start=kc == 0, stop=kc == 1)
                        nc.tensor.matmul(ps2[:], xtT[:, kc, tt_, :],
                                         wc[:, jt + 3 * kc + 2, :],
                                         start=kc == 0, stop=kc == 1)
                    cpy(tl_t[:, tt_, 0, :], ps[:, 0, :])
                    scpy(tl_t[:, tt_, 1, :], ps[:, 1, :], degt[:, tt_:tt_ + 1])
                    cpy(tt8[:, tt_, :], ps2[:])

        # Abt stream pool: lives only through the layer pipeline, freed
        # before the final-phase pools are first used.
        bpool_cm = tc.tile_pool(name="abt_stream", bufs=3)
        bpool = bpool_cm.__enter__()

        def phase_B(i):
            """partial_t via DoubleRow fp8; trigger AllReduce."""
            pscr = ppool.tile([128, 2, TP], BF, tag="pscr")
            with tc.tile_pool(name=f"psB{i}", bufs=1, space="PSUM") as ptp:
                pt = [ptp.tile([128, TP], F32, tag=f"pt{dh}",
                               name=f"pt{i}_{dh}") for dh in range(2)]
                for cp in range(NBP):
                    for sh in range(2):
                        ab = bpool.tile([128, 2, TP // 2], F8, tag="abt",
                                        bufs=5)
                        (nc.sync if sh == 0 else nc.gpsimd).dma_start(
                            ab[:], Abt_d[cp][:, :, ts(sh, TP // 2)])
                        for dh in range(2):
                            for s in range(2):
                                nc.tensor.matmul(
                                    pt[dh][:, ts(2 * sh + s, 512)],
                                    tb8[:, ds(2 * cp, 2), dh, :],
                                    ab[:, :, ts(s, 512)],
                                    start=cp == 0, stop=cp == NBP - 1,
                                    perf_mode=DR)
                nc.vector.tensor_copy(pscr[:, 0, :], pt[0][:])
                nc.scalar.copy(pscr[:, 1, :], pt[1][:])
            bounce_in = dpool.tile([128, 2, TP], BF, tag="bin")
            bounce_out = dpool.tile([128, 2, TP], BF, tag="bout",
                                    addr_space="Shared")
            nc.scalar.dma_start(bounce_in[:], pscr[:])
            nc.gpsimd.collective_compute(
                "AllReduce", ALU.add, replica_groups=[list(range(NC))],
                ins=[bounce_in.opt()], outs=[bounce_out.opt()])
            return bounce_out

        def phase_C(i, glo=0, ghi=NBG):
            with tc.tile_pool(name=f"psC{i}_{glo}", bufs=6,
                              space="PSUM") as mmC:
                for g in range(glo, ghi):
                    ntiles = min(GRP, NBT - g * GRP)
                    pss = [mmC.tile([128, D], F32, tag="mm",
                                    name=f"cps{i}_{g}_{k}")
                           for k in range(ntiles)]
                    at = spool.tile([128, NTT, GW], F8, tag="at")
                    nc.sync.dma_start(at[:], At_d[g])
                    for t2 in range(NTT // 2):
                        for k in range(ntiles):
                            nc.tensor.matmul(pss[k][:],
                                             at[:, ds(2 * t2, 2), ts(k, 128)],
                                             tt8[:, ds(2 * t2, 2), :],
                                             start=t2 == 0, stop=False,
                                             perf_mode=DR)
                    for k in range(ntiles):
                        nt = g * GRP + k
                        nc.tensor.matmul(pss[k][:], ident[:],
                                         tl_b[:, nt, 1, :],
                                         start=False, stop=True)
                    nts = [g * GRP + k for k in range(ntiles)]
                    sage_epi_group([pss[k][:] for k in range(ntiles)],
                                   [tl_b[:, nt, 0, :] for nt in nts],
                                   [xb[:, nt, :] for nt in nts])

        def phase_D(i, bounce_out):
            jl = wi[("wl", i)]
            pm = ppool.tile([128, 2, TP], BF, tag="pscr")
            nc.sync.dma_start(pm[:], bounce_out[:])
            with tc.tile_pool(name=f"psD{i}", bufs=5, space="PSUM") as mmD:
                for g in range(NTT // GRP):
                    pss = []
                    for k in range(GRP):
                        tt_ = g * GRP + k
                        ps = mmD.tile([128, D], F32, tag="mm")
                        for kc in range(2):
                            nc.tensor.matmul(ps[:], pm[:, kc, ts(tt_, 128)],
                                             wc[:, jl + kc, :],
                                             start=kc == 0, stop=False)
                        nc.tensor.matmul(ps[:], ident[:], tl_t[:, tt_, 1, :],
                                         start=False, stop=True)
                        pss.append(ps)
                    tts = [g * GRP + k for k in range(GRP)]
                    sage_epi_group([p_[:] for p_ in pss],
                                   [tl_t[:, t_, 0, :] for t_ in tts],
                                   [xt[:, t_, :] for t_ in tts])

        # ================= main pipeline =================
        phase_A1b(0)
        phase_A2b(0)
        phase_A1t(0)
        phase_A2t(0)
        phase_C(0, 0, 2)
        bo = phase_B(0)

        # final-phase tiles (tag reuse: layer tiles dead by first use)
        jfb = wi["fb"]
        jft = wi["ft"]
        jbb = wi["p1bb"]
        fl_b = fpool.tile([128, NBT, 2, D], BF, tag="tl_b")   # [lmp|hb1a]
        xtm_bf = fpool.tile([128, NTT, D], BF, tag="xt")
        htr_bf = fpool.tile([128, NTT, D], BF, tag="tl_t")
        mpz_bf = fpool.tile([128, NBT, D], BF, tag="xb")
        hball = fpool.tile([128, NBT, D], BF, tag="xbT")
        hbn_all = fpool.tile([128, NBT, 128], BF, tag="xtT")
        htn_T = fpool.tile([128, TP], BF, tag="tb8")

        def F1b_bacteria():
            with tc.tile_pool(name="psF1b", bufs=4, space="PSUM") as mmF:
                for nt in range(NBT):
                    ps = mmF.tile([128, 2, 256], F32, tag="mmb")
                    for kc in range(2):
                        nc.tensor.matmul(ps[:], xbT[:, kc, nt, :],
                                         wc[:, ds(jfb + 2 * kc, 2), :],
                                         start=kc == 0, stop=kc == 1)
                    cpy(fl_b[:, nt, :, :], ps[:])

        def F1b_traits():
            with tc.tile_pool(name="psF1t", bufs=4, space="PSUM") as mmF:
                for tt_ in range(NTT):
                    ps = mmF.tile([128, 512], F32, tag="mmt")
                    for kc in range(2):
                        nc.tensor.matmul(ps[:], xtT[:, kc, tt_, :],
                                         wc[:, ds(jft + 2 * kc, 2), :],
                                         start=kc == 0, stop=kc == 1)
                    cpy(xtm_bf[:, tt_, :], ps[:, 0:256])
                    ln_relu(htr_bf[:, tt_, :], ps[:, 256:512])

        for i in range(L):
            phase_C(i, 2, NBG)
            if i + 1 < L:
                phase_A1b(i + 1)
                phase_A2b(i + 1)
                phase_D(i, bo)
                phase_A1t(i + 1)
                phase_A2t(i + 1)
                phase_C(i + 1, 0, 2)
                bo = phase_B(i + 1)
            else:
                # last layer: overlap D's epilogues with the bacteria-side
                # head matmuls (which only need xb/C(L-1))
                phase_A1b(9)
                F1b_bacteria()
                phase_D(i, bo)
                bpool_cm.__exit__(None, None, None)
                phase_A1t(9)
                F1b_traits()

        def _f3_stage1(g, trS, mmS1):
            """mpz tiles of group g -> transpose -> p1 -> relu-LN -> hball."""
            lo = g * GRP
            hi = min(lo + GRP, NBT)
            for nt0 in range(lo, hi, 2):
                pst = trS.tile([128, 2, 2, 128], BF, tag="tr")
                for k in range(2):
                    for kcc in range(2):
                        nc.tensor.transpose(pst[:, kcc, k, :],
                                            mpz_bf[:, nt0 + k, ts(kcc, 128)],
                                            ident[:])
                mpTp = hpool.tile([128, 2, 2, 128], BF, tag="htT")
                cpy(mpTp[:], pst[:])
                for k in range(2):
                    nt = nt0 + k
                    ps = mmS1.tile([128, D], F32, tag="mm")
                    for kc in range(2):
                        nc.tensor.matmul(ps[:], mpTp[:, kc, k, :],
                                         wc[:, jbb + kc, :],
                                         start=kc == 0, stop=kc == 1)
                    hv = epool.tile([128, D], F32, tag="cbv")
                    nc.vector.scalar_tensor_tensor(hv[:], ps[:], 1.0,
                                                   fl_b[:, nt, 1, :],
                                                   ALU.mult, ALU.add)
                    ln_relu(hball[:, nt, :], hv[:])

        simrec = {}

        def _f3_stage2(p, trP, mmP):
            """hball pair p -> transpose -> p2 -> hbn_all (UNnormalized;
            the 1/||hb|| row scale rides the sim output copy instead,
            keeping the reciprocal off the critical chain)."""
            nt0 = 2 * p
            psh = trP.tile([128, 2, 2, 128], BF, tag="tr")
            for k in range(2):
                for kcc in range(2):
                    nc.tensor.transpose(psh[:, kcc, k, :],
                                        hball[:, nt0 + k, ts(kcc, 128)],
                                        ident[:])
            hbTp = hpool.tile([128, 2, 2, 128], BF, tag="htT")
            cpy(hbTp[:], psh[:])
            hns = []
            for k in range(2):
                ps2 = mmP.tile([128, 128], F32, tag="mm2")
                for kc in range(2):
                    nc.tensor.matmul(ps2[:], hbTp[:, kc, k, :],
                                     w128[:, kc, :],
                                     start=kc == 0, stop=kc == 1)
                simrec[nt0 + k] = l2_rec(ps2[:])
                hn = hpool.tile([128, 128], BF, tag="hn")
                cpy(hn[:], ps2[:])
                hns.append(hn)
            psn = trP.tile([128, 2, 128], BF, tag="trn", bufs=2)
            for k in range(2):
                nc.tensor.transpose(psn[:, k, :], hns[k][:], ident[:])
            cpy(hbn_all[:, nt0:nt0 + 2, :], psn[:])

        def _f1c_pair(p, trp, mmp):
            """trait head pair p: htr -> transpose -> p2 -> l2 -> htn_T."""
            t0 = 2 * p
            pst = trp.tile([128, 2, 2, 128], BF, tag="tr")
            for k in range(2):
                for kcc in range(2):
                    nc.tensor.transpose(pst[:, kcc, k, :],
                                        htr_bf[:, t0 + k, ts(kcc, 128)],
                                        ident[:])
            htTp = hpool.tile([128, 2, 2, 128], BF, tag="htT")
            cpy(htTp[:], pst[:])
            hns = []
            for k in range(2):
                ps2 = mmp.tile([128, 128], F32, tag="mm2")
                for kc in range(2):
                    nc.tensor.matmul(ps2[:], htTp[:, kc, k, :],
                                     w128[:, 2 + kc, :],
                                     start=kc == 0, stop=kc == 1)
                rec = l2_rec(ps2[:], scale=temp)
                hn = hpool.tile([128, 128], BF, tag="hn")
                nc.vector.tensor_scalar_mul(hn[:], ps2[:], rec[:])
                hns.append(hn)
            psn = trp.tile([128, 2, 2, 128], BF, tag="tr")
            for k in range(2):
                nc.tensor.transpose(psn[:, 0, k, :], hns[k][:], ident[:])
            cpy(htn_T[:, ds(t0 * 128, 256)], psn[:, 0, :, :])

        # ---- F2 (metapath agg) fused with F3 stage 1 + F1c trait head ----
        with tc.tile_pool(name="psF2", bufs=5, space="PSUM") as mmZ, \
             tc.tile_pool(name="psS1t", bufs=2, space="PSUM") as trS, \
             tc.tile_pool(name="psF1c", bufs=1, space="PSUM") as mmH:
            for g in range(NBG):
                ntiles = min(GRP, NBT - g * GRP)
                pss = [mmZ.tile([128, D], F32, tag="mm", name=f"zps{g}_{k}")
                       for k in range(ntiles)]
                at = spool.tile([128, NTT, GW], F8, tag="at")
                nc.sync.dma_start(at[:], Amp_d[g])
                for tck in range(NTT):
                    for k in range(ntiles):
                        nc.tensor.matmul(pss[k][:], at[:, tck, ts(k, 128)],
                                         xtm_bf[:, tck, :],
                                         start=tck == 0, stop=tck == NTT - 1)
                zvg = epool.tile([128, GRP, D], F32, tag="s1g", bufs=2)
                for k in range(ntiles):
                    nt = g * GRP + k
                    nc.vector.scalar_tensor_tensor(zvg[:, k, :], pss[k][:],
                                                   1.0, fl_b[:, nt, 0, :],
                                                   ALU.mult, ALU.add)
                ln_z_group(zvg, ntiles,
                           [mpz_bf[:, g * GRP + k, :]
                            for k in range(ntiles)])
                _f1c_pair(g, trS, mmH)
                # stage 1 for the previous group's tiles (mpz ready)
                if g > 0:
                    _f3_stage1(g - 1, trS, mmZ)
            _f3_stage1(NBG - 1, trS, mmZ)

        # ---- F3 stage 2 (p2 head + normalize) fused with sim writeout ----
        with tc.tile_pool(name="psS2", bufs=2, space="PSUM") as mmP, \
             tc.tile_pool(name="psS2t", bufs=2, space="PSUM") as trP, \
             tc.tile_pool(name="psSim", bufs=2, space="PSUM") as mmS:
            def _sim_tile(nt):
                for s2 in range(2):
                    ob = hpool.tile([128, 1024], BF, tag="ob", bufs=2)
                    for s in range(2):
                        pso = mmS.tile([128, 512], F32, tag="sim")
                        nc.tensor.matmul(
                            pso[:], hbn_all[:, nt, :],
                            htn_T[:, ds(s2 * 1024 + s * 512, 512)],
                            start=True, stop=True)
                        scpy(ob[:, ts(s, 512)], pso[:], simrec[nt][:])
                    nc.sync.dma_start(sim_d[nt][:, ts(s2, 1024)], ob[:])

            # sim tiles lag stage-2 by one pair so their PE work fills
            # the gaps left by stage-2's scalar/vector latency chains
            for p in range(NBP):
                _f3_stage2(p, trP, mmP)
                if p > 0:
                    _sim_tile(2 * p - 2)
                    _sim_tile(2 * p - 1)
            _sim_tile(NBT - 2)
            _sim_tile(NBT - 1)

    nc.compile()
    return nc


def kernel(**inputs):
    in_maps, meta = _prep(inputs)
    nc = build_program(meta)
    res = run_bass_kernel_spmd(nc, in_maps, core_ids=list(range(NC)))
    sim = np.empty((N_B, N_T), np.float32)
    for c in range(NC):
        shard = np.asarray(res.results[c]["simO"], np.float32).reshape(BP, TP)
        sim[c * B_SH:(c + 1) * B_SH] = shard[:B_SH, :N_T]
    if meta["simb"] != 0.0:
        sim += np.float32(meta["simb"])
    return sim


# revision 81
# speedup vs baseline: 1.0356x; 1.0356x over previous
"""Trainium2 Bass kernel for nn_HCMGNNBasedMetaPathModel (v4).

Bacteria rows sharded over 8 cores (3750 -> padded 3840); trait side and
weights replicated. Edge segment ops are dense adjacency matmuls with
EXACT fp8e4m3 edge counts; the 1/max(deg,1) mean normalization is
applied post-matmul in f32.

v4 over v3 (957us -> ~805us):
 - phases B and C use fp8 DoubleRow matmuls (both operands fp8): the
   aggregation features (tb, lin_l-transformed tt) are cast to fp8e4m3,
   paired along the contraction dim with the fp8 adjacency tiles.
   Numerically validated: adds <0.1% to the final max-rel error.
 - layer pipeline reordered: C(i), A1b(i+1), A2b(i+1), D(i),
   A1t(i+1), A2t(i+1), B(i+1)+AR trigger.  The AllReduce of layer i
   (~40-55us fixed latency) completes under C(i)+A1b+A2b; D's
   vector/scalar epilogues overlap A1t/A2t and B's DMA-bound stretch.
 - epilogues use LN/l2 per-row scale invariance twice: the deg-scaled
   lin_r term is matmul-accumulated into the aggregation PSUM via the
   identity (so no cbv STT), and LN(u/||u||+res) == LN(u+||u||*res)
   (so no reciprocal).  Per-group batching of the [128,1]-wide stat
   ops (sqrt/recip/mean*inv) cuts small-op overhead ~4x.
 - final phase fully pipelined: F1b bacteria head overlaps D(2); F2
   metapath groups fused with the F3 p1-head stage and the F1c trait
   head; F3 p2-head keeps hb UNnormalized and applies 1/||hb|| on the
   sim output copy; sim matmuls+writeout lag one pair so the 15MB
   output DMA spreads across the whole tail.
 - Abt stream: fp8 pair tiles split into halves on two DMA queues
   (sync+gpsimd) with a 5-deep pool; fp8 casts round-robin over
   gpsimd/vector/scalar.
"""
import contextlib
import sys

for _p in ("/opt/trn_rl_repo",):
    if _p not in sys.path:
        sys.path.insert(0, _p)

import numpy as np
import ml_dtypes

import concourse.bass as bass
import concourse.tile as tile
from concourse import bacc, mybir
from concourse.bass_utils import run_bass_kernel_spmd

BF16 = ml_dtypes.bfloat16
FP8 = ml_dtypes.float8_e4m3
F32 = mybir.dt.float32
BF = mybir.dt.bfloat16
F8 = mybir.dt.float8e4
AF = mybir.ActivationFunctionType
ALU = mybir.AluOpType
DR = mybir.MatmulPerfMode.DoubleRow
ts, ds = bass.ts, bass.ds

N_B, N_T, D, L, M = 30000, 2000, 256, 3, 2
NC = 8
B_SH = 3750          # real bacteria rows per core
BP = 3840            # padded bacteria rows per core
NBT = BP // 128      # 30 node tiles
NBP = NBT // 2       # 15 node-tile pairs
TP = 2048            # padded trait rows
NTT = TP // 128      # 16 trait tiles
GRP = 4              # bacteria tiles per aggregation group
NBG = 8              # ceil(30/4) groups
GW = GRP * 128       # 512 group width (last group: 2 real tiles + pad)
LN_EPS = 1e-5
NW = 12 * 3 + 10     # wc entries

# ---------------------------------------------------------------------------
# Host-side preprocessing
# ---------------------------------------------------------------------------


def _counts(src, dst, n_dst, n_src):
    """A[d, s] = #edges (s->d) as float32; plus per-dst degree."""
    idx = dst.astype(np.int64) * n_src + src.astype(np.int64)
    A = np.bincount(idx, minlength=n_dst * n_src).astype(np.float32)
    A = A.reshape(n_dst, n_src)
    deg = np.bincount(dst.astype(np.int64), minlength=n_dst).astype(np.float32)
    return A, deg


def _to_fp8_exact(A, what):
    A8 = A.astype(FP8)
    assert np.array_equal(A8.astype(np.float32), A), f"{what} not fp8-exact"
    return A8


def _prep(inp):
    f32 = np.float32
    emb_b = np.asarray(inp["emb_b"], f32)
    emb_t = np.asarray(inp["emb_t"], f32)

    A_tb, deg_b = _counts(np.asarray(inp["src_tb"]), np.asarray(inp["dst_tb"]),
                          N_B, N_T)
    A_bt, deg_t = _counts(np.asarray(inp["src_bt"]), np.asarray(inp["dst_bt"]),
                          N_T, N_B)
    mpw = np.asarray(inp["mp_w"], np.float64)
    e = np.exp(mpw - mpw.max())
    w = e / e.sum()
    sw = float(w.sum())
    mp_adj = np.asarray(inp["mp_adj"], f32)
    A_mp = (w[0] * mp_adj[0].astype(np.float64) +
            w[1] * mp_adj[1].astype(np.float64)).astype(f32)

    xb0 = np.zeros((NC, NBT, 128, D), BF16)
    xb0.reshape(NC, BP, D)[:, :B_SH] = emb_b.reshape(NC, B_SH, D).astype(BF16)
    xt0 = np.zeros((NTT, 128, D), BF16)
    xt0.reshape(TP, D)[:N_T] = emb_t.astype(BF16)

    BPG = NBG * GW  # 4096 padded for group layout

    def shard_T(A):  # [N_B, N_T] -> per-core [NBG, 128, NTT, GW] trait-major
        out = np.zeros((NC, NBG, 128, NTT, GW), f32)
        for c in range(NC):
            blk = np.zeros((TP, BPG), f32)
            blk[:N_T, :B_SH] = A[c * B_SH:(c + 1) * B_SH].T
            out[c] = blk.reshape(NTT, 128, NBG, GW).transpose(2, 1, 0, 3)
        return out

    At8 = _to_fp8_exact(shard_T(A_tb), "A_tb counts")
    Amp8 = _to_fp8_exact(shard_T(A_mp), "A_mp")
    # Abt in node-tile PAIRS for DoubleRow: [NBP, 128, 2, TP]
    Abt8 = np.zeros((NC, NBP, 128, 2, TP), FP8)
    for c in range(NC):
        blk = np.zeros((BP, TP), f32)
        blk[:B_SH, :N_T] = A_bt[:, c * B_SH:(c + 1) * B_SH].T
        Abt8[c] = _to_fp8_exact(
            blk.reshape(NBP, 2, 128, TP).transpose(0, 2, 1, 3), "A_bt counts")

    # degree normalizers: deg_real = max(deg, 1) (exact f32 ints).
    # The kernel exploits LN/l2 per-row scale invariance: instead of
    # l2(agg/deg + lr) it computes l2(agg + deg*lr), so only deg_real
    # is needed.
    degR = np.maximum(deg_b, 1.0)
    degbR = np.zeros((NC, 128, NBT), f32)
    for c in range(NC):
        v = np.ones(BP, f32)
        v[:B_SH] = degR[c * B_SH:(c + 1) * B_SH]
        degbR[c] = v.reshape(NBT, 128).T
    dtr = np.ones(TP, f32)
    dtr[:N_T] = np.maximum(deg_t, 1.0)
    degtR = np.ascontiguousarray(dtr.reshape(NTT, 128).T)

    # ---- weights: gamma folds; all biases must be zero ----
    for nm in ("bt_b", "bt_t", "bl_b", "bl_t", "mpb", "bp1b", "bp1t",
               "bp2b", "bp2t", "lnb_b", "lnb_t", "mplnb", "plnbb", "plnbt"):
        assert not np.any(np.asarray(inp[nm])), f"{nm} must be zero"
    plngb = np.asarray(inp["plngb"], f32)
    plngt = np.asarray(inp["plngt"], f32)
    assert (plngb > 0).all() and (plngt > 0).all()

    Wt_b = np.asarray(inp["Wt_b"], f32)
    Wt_t = np.asarray(inp["Wt_t"], f32)
    Wl_b = np.asarray(inp["Wl_b"], f32)
    Wr_b = np.asarray(inp["Wr_b"], f32)
    Wl_t = np.asarray(inp["Wl_t"], f32)
    Wr_t = np.asarray(inp["Wr_t"], f32)
    lng_b = np.asarray(inp["lng_b"], f32)
    lng_t = np.asarray(inp["lng_t"], f32)

    wlist = []

    def addW(*WTs):
        """for kc in (0,1): for each WT: append WT[kc*128:(kc+1)*128]."""
        base = len(wlist)
        for kc in range(2):
            for WT in WTs:
                wlist.append(np.ascontiguousarray(
                    WT[kc * 128:(kc + 1) * 128]).astype(BF16))
        return base

    wi = {}
    g_b = np.ones(D, f32)
    g_t = np.ones(D, f32)
    for i in range(L):
        WtTb = Wt_b[i].T * g_b[:, None]
        WcTb = (Wr_b[i] @ Wt_b[i]).T * g_b[:, None]
        WtTt = Wt_t[i].T * g_t[:, None]
        WctT = (Wl_b[i] @ Wt_t[i]).T * g_t[:, None]
        WcTt = (Wr_t[i] @ Wt_t[i]).T * g_t[:, None]
        wi[("b", i)] = addW(WtTb, WcTb)          # stride 2 per kc
        wi[("t", i)] = addW(WtTt, WcTt, WctT)    # [tt, lrt, ttl] per kc
        wi[("wl", i)] = addW(Wl_t[i].T)          # stride 1 per kc
        g_b, g_t = lng_b[i], lng_t[i]

    mpW = np.asarray(inp["mpW"], f32)
    g_mp = np.asarray(inp["mplng"], f32)
    Wp1b = np.asarray(inp["Wp1b"], f32)
    Wp1t = np.asarray(inp["Wp1t"], f32)
    wi["fb"] = addW(sw * (mpW.T * g_b[:, None]), Wp1b[:, :D].T * g_b[:, None])
    wi["ft"] = addW(mpW.T * g_t[:, None], Wp1t.T * g_t[:, None])
    wi["p1bb"] = addW(Wp1b[:, D:].T * g_mp[:, None])
    Wc = np.stack(wlist)
    assert Wc.shape[0] == NW, Wc.shape

    W2b = (np.asarray(inp["Wp2b"], f32) * plngb).T
    W2t = (np.asarray(inp["Wp2t"], f32) * plngt).T
    w128 = np.stack([
        np.ascontiguousarray(W2b[:128]).astype(BF16),
        np.ascontiguousarray(W2b[128:]).astype(BF16),
        np.ascontiguousarray(W2t[:128]).astype(BF16),
        np.ascontiguousarray(W2t[128:]).astype(BF16),
    ])

    ident = np.eye(128, dtype=f32).astype(BF16)
    temp = float(np.asarray(inp["temperature"]).reshape(-1)[0])
    simb = float(np.asarray(inp["sim_bias"]).reshape(-1)[0])

    shared = dict(xt0=xt0, Wc=Wc, W128=w128, degt=degtR, ident=ident)
    in_maps = []
    for c in range(NC):
        m = dict(shared)
        m["xb0"] = np.ascontiguousarray(xb0[c])
        m["At8"] = np.ascontiguousarray(At8[c])
        m["Abt8"] = np.ascontiguousarray(Abt8[c])
        m["Amp8"] = np.ascontiguousarray(Amp8[c])
        m["degb"] = np.ascontiguousarray(degbR[c])
        in_maps.append(m)
    meta = dict(wi=wi, temp=temp, simb=simb)
    return in_maps, meta


# ---------------------------------------------------------------------------
# Device program
# ---------------------------------------------------------------------------


def build_program(meta):
    nc = bacc.Bacc("TRN2", target_bir_lowering=False, debug=False,
                   num_devices=NC)
    wi = meta["wi"]
    temp = meta["temp"]

    xb0_d = nc.dram_tensor("xb0", [NBT, 128, D], BF, kind="ExternalInput")
    xt0_d = nc.dram_tensor("xt0", [NTT, 128, D], BF, kind="ExternalInput")
    At_d = nc.dram_tensor("At8", [NBG, 128, NTT, GW], F8, kind="ExternalInput")
    Abt_d = nc.dram_tensor("Abt8", [NBP, 128, 2, TP], F8,
                           kind="ExternalInput")
    Amp_d = nc.dram_tensor("Amp8", [NBG, 128, NTT, GW], F8,
                           kind="ExternalInput")
    Wc_d = nc.dram_tensor("Wc", [NW, 128, D], BF, kind="ExternalInput")
    W128_d = nc.dram_tensor("W128", [4, 128, 128], BF, kind="ExternalInput")
    degb_d = nc.dram_tensor("degb", [128, NBT], F32, kind="ExternalInput")
    degt_d = nc.dram_tensor("degt", [128, NTT], F32, kind="ExternalInput")
    id_d = nc.dram_tensor("ident", [128, 128], BF, kind="ExternalInput")
    sim_d = nc.dram_tensor("simO", [NBT, 128, TP], BF, kind="ExternalOutput")

    with tile.TileContext(nc) as tc, contextlib.ExitStack() as ctx:
        cpool = ctx.enter_context(tc.tile_pool(name="const", bufs=1))
        fpool = ctx.enter_context(tc.tile_pool(name="feat", bufs=1))
        spool = ctx.enter_context(tc.tile_pool(name="at_stream", bufs=3))
        epool = ctx.enter_context(tc.tile_pool(name="epi", bufs=3))
        hpool = ctx.enter_context(tc.tile_pool(name="hbf", bufs=3))
        qpool = ctx.enter_context(tc.tile_pool(name="sq", bufs=1))
        tpool = ctx.enter_context(tc.tile_pool(name="tiny", bufs=24))
        ppool = ctx.enter_context(tc.tile_pool(name="pscr", bufs=2))
        dpool = ctx.enter_context(tc.tile_pool(name="dram", bufs=2,
                                               space="DRAM"))

        # ---- persistent features first (layer-0 transposes need them);
        # chunked so the first transposes start almost immediately ----
        ident = cpool.tile([128, 128], BF)
        nc.sync.dma_start(ident[:], id_d[:])
        wc = cpool.tile([128, NW, D], BF)
        nc.sync.dma_start(wc[:, 0:12, :],
                          Wc_d[0:12].rearrange("n p d -> p n d"))
        xb = fpool.tile([128, NBT, D], BF, tag="xb")
        for t0 in range(0, NBT, 2):
            nc.sync.dma_start(xb[:, t0:t0 + 2, :],
                              xb0_d[t0:t0 + 2].rearrange("n p d -> p n d"))
        xt = fpool.tile([128, NTT, D], BF, tag="xt")
        for t0 in range(0, NTT, 4):
            nc.gpsimd.dma_start(xt[:, t0:t0 + 4, :],
                                xt0_d[t0:t0 + 4].rearrange("n p d -> p n d"))

        # ---- constants (stream under the first transposes) ----
        for j0 in range(12, NW, 12):
            j1 = min(j0 + 12, NW)
            nc.sync.dma_start(wc[:, j0:j1, :],
                              Wc_d[j0:j1].rearrange("n p d -> p n d"))
        w128 = cpool.tile([128, 4, 128], BF)
        nc.sync.dma_start(w128[:], W128_d.rearrange("n p d -> p n d"))
        degb = cpool.tile([128, NBT], F32)
        nc.sync.dma_start(degb[:], degb_d[:])
        degt = cpool.tile([128, NTT], F32)
        nc.sync.dma_start(degt[:], degt_d[:])
        epsb = cpool.tile([128, 1], F32, name="epsb")
        nc.gpsimd.memset(epsb[:], LN_EPS)
        eps24 = cpool.tile([128, 1], F32, name="eps24")
        nc.gpsimd.memset(eps24[:], 1e-24)

        # feature tiles: [tb|lrb], [tt|lrt], fp8 copies for DoubleRow
        tl_b = fpool.tile([128, NBT, 2, D], BF, tag="tl_b")
        tl_t = fpool.tile([128, NTT, 2, D], BF, tag="tl_t")
        tb8 = fpool.tile([128, NBT, 2, 128], F8, tag="tb8")
        tt8 = fpool.tile([128, NTT, D], F8, tag="tt8")
        xbT = fpool.tile([128, 2, NBT, 128], BF, tag="xbT")
        xtT = fpool.tile([128, 2, NTT, 128], BF, tag="xtT")

        # engine alternation for plain psum->sbuf copies
        _alt = [0]

        def cpy(dst, src):
            _alt[0] ^= 1
            (nc.vector.tensor_copy if _alt[0] else nc.scalar.copy)(dst, src)

        # 3-way rotation for sbuf->sbuf fp8 casts (gpsimd is slow alone)
        _c3 = [0]

        def cast3(dst, src):
            _c3[0] = (_c3[0] + 1) % 3
            eng = (nc.gpsimd, nc.vector, nc.scalar)[_c3[0]]
            (eng.copy if eng is nc.scalar else eng.tensor_copy)(dst, src)

        def scpy(dst, src, scale_ap):
            """psum->sbuf copy with per-partition scale, alternating."""
            _alt[0] ^= 1
            if _alt[0]:
                nc.vector.tensor_scalar_mul(dst, src, scale_ap)
            else:
                nc.scalar.activation(dst, src, AF.Copy, scale=scale_ap)

        def transpose_into(dst, src_tile, n_tiles, trp):
            """dst [128, 2, n_tiles, 128] <- per-tile transposes of
            src_tile [128, n_tiles, 256]; two node tiles per psum buf,
            one merged copy per pair."""
            for nt0 in range(0, n_tiles, 2):
                ps = trp.tile([128, 2, 2, 128], BF, tag="tr")
                for k in range(2):
                    for kcc in range(2):
                        nc.tensor.transpose(ps[:, kcc, k, :],
                                            src_tile[:, nt0 + k, ts(kcc, 128)],
                                            ident[:])
                cpy(dst[:, :, nt0:nt0 + 2, :], ps[:])

        def ln_z(out_ap, s1_ap):
            """out = normalize(s1) along free dim (gamma folded downstream).
            The 256-wide apply alternates between vector and scalar."""
            st6 = tpool.tile([128, 6], F32, tag="st6")
            nc.vector.bn_stats(st6[:], s1_ap)
            mv = tpool.tile([128, 2], F32, tag="mv")
            nc.vector.bn_aggr(mv[:], st6[:])
            std = tpool.tile([128, 1], F32, tag="std")
            nc.scalar.activation(std[:], mv[:, 1:2], AF.Sqrt, bias=epsb[:])
            inv = tpool.tile([128, 1], F32, tag="inv")
            nc.vector.reciprocal(inv[:], std[:])
            _alt[0] ^= 1
            if _alt[0]:
                mi = tpool.tile([128, 1], F32, tag="mi")
                nc.scalar.activation(mi[:], mv[:, 0:1], AF.Copy, scale=inv[:])
                nc.vector.tensor_scalar(out_ap, s1_ap, inv[:], mi[:],
                                        ALU.mult, ALU.subtract)
            else:
                nmi = tpool.tile([128, 1], F32, tag="mi")
                nc.vector.tensor_scalar(nmi[:], mv[:, 0:1], inv[:], -1.0,
                                        ALU.mult, ALU.mult)
                nc.scalar.activation(out_ap, s1_ap, AF.Identity, bias=nmi[:],
                                     scale=inv[:])

        def l2_rec(v_ap, scale=None):
            """[128,1] 1/sqrt(||v||^2+1e-24) per row, optionally * scale."""
            ssq = tpool.tile([128, 1], F32, tag="ssq")
            scr = qpool.tile([128, D], F32, tag="sq")
            nc.scalar.activation(scr[:, :v_ap.shape[-1]], v_ap, AF.Square,
                                 accum_out=ssq[:])
            nrm = tpool.tile([128, 1], F32, tag="l2n")
            nc.scalar.activation(nrm[:], ssq[:], AF.Sqrt, bias=eps24[:])
            rec = tpool.tile([128, 1], F32, tag="l2r")
            nc.vector.reciprocal(rec[:], nrm[:])
            if scale is not None:
                nc.scalar.mul(rec[:], rec[:], scale)
            return rec

        def ln_z_group(s1g, n, out_aps):
            """Batched LN over n windows s1g[:, k, :]: the [128,1]-ish
            stat ops run once per group instead of once per tile."""
            st6g = tpool.tile([128, GRP, 6], F32, tag="st6g", bufs=4)
            for k in range(n):
                nc.vector.bn_stats(st6g[:, k, :], s1g[:, k, :])
            mv4 = tpool.tile([128, GRP, 2], F32, tag="mv4", bufs=4)
            for k in range(n):
                nc.vector.bn_aggr(mv4[:, k, :], st6g[:, k, :])
            stdg = tpool.tile([128, GRP], F32, tag="stdg", bufs=4)
            nc.scalar.activation(stdg[:, 0:n], mv4[:, 0:n, 1], AF.Sqrt,
                                 bias=epsb[:])
            invg = tpool.tile([128, GRP], F32, tag="invg", bufs=4)
            nc.vector.reciprocal(invg[:, 0:n], stdg[:, 0:n])
            mig = tpool.tile([128, GRP], F32, tag="mig", bufs=4)
            nc.vector.tensor_tensor(mig[:, 0:n], mv4[:, 0:n, 0],
                                    invg[:, 0:n], ALU.mult)
            nmig = tpool.tile([128, GRP], F32, tag="nmig", bufs=4)
            nc.vector.tensor_scalar_mul(nmig[:, 0:n], mig[:, 0:n], -1.0)
            for k in range(n):
                _alt[0] ^= 1
                if _alt[0]:
                    nc.vector.tensor_scalar(out_aps[k], s1g[:, k, :],
                                            invg[:, k:k + 1], mig[:, k:k + 1],
                                            ALU.mult, ALU.subtract)
                else:
                    nc.scalar.activation(out_aps[k], s1g[:, k, :],
                                         AF.Identity, bias=nmig[:, k:k + 1],
                                         scale=invg[:, k:k + 1])

        def sage_epi_group(pss, res_aps, out_aps):
            """Batched: out = LN(l2(u) + res) per tile, u in PSUM
            (deg-scaled lr was matmul-accumulated via the identity).
            Uses LN's per-row scale invariance: LN(u/||u|| + res) ==
            LN(u + ||u||*res), so no reciprocal on the l2 side."""
            n = len(pss)
            ssqg = tpool.tile([128, GRP], F32, tag="ssqg", bufs=4)
            for k in range(n):
                scr = qpool.tile([128, D], F32, tag="sq")
                nc.scalar.activation(scr[:], pss[k], AF.Square,
                                     accum_out=ssqg[:, k:k + 1])
            nrmg = tpool.tile([128, GRP], F32, tag="nrmg", bufs=4)
            nc.scalar.activation(nrmg[:, 0:n], ssqg[:, 0:n], AF.Sqrt,
                                 bias=eps24[:])
            s1g = epool.tile([128, GRP, D], F32, tag="s1g", bufs=2)
            for k in range(n):
                nc.vector.scalar_tensor_tensor(s1g[:, k, :], res_aps[k],
                                               nrmg[:, k:k + 1], pss[k],
                                               ALU.mult, ALU.add)
            ln_z_group(s1g, n, out_aps)

        def ln_relu(out_ap, h_ap):
            """out = relu(normalize(h)) (gamma>0 folded downstream)."""
            st6 = tpool.tile([128, 6], F32, tag="st6")
            nc.vector.bn_stats(st6[:], h_ap)
            mv = tpool.tile([128, 2], F32, tag="mv")
            nc.vector.bn_aggr(mv[:], st6[:])
            std = tpool.tile([128, 1], F32, tag="std")
            nc.scalar.activation(std[:], mv[:, 1:2], AF.Sqrt, bias=epsb[:])
            inv = tpool.tile([128, 1], F32, tag="inv")
            nc.vector.reciprocal(inv[:], std[:])
            nmi = tpool.tile([128, 1], F32, tag="nmi")
            nc.vector.tensor_scalar(nmi[:], mv[:, 0:1], inv[:], -1.0,
                                    ALU.mult, ALU.mult)
            nc.scalar.activation(out_ap, h_ap, AF.Relu, bias=nmi[:],
                                 scale=inv[:])

        # ---------------- phase builders ----------------
        def phase_A1b(i):
            with tc.tile_pool(name=f"psAtb{i}", bufs=2, space="PSUM") as trA:
                transpose_into(xbT, xb, NBT, trA)

        def phase_A1t(i):
            with tc.tile_pool(name=f"psAtt{i}", bufs=2, space="PSUM") as trA:
                transpose_into(xtT, xt, NTT, trA)

        def phase_A2b(i):
            jb = wi[("b", i)]
            with tc.tile_pool(name=f"psA{i}", bufs=3, space="PSUM") as mmA:
                for nt in range(NBT):
                    ps = mmA.tile([128, 2, 256], F32, tag="mmb")
                    for kc in range(2):
                        nc.tensor.matmul(ps[:], xbT[:, kc, nt, :],
                                         wc[:, ds(jb + 2 * kc, 2), :],
                                         start=kc == 0, stop=kc == 1)
                    cpy(tl_b[:, nt, 0, :], ps[:, 0, :])
                    scpy(tl_b[:, nt, 1, :], ps[:, 1, :], degb[:, nt:nt + 1])
                    cast3(tb8[:, nt, :, :], tl_b[:, nt, 0, :])

        def phase_A2t(i):
            jt = wi[("t", i)]
            with tc.tile_pool(name=f"psAt{i}", bufs=3, space="PSUM") as mmA:
                for tt_ in range(NTT):
                    ps = mmA.tile([128, 2, 256], F32, tag="mmt", bufs=2)
                    ps2 = mmA.tile([128, 256], F32, tag="mmt2", bufs=2)
                    for kc in range(2):
                        # [tt|lrt] one 512-wide group; ttl separate bank
                        nc.tensor.matmul(ps[:], xtT[:, kc, tt_, :],
                                         wc[:, ds(jt + 3 * kc, 2), :],
                                         start=kc == 0, stop=kc == 1)
                        nc.tensor.matmul(ps2[:], xtT[:, kc, tt_, :],
                                         wc[:, jt + 3 * kc + 2, :],
                                         start=kc == 0, stop=kc == 1)
                    cpy(tl_t[:, tt_, 0, :], ps[:, 0, :])
                    scpy(tl_t[:, tt_, 1, :], ps[:, 1, :], degt[:, tt_:tt_ + 1])
                    cpy(tt8[:, tt_, :], ps2[:])

        # Abt stream pool: lives only through the layer pipeline, freed
        # before the final-phase pools are first used.
        bpool_cm = tc.tile_pool(name="abt_stream", bufs=3)
        bpool = bpool_cm.__enter__()

        def phase_B(i):
            """partial_t via DoubleRow fp8; trigger AllReduce."""
            pscr = ppool.tile([128, 2, TP], BF, tag="pscr")
            with tc.tile_pool(name=f"psB{i}", bufs=1, space="PSUM") as ptp:
                pt = [ptp.tile([128, TP], F32, tag=f"pt{dh}",
                               name=f"pt{i}_{dh}") for dh in range(2)]
                for cp in range(NBP):
                    for sh in range(2):
                        ab = bpool.tile([128, 2, TP // 2], F8, tag="abt",
                                        bufs=5)
                        (nc.sync if sh == 0 else nc.gpsimd).dma_start(
                            ab[:], Abt_d[cp][:, :, ts(sh, TP // 2)])
                        for dh in range(2):
                            for s in range(2):
                                nc.tensor.matmul(
                                    pt[dh][:, ts(2 * sh + s, 512)],
                                    tb8[:, ds(2 * cp, 2), dh, :],
                                    ab[:, :, ts(s, 512)],
                                    start=cp == 0, stop=cp == NBP - 1,
                                    perf_mode=DR)
                nc.vector.tensor_copy(pscr[:, 0, :], pt[0][:])
                nc.scalar.copy(pscr[:, 1, :], pt[1][:])
            bounce_in = dpool.tile([128, 2, TP], BF, tag="bin")
            bounce_out = dpool.tile([128, 2, TP], BF, tag="bout",
                                    addr_space="Shared")
            nc.scalar.dma_start(bounce_in[:], pscr[:])
            nc.gpsimd.collective_compute(
                "AllReduce", ALU.add, replica_groups=[list(range(NC))],
                ins=[bounce_in.opt()], outs=[bounce_out.opt()])
            return bounce_out

        def phase_C(i, glo=0, ghi=NBG):
            with tc.tile_pool(name=f"psC{i}_{glo}", bufs=6,
                              space="PSUM") as mmC:
                for g in range(glo, ghi):
                    ntiles = min(GRP, NBT - g * GRP)
                    pss = [mmC.tile([128, D], F32, tag="mm",
                                    name=f"cps{i}_{g}_{k}")
                           for k in range(ntiles)]
                    at = spool.tile([128, NTT, GW], F8, tag="at")
                    nc.sync.dma_start(at[:], At_d[g])
                    for t2 in range(NTT // 2):
                        for k in range(ntiles):
                            nc.tensor.matmul(pss[k][:],
                                             at[:, ds(2 * t2, 2), ts(k, 128)],
                                             tt8[:, ds(2 * t2, 2), :],
                                             start=t2 == 0, stop=False,
                                             perf_mode=DR)
                    for k in range(ntiles):
                        nt = g * GRP + k
                        nc.tensor.matmul(pss[k][:], ident[:],
                                         tl_b[:, nt, 1, :],
                                         start=False, stop=True)
                    nts = [g * GRP + k for k in range(ntiles)]
                    sage_epi_group([pss[k][:] for k in range(ntiles)],
                                   [tl_b[:, nt, 0, :] for nt in nts],
                                   [xb[:, nt, :] for nt in nts])

        def phase_D(i, bounce_out):
            jl = wi[("wl", i)]
            pm = ppool.tile([128, 2, TP], BF, tag="pscr")
            nc.sync.dma_start(pm[:], bounce_out[:])
            with tc.tile_pool(name=f"psD{i}", bufs=5, space="PSUM") as mmD:
                for g in range(NTT // GRP):
                    pss = []
                    for k in range(GRP):
                        tt_ = g * GRP + k
                        ps = mmD.tile([128, D], F32, tag="mm")
                        for kc in range(2):
                            nc.tensor.matmul(ps[:], pm[:, kc, ts(tt_, 128)],
                                             wc[:, jl + kc, :],
                                             start=kc == 0, stop=False)
                        nc.tensor.matmul(ps[:], ident[:], tl_t[:, tt_, 1, :],
                                         start=False, stop=True)
                        pss.append(ps)
                    tts = [g * GRP + k for k in range(GRP)]
                    sage_epi_group([p_[:] for p_ in pss],
                                   [tl_t[:, t_, 0, :] for t_ in tts],
                                   [xt[:, t_, :] for t_ in tts])

        # ================= main pipeline =================
        phase_A1b(0)
        phase_A2b(0)
        phase_A1t(0)
        phase_A2t(0)
        bo = phase_B(0)

        # final-phase tiles (tag reuse: layer tiles dead by first use)
        jfb = wi["fb"]
        jft = wi["ft"]
        jbb = wi["p1bb"]
        fl_b = fpool.tile([128, NBT, 2, D], BF, tag="tl_b")   # [lmp|hb1a]
        xtm_bf = fpool.tile([128, NTT, D], BF, tag="xt")
        htr_bf = fpool.tile([128, NTT, D], BF, tag="tl_t")
        mpz_bf = fpool.tile([128, NBT, D], BF, tag="xb")
        hball = fpool.tile([128, NBT, D], BF, tag="xbT")
        hbn_all = fpool.tile([128, NBT, 128], BF, tag="xtT")
        htn_T = fpool.tile([128, TP], BF, tag="tb8")

        def F1b_bacteria():
            with tc.tile_pool(name="psF1b", bufs=4, space="PSUM") as mmF:
                for nt in range(NBT):
                    ps = mmF.tile([128, 2, 256], F32, tag="mmb")
                    for kc in range(2):
                        nc.tensor.matmul(ps[:], xbT[:, kc, nt, :],
                                         wc[:, ds(jfb + 2 * kc, 2), :],
                                         start=kc == 0, stop=kc == 1)
                    cpy(fl_b[:, nt, :, :], ps[:])

        def F1b_traits():
            with tc.tile_pool(name="psF1t", bufs=4, space="PSUM") as mmF:
                for tt_ in range(NTT):
                    ps = mmF.tile([128, 512], F32, tag="mmt")
                    for kc in range(2):
                        nc.tensor.matmul(ps[:], xtT[:, kc, tt_, :],
                                         wc[:, ds(jft + 2 * kc, 2), :],
                                         start=kc == 0, stop=kc == 1)
                    cpy(xtm_bf[:, tt_, :], ps[:, 0:256])
                    ln_relu(htr_bf[:, tt_, :], ps[:, 256:512])

        for i in range(L):
            phase_C(i)
            if i + 1 < L:
                phase_A1b(i + 1)
                phase_A2b(i + 1)
                phase_D(i, bo)
                phase_A1t(i + 1)
                phase_A2t(i + 1)
                bo = phase_B(i + 1)
            else:
                # last layer: overlap D's epilogues with the bacteria-side
                # head matmuls (which only need xb/C(L-1))
                phase_A1b(9)
                F1b_bacteria()
                phase_D(i, bo)
                bpool_cm.__exit__(None, None, None)
                phase_A1t(9)
                F1b_traits()

        def _f3_stage1(g, trS, mmS1):
            """mpz tiles of group g -> transpose -> p1 -> relu-LN -> hball."""
            lo = g * GRP
            hi = min(lo + GRP, NBT)
            for nt0 in range(lo, hi, 2):
                pst = trS.tile([128, 2, 2, 128], BF, tag="tr")
                for k in range(2):
                    for kcc in range(2):
                        nc.tensor.transpose(pst[:, kcc, k, :],
                                            mpz_bf[:, nt0 + k, ts(kcc, 128)],
                                            ident[:])
                mpTp = hpool.tile([128, 2, 2, 128], BF, tag="htT")
                cpy(mpTp[:], pst[:])
                for k in range(2):
                    nt = nt0 + k
                    ps = mmS1.tile([128, D], F32, tag="mm")
                    for kc in range(2):
                        nc.tensor.matmul(ps[:], mpTp[:, kc, k, :],
                                         wc[:, jbb + kc, :],
                                         start=kc == 0, stop=kc == 1)
                    hv = epool.tile([128, D], F32, tag="cbv")
                    nc.vector.scalar_tensor_tensor(hv[:], ps[:], 1.0,
                                                   fl_b[:, nt, 1, :],
                                                   ALU.mult, ALU.add)
                    ln_relu(hball[:, nt, :], hv[:])

        simrec = {}

        def _f3_stage2(p, trP, mmP):
            """hball pair p -> transpose -> p2 -> hbn_all (UNnormalized;
            the 1/||hb|| row scale rides the sim output copy instead,
            keeping the reciprocal off the critical chain)."""
            nt0 = 2 * p
            psh = trP.tile([128, 2, 2, 128], BF, tag="tr")
            for k in range(2):
                for kcc in range(2):
                    nc.tensor.transpose(psh[:, kcc, k, :],
                                        hball[:, nt0 + k, ts(kcc, 128)],
                                        ident[:])
            hbTp = hpool.tile([128, 2, 2, 128], BF, tag="htT")
            cpy(hbTp[:], psh[:])
            hns = []
            for k in range(2):
                ps2 = mmP.tile([128, 128], F32, tag="mm2")
                for kc in range(2):
                    nc.tensor.matmul(ps2[:], hbTp[:, kc, k, :],
                                     w128[:, kc, :],
                                     start=kc == 0, stop=kc == 1)
                simrec[nt0 + k] = l2_rec(ps2[:])
                hn = hpool.tile([128, 128], BF, tag="hn")
                cpy(hn[:], ps2[:])
                hns.append(hn)
            psn = trP.tile([128, 2, 128], BF, tag="trn", bufs=2)
            for k in range(2):
                nc.tensor.transpose(psn[:, k, :], hns[k][:], ident[:])
            cpy(hbn_all[:, nt0:nt0 + 2, :], psn[:])

        def _f1c_pair(p, trp, mmp):
            """trait head pair p: htr -> transpose -> p2 -> l2 -> htn_T."""
            t0 = 2 * p
            pst = trp.tile([128, 2, 2, 128], BF, tag="tr")
            for k in range(2):
                for kcc in range(2):
                    nc.tensor.transpose(pst[:, kcc, k, :],
                                        htr_bf[:, t0 + k, ts(kcc, 128)],
                                        ident[:])
            htTp = hpool.tile([128, 2, 2, 128], BF, tag="htT")
            cpy(htTp[:], pst[:])
            hns = []
            for k in range(2):
                ps2 = mmp.tile([128, 128], F32, tag="mm2")
                for kc in range(2):
                    nc.tensor.matmul(ps2[:], htTp[:, kc, k, :],
                                     w128[:, 2 + kc, :],
                                     start=kc == 0, stop=kc == 1)
                rec = l2_rec(ps2[:], scale=temp)
                hn = hpool.tile([128, 128], BF, tag="hn")
                nc.vector.tensor_scalar_mul(hn[:], ps2[:], rec[:])
                hns.append(hn)
            psn = trp.tile([128, 2, 2, 128], BF, tag="tr")
            for k in range(2):
                nc.tensor.transpose(psn[:, 0, k, :], hns[k][:], ident[:])
            cpy(htn_T[:, ds(t0 * 128, 256)], psn[:, 0, :, :])

        # ---- F2 (metapath agg) fused with F3 stage 1 + F1c trait head ----
        with tc.tile_pool(name="psF2", bufs=5, space="PSUM") as mmZ, \
             tc.tile_pool(name="psS1t", bufs=2, space="PSUM") as trS, \
             tc.tile_pool(name="psF1c", bufs=1, space="PSUM") as mmH:
            for g in range(NBG):
                ntiles = min(GRP, NBT - g * GRP)
                pss = [mmZ.tile([128, D], F32, tag="mm", name=f"zps{g}_{k}")
                       for k in range(ntiles)]
                at = spool.tile([128, NTT, GW], F8, tag="at")
                nc.sync.dma_start(at[:], Amp_d[g])
                for tck in range(NTT):
                    for k in range(ntiles):
                        nc.tensor.matmul(pss[k][:], at[:, tck, ts(k, 128)],
                                         xtm_bf[:, tck, :],
                                         start=tck == 0, stop=tck == NTT - 1)
                zvg = epool.tile([128, GRP, D], F32, tag="s1g", bufs=2)
                for k in range(ntiles):
                    nt = g * GRP + k
                    nc.vector.scalar_tensor_tensor(zvg[:, k, :], pss[k][:],
                                                   1.0, fl_b[:, nt, 0, :],
                                                   ALU.mult, ALU.add)
                ln_z_group(zvg, ntiles,
                           [mpz_bf[:, g * GRP + k, :]
                            for k in range(ntiles)])
                _f1c_pair(g, trS, mmH)
                # stage 1 for the previous group's tiles (mpz ready)
                if g > 0:
                    _f3_stage1(g - 1, trS, mmZ)
            _f3_stage1(NBG - 1, trS, mmZ)

        # ---- F3 stage 2 (p2 head + normalize) fused with sim writeout ----
        with tc.tile_pool(name="psS2", bufs=2, space="PSUM") as mmP, \
             tc.tile_pool(name="psS2t", bufs=2, space="PSUM") as trP, \
             tc.tile_pool(name="psSim", bufs=2, space="PSUM") as mmS:
            def _sim_tile(nt):
                for s2 in range(2):
                    ob = hpool.tile([128, 1024], BF, tag="ob", bufs=2)
                    for s in range(2):
                        pso = mmS.tile([128, 512], F32, tag="sim")
                        nc.tensor.matmul(
                            pso[:], hbn_all[:, nt, :],
                            htn_T[:, ds(s2 * 1024 + s * 512, 512)],
                            start=True, stop=True)
                        scpy(ob[:, ts(s, 512)], pso[:], simrec[nt][:])
                    nc.sync.dma_start(sim_d[nt][:, ts(s2, 1024)], ob[:])

            # sim tiles lag stage-2 by one pair so their PE work fills
            # the gaps left by stage-2's scalar/vector latency chains
            for p in range(NBP):
                _f3_stage2(p, trP, mmP)
                if p > 0:
                    _sim_tile(2 * p - 2)
                    _sim_tile(2 * p - 1)
            _sim_tile(NBT - 2)
            _sim_tile(NBT - 1)

    nc.compile()
    return nc


def kernel(**inputs):
    in_maps, meta = _prep(inputs)
    nc = build_program(meta)
    res = run_bass_kernel_spmd(nc, in_maps, core_ids=list(range(NC)))
    sim = np.empty((N_B, N_T), np.float32)
    for c in range(NC):
        shard = np.asarray(res.results[c]["simO"], np.float32).reshape(BP, TP)
        sim[c * B_SH:(c + 1) * B_SH] = shard[:B_SH, :N_T]
    if meta["simb"] != 0.0:
        sim += np.float32(meta["simb"])
    return sim


# revision 82
# speedup vs baseline: 1.0371x; 1.0015x over previous
"""Trainium2 Bass kernel for nn_HCMGNNBasedMetaPathModel (v4).

Bacteria rows sharded over 8 cores (3750 -> padded 3840); trait side and
weights replicated. Edge segment ops are dense adjacency matmuls with
EXACT fp8e4m3 edge counts; the 1/max(deg,1) mean normalization is
applied post-matmul in f32.

v4 over v3 (957us -> ~805us):
 - phases B and C use fp8 DoubleRow matmuls (both operands fp8): the
   aggregation features (tb, lin_l-transformed tt) are cast to fp8e4m3,
   paired along the contraction dim with the fp8 adjacency tiles.
   Numerically validated: adds <0.1% to the final max-rel error.
 - layer pipeline reordered: C(i), A1b(i+1), A2b(i+1), D(i),
   A1t(i+1), A2t(i+1), B(i+1)+AR trigger.  The AllReduce of layer i
   (~40-55us fixed latency) completes under C(i)+A1b+A2b; D's
   vector/scalar epilogues overlap A1t/A2t and B's DMA-bound stretch.
 - epilogues use LN/l2 per-row scale invariance twice: the deg-scaled
   lin_r term is matmul-accumulated into the aggregation PSUM via the
   identity (so no cbv STT), and LN(u/||u||+res) == LN(u+||u||*res)
   (so no reciprocal).  Per-group batching of the [128,1]-wide stat
   ops (sqrt/recip/mean*inv) cuts small-op overhead ~4x.
 - final phase fully pipelined: F1b bacteria head overlaps D(2); F2
   metapath groups fused with the F3 p1-head stage and the F1c trait
   head; F3 p2-head keeps hb UNnormalized and applies 1/||hb|| on the
   sim output copy; sim matmuls+writeout lag one pair so the 15MB
   output DMA spreads across the whole tail.
 - Abt stream: fp8 pair tiles split into halves on two DMA queues
   (sync+gpsimd) with a 5-deep pool; fp8 casts round-robin over
   gpsimd/vector/scalar.
"""
import contextlib
import sys

for _p in ("/opt/trn_rl_repo",):
    if _p not in sys.path:
        sys.path.insert(0, _p)

import numpy as np
import ml_dtypes

import concourse.bass as bass
import concourse.tile as tile
from concourse import bacc, mybir
from concourse.bass_utils import run_bass_kernel_spmd

BF16 = ml_dtypes.bfloat16
FP8 = ml_dtypes.float8_e4m3
F32 = mybir.dt.float32
BF = mybir.dt.bfloat16
F8 = mybir.dt.float8e4
AF = mybir.ActivationFunctionType
ALU = mybir.AluOpType
DR = mybir.MatmulPerfMode.DoubleRow
ts, ds = bass.ts, bass.ds

N_B, N_T, D, L, M = 30000, 2000, 256, 3, 2
NC = 8
B_SH = 3750          # real bacteria rows per core
BP = 3840            # padded bacteria rows per core
NBT = BP // 128      # 30 node tiles
NBP = NBT // 2       # 15 node-tile pairs
TP = 2048            # padded trait rows
NTT = TP // 128      # 16 trait tiles
GRP = 4              # bacteria tiles per aggregation group
NBG = 8              # ceil(30/4) groups
GW = GRP * 128       # 512 group width (last group: 2 real tiles + pad)
LN_EPS = 1e-5
NW = 12 * 3 + 10     # wc entries

# ---------------------------------------------------------------------------
# Host-side preprocessing
# ---------------------------------------------------------------------------


def _counts(src, dst, n_dst, n_src):
    """A[d, s] = #edges (s->d) as float32; plus per-dst degree."""
    idx = dst.astype(np.int64) * n_src + src.astype(np.int64)
    A = np.bincount(idx, minlength=n_dst * n_src).astype(np.float32)
    A = A.reshape(n_dst, n_src)
    deg = np.bincount(dst.astype(np.int64), minlength=n_dst).astype(np.float32)
    return A, deg


def _to_fp8_exact(A, what):
    A8 = A.astype(FP8)
    assert np.array_equal(A8.astype(np.float32), A), f"{what} not fp8-exact"
    return A8


def _prep(inp):
    f32 = np.float32
    emb_b = np.asarray(inp["emb_b"], f32)
    emb_t = np.asarray(inp["emb_t"], f32)

    A_tb, deg_b = _counts(np.asarray(inp["src_tb"]), np.asarray(inp["dst_tb"]),
                          N_B, N_T)
    A_bt, deg_t = _counts(np.asarray(inp["src_bt"]), np.asarray(inp["dst_bt"]),
                          N_T, N_B)
    mpw = np.asarray(inp["mp_w"], np.float64)
    e = np.exp(mpw - mpw.max())
    w = e / e.sum()
    sw = float(w.sum())
    mp_adj = np.asarray(inp["mp_adj"], f32)
    A_mp = (w[0] * mp_adj[0].astype(np.float64) +
            w[1] * mp_adj[1].astype(np.float64)).astype(f32)

    xb0 = np.zeros((NC, NBT, 128, D), BF16)
    xb0.reshape(NC, BP, D)[:, :B_SH] = emb_b.reshape(NC, B_SH, D).astype(BF16)
    xt0 = np.zeros((NTT, 128, D), BF16)
    xt0.reshape(TP, D)[:N_T] = emb_t.astype(BF16)

    BPG = NBG * GW  # 4096 padded for group layout

    def shard_T(A):  # [N_B, N_T] -> per-core [NBG, 128, NTT, GW] trait-major
        out = np.zeros((NC, NBG, 128, NTT, GW), f32)
        for c in range(NC):
            blk = np.zeros((TP, BPG), f32)
            blk[:N_T, :B_SH] = A[c * B_SH:(c + 1) * B_SH].T
            out[c] = blk.reshape(NTT, 128, NBG, GW).transpose(2, 1, 0, 3)
        return out

    At8 = _to_fp8_exact(shard_T(A_tb), "A_tb counts")
    Amp8 = _to_fp8_exact(shard_T(A_mp), "A_mp")
    # Abt in node-tile PAIRS for DoubleRow: [NBP, 128, 2, TP]
    Abt8 = np.zeros((NC, NBP, 128, 2, TP), FP8)
    for c in range(NC):
        blk = np.zeros((BP, TP), f32)
        blk[:B_SH, :N_T] = A_bt[:, c * B_SH:(c + 1) * B_SH].T
        Abt8[c] = _to_fp8_exact(
            blk.reshape(NBP, 2, 128, TP).transpose(0, 2, 1, 3), "A_bt counts")

    # degree normalizers: deg_real = max(deg, 1) (exact f32 ints).
    # The kernel exploits LN/l2 per-row scale invariance: instead of
    # l2(agg/deg + lr) it computes l2(agg + deg*lr), so only deg_real
    # is needed.
    degR = np.maximum(deg_b, 1.0)
    degbR = np.zeros((NC, 128, NBT), f32)
    for c in range(NC):
        v = np.ones(BP, f32)
        v[:B_SH] = degR[c * B_SH:(c + 1) * B_SH]
        degbR[c] = v.reshape(NBT, 128).T
    dtr = np.ones(TP, f32)
    dtr[:N_T] = np.maximum(deg_t, 1.0)
    degtR = np.ascontiguousarray(dtr.reshape(NTT, 128).T)

    # ---- weights: gamma folds; all biases must be zero ----
    for nm in ("bt_b", "bt_t", "bl_b", "bl_t", "mpb", "bp1b", "bp1t",
               "bp2b", "bp2t", "lnb_b", "lnb_t", "mplnb", "plnbb", "plnbt"):
        assert not np.any(np.asarray(inp[nm])), f"{nm} must be zero"
    plngb = np.asarray(inp["plngb"], f32)
    plngt = np.asarray(inp["plngt"], f32)
    assert (plngb > 0).all() and (plngt > 0).all()

    Wt_b = np.asarray(inp["Wt_b"], f32)
    Wt_t = np.asarray(inp["Wt_t"], f32)
    Wl_b = np.asarray(inp["Wl_b"], f32)
    Wr_b = np.asarray(inp["Wr_b"], f32)
    Wl_t = np.asarray(inp["Wl_t"], f32)
    Wr_t = np.asarray(inp["Wr_t"], f32)
    lng_b = np.asarray(inp["lng_b"], f32)
    lng_t = np.asarray(inp["lng_t"], f32)

    wlist = []

    def addW(*WTs):
        """for kc in (0,1): for each WT: append WT[kc*128:(kc+1)*128]."""
        base = len(wlist)
        for kc in range(2):
            for WT in WTs:
                wlist.append(np.ascontiguousarray(
                    WT[kc * 128:(kc + 1) * 128]).astype(BF16))
        return base

    wi = {}
    g_b = np.ones(D, f32)
    g_t = np.ones(D, f32)
    for i in range(L):
        WtTb = Wt_b[i].T * g_b[:, None]
        WcTb = (Wr_b[i] @ Wt_b[i]).T * g_b[:, None]
        WtTt = Wt_t[i].T * g_t[:, None]
        WctT = (Wl_b[i] @ Wt_t[i]).T * g_t[:, None]
        WcTt = (Wr_t[i] @ Wt_t[i]).T * g_t[:, None]
        wi[("b", i)] = addW(WtTb, WcTb)          # stride 2 per kc
        wi[("t", i)] = addW(WtTt, WcTt, WctT)    # [tt, lrt, ttl] per kc
        wi[("wl", i)] = addW(Wl_t[i].T)          # stride 1 per kc
        g_b, g_t = lng_b[i], lng_t[i]

    mpW = np.asarray(inp["mpW"], f32)
    g_mp = np.asarray(inp["mplng"], f32)
    Wp1b = np.asarray(inp["Wp1b"], f32)
    Wp1t = np.asarray(inp["Wp1t"], f32)
    wi["fb"] = addW(sw * (mpW.T * g_b[:, None]), Wp1b[:, :D].T * g_b[:, None])
    wi["ft"] = addW(mpW.T * g_t[:, None], Wp1t.T * g_t[:, None])
    wi["p1bb"] = addW(Wp1b[:, D:].T * g_mp[:, None])
    Wc = np.stack(wlist)
    assert Wc.shape[0] == NW, Wc.shape

    W2b = (np.asarray(inp["Wp2b"], f32) * plngb).T
    W2t = (np.asarray(inp["Wp2t"], f32) * plngt).T
    w128 = np.stack([
        np.ascontiguousarray(W2b[:128]).astype(BF16),
        np.ascontiguousarray(W2b[128:]).astype(BF16),
        np.ascontiguousarray(W2t[:128]).astype(BF16),
        np.ascontiguousarray(W2t[128:]).astype(BF16),
    ])

    ident = np.eye(128, dtype=f32).astype(BF16)
    temp = float(np.asarray(inp["temperature"]).reshape(-1)[0])
    simb = float(np.asarray(inp["sim_bias"]).reshape(-1)[0])

    shared = dict(xt0=xt0, Wc=Wc, W128=w128, degt=degtR, ident=ident)
    in_maps = []
    for c in range(NC):
        m = dict(shared)
        m["xb0"] = np.ascontiguousarray(xb0[c])
        m["At8"] = np.ascontiguousarray(At8[c])
        m["Abt8"] = np.ascontiguousarray(Abt8[c])
        m["Amp8"] = np.ascontiguousarray(Amp8[c])
        m["degb"] = np.ascontiguousarray(degbR[c])
        in_maps.append(m)
    meta = dict(wi=wi, temp=temp, simb=simb)
    return in_maps, meta


# ---------------------------------------------------------------------------
# Device program
# ---------------------------------------------------------------------------


def build_program(meta):
    nc = bacc.Bacc("TRN2", target_bir_lowering=False, debug=False,
                   num_devices=NC)
    wi = meta["wi"]
    temp = meta["temp"]

    xb0_d = nc.dram_tensor("xb0", [NBT, 128, D], BF, kind="ExternalInput")
    xt0_d = nc.dram_tensor("xt0", [NTT, 128, D], BF, kind="ExternalInput")
    At_d = nc.dram_tensor("At8", [NBG, 128, NTT, GW], F8, kind="ExternalInput")
    Abt_d = nc.dram_tensor("Abt8", [NBP, 128, 2, TP], F8,
                           kind="ExternalInput")
    Amp_d = nc.dram_tensor("Amp8", [NBG, 128, NTT, GW], F8,
                           kind="ExternalInput")
    Wc_d = nc.dram_tensor("Wc", [NW, 128, D], BF, kind="ExternalInput")
    W128_d = nc.dram_tensor("W128", [4, 128, 128], BF, kind="ExternalInput")
    degb_d = nc.dram_tensor("degb", [128, NBT], F32, kind="ExternalInput")
    degt_d = nc.dram_tensor("degt", [128, NTT], F32, kind="ExternalInput")
    id_d = nc.dram_tensor("ident", [128, 128], BF, kind="ExternalInput")
    sim_d = nc.dram_tensor("simO", [NBT, 128, TP], BF, kind="ExternalOutput")

    with tile.TileContext(nc) as tc, contextlib.ExitStack() as ctx:
        cpool = ctx.enter_context(tc.tile_pool(name="const", bufs=1))
        fpool = ctx.enter_context(tc.tile_pool(name="feat", bufs=1))
        spool = ctx.enter_context(tc.tile_pool(name="at_stream", bufs=3))
        epool = ctx.enter_context(tc.tile_pool(name="epi", bufs=3))
        hpool = ctx.enter_context(tc.tile_pool(name="hbf", bufs=3))
        qpool = ctx.enter_context(tc.tile_pool(name="sq", bufs=1))
        tpool = ctx.enter_context(tc.tile_pool(name="tiny", bufs=24))
        ppool = ctx.enter_context(tc.tile_pool(name="pscr", bufs=2))
        dpool = ctx.enter_context(tc.tile_pool(name="dram", bufs=2,
                                               space="DRAM"))

        # ---- persistent features first (layer-0 transposes need them);
        # chunked so the first transposes start almost immediately ----
        ident = cpool.tile([128, 128], BF)
        nc.sync.dma_start(ident[:], id_d[:])
        wc = cpool.tile([128, NW, D], BF)
        nc.sync.dma_start(wc[:, 0:12, :],
                          Wc_d[0:12].rearrange("n p d -> p n d"))
        xb = fpool.tile([128, NBT, D], BF, tag="xb")
        for t0 in range(0, NBT, 2):
            nc.sync.dma_start(xb[:, t0:t0 + 2, :],
                              xb0_d[t0:t0 + 2].rearrange("n p d -> p n d"))
        xt = fpool.tile([128, NTT, D], BF, tag="xt")
        for t0 in range(0, NTT, 4):
            nc.gpsimd.dma_start(xt[:, t0:t0 + 4, :],
                                xt0_d[t0:t0 + 4].rearrange("n p d -> p n d"))

        # ---- constants (stream under the first transposes) ----
        for j0 in range(12, NW, 12):
            j1 = min(j0 + 12, NW)
            nc.sync.dma_start(wc[:, j0:j1, :],
                              Wc_d[j0:j1].rearrange("n p d -> p n d"))
        w128 = cpool.tile([128, 4, 128], BF)
        nc.sync.dma_start(w128[:], W128_d.rearrange("n p d -> p n d"))
        degb = cpool.tile([128, NBT], F32)
        nc.sync.dma_start(degb[:], degb_d[:])
        degt = cpool.tile([128, NTT], F32)
        nc.sync.dma_start(degt[:], degt_d[:])
        epsb = cpool.tile([128, 1], F32, name="epsb")
        nc.gpsimd.memset(epsb[:], LN_EPS)
        eps24 = cpool.tile([128, 1], F32, name="eps24")
        nc.gpsimd.memset(eps24[:], 1e-24)

        # feature tiles: [tb|lrb], [tt|lrt], fp8 copies for DoubleRow
        tl_b = fpool.tile([128, NBT, 2, D], BF, tag="tl_b")
        tl_t = fpool.tile([128, NTT, 2, D], BF, tag="tl_t")
        tb8 = fpool.tile([128, NBT, 2, 128], F8, tag="tb8")
        tt8 = fpool.tile([128, NTT, D], F8, tag="tt8")
        xbT = fpool.tile([128, 2, NBT, 128], BF, tag="xbT")
        xtT = fpool.tile([128, 2, NTT, 128], BF, tag="xtT")

        # engine alternation for plain psum->sbuf copies
        _alt = [0]

        def cpy(dst, src):
            _alt[0] ^= 1
            (nc.vector.tensor_copy if _alt[0] else nc.scalar.copy)(dst, src)

        # 3-way rotation for sbuf->sbuf fp8 casts (gpsimd is slow alone)
        _c3 = [0]

        def cast3(dst, src):
            _c3[0] = (_c3[0] + 1) % 3
            eng = (nc.gpsimd, nc.vector, nc.scalar)[_c3[0]]
            (eng.copy if eng is nc.scalar else eng.tensor_copy)(dst, src)

        def scpy(dst, src, scale_ap):
            """psum->sbuf copy with per-partition scale, alternating."""
            _alt[0] ^= 1
            if _alt[0]:
                nc.vector.tensor_scalar_mul(dst, src, scale_ap)
            else:
                nc.scalar.activation(dst, src, AF.Copy, scale=scale_ap)

        def transpose_into(dst, src_tile, n_tiles, trp):
            """dst [128, 2, n_tiles, 128] <- per-tile transposes of
            src_tile [128, n_tiles, 256]; two node tiles per psum buf,
            one merged copy per pair."""
            for nt0 in range(0, n_tiles, 2):
                ps = trp.tile([128, 2, 2, 128], BF, tag="tr")
                for k in range(2):
                    for kcc in range(2):
                        nc.tensor.transpose(ps[:, kcc, k, :],
                                            src_tile[:, nt0 + k, ts(kcc, 128)],
                                            ident[:])
                cpy(dst[:, :, nt0:nt0 + 2, :], ps[:])

        def ln_z(out_ap, s1_ap):
            """out = normalize(s1) along free dim (gamma folded downstream).
            The 256-wide apply alternates between vector and scalar."""
            st6 = tpool.tile([128, 6], F32, tag="st6")
            nc.vector.bn_stats(st6[:], s1_ap)
            mv = tpool.tile([128, 2], F32, tag="mv")
            nc.vector.bn_aggr(mv[:], st6[:])
            std = tpool.tile([128, 1], F32, tag="std")
            nc.scalar.activation(std[:], mv[:, 1:2], AF.Sqrt, bias=epsb[:])
            inv = tpool.tile([128, 1], F32, tag="inv")
            nc.vector.reciprocal(inv[:], std[:])
            _alt[0] ^= 1
            if _alt[0]:
                mi = tpool.tile([128, 1], F32, tag="mi")
                nc.scalar.activation(mi[:], mv[:, 0:1], AF.Copy, scale=inv[:])
                nc.vector.tensor_scalar(out_ap, s1_ap, inv[:], mi[:],
                                        ALU.mult, ALU.subtract)
            else:
                nmi = tpool.tile([128, 1], F32, tag="mi")
                nc.vector.tensor_scalar(nmi[:], mv[:, 0:1], inv[:], -1.0,
                                        ALU.mult, ALU.mult)
                nc.scalar.activation(out_ap, s1_ap, AF.Identity, bias=nmi[:],
                                     scale=inv[:])

        def l2_rec(v_ap, scale=None):
            """[128,1] 1/sqrt(||v||^2+1e-24) per row, optionally * scale."""
            ssq = tpool.tile([128, 1], F32, tag="ssq")
            scr = qpool.tile([128, D], F32, tag="sq")
            nc.scalar.activation(scr[:, :v_ap.shape[-1]], v_ap, AF.Square,
                                 accum_out=ssq[:])
            nrm = tpool.tile([128, 1], F32, tag="l2n")
            nc.scalar.activation(nrm[:], ssq[:], AF.Sqrt, bias=eps24[:])
            rec = tpool.tile([128, 1], F32, tag="l2r")
            nc.vector.reciprocal(rec[:], nrm[:])
            if scale is not None:
                nc.scalar.mul(rec[:], rec[:], scale)
            return rec

        def ln_z_group(s1g, n, out_aps):
            """Batched LN over n windows s1g[:, k, :]: the [128,1]-ish
            stat ops run once per group instead of once per tile."""
            st6g = tpool.tile([128, GRP, 6], F32, tag="st6g", bufs=4)
            for k in range(n):
                nc.vector.bn_stats(st6g[:, k, :], s1g[:, k, :])
            mv4 = tpool.tile([128, GRP, 2], F32, tag="mv4", bufs=4)
            for k in range(n):
                nc.vector.bn_aggr(mv4[:, k, :], st6g[:, k, :])
            stdg = tpool.tile([128, GRP], F32, tag="stdg", bufs=4)
            nc.scalar.activation(stdg[:, 0:n], mv4[:, 0:n, 1], AF.Sqrt,
                                 bias=epsb[:])
            invg = tpool.tile([128, GRP], F32, tag="invg", bufs=4)
            nc.vector.reciprocal(invg[:, 0:n], stdg[:, 0:n])
            mig = tpool.tile([128, GRP], F32, tag="mig", bufs=4)
            nc.vector.tensor_tensor(mig[:, 0:n], mv4[:, 0:n, 0],
                                    invg[:, 0:n], ALU.mult)
            nmig = tpool.tile([128, GRP], F32, tag="nmig", bufs=4)
            nc.vector.tensor_scalar_mul(nmig[:, 0:n], mig[:, 0:n], -1.0)
            for k in range(n):
                _c3[0] = (_c3[0] + 1) % 3
                if _c3[0] != 0:
                    nc.vector.tensor_scalar(out_aps[k], s1g[:, k, :],
                                            invg[:, k:k + 1], mig[:, k:k + 1],
                                            ALU.mult, ALU.subtract)
                else:
                    nc.scalar.activation(out_aps[k], s1g[:, k, :],
                                         AF.Identity, bias=nmig[:, k:k + 1],
                                         scale=invg[:, k:k + 1])

        def sage_epi_group(pss, res_aps, out_aps):
            """Batched: out = LN(l2(u) + res) per tile, u in PSUM
            (deg-scaled lr was matmul-accumulated via the identity).
            Uses LN's per-row scale invariance: LN(u/||u|| + res) ==
            LN(u + ||u||*res), so no reciprocal on the l2 side."""
            n = len(pss)
            ssqg = tpool.tile([128, GRP], F32, tag="ssqg", bufs=4)
            for k in range(n):
                scr = qpool.tile([128, D], F32, tag="sq")
                nc.scalar.activation(scr[:], pss[k], AF.Square,
                                     accum_out=ssqg[:, k:k + 1])
            nrmg = tpool.tile([128, GRP], F32, tag="nrmg", bufs=4)
            nc.scalar.activation(nrmg[:, 0:n], ssqg[:, 0:n], AF.Sqrt,
                                 bias=eps24[:])
            s1g = epool.tile([128, GRP, D], F32, tag="s1g", bufs=2)
            for k in range(n):
                nc.vector.scalar_tensor_tensor(s1g[:, k, :], res_aps[k],
                                               nrmg[:, k:k + 1], pss[k],
                                               ALU.mult, ALU.add)
            ln_z_group(s1g, n, out_aps)

        def ln_relu(out_ap, h_ap):
            """out = relu(normalize(h)) (gamma>0 folded downstream)."""
            st6 = tpool.tile([128, 6], F32, tag="st6")
            nc.vector.bn_stats(st6[:], h_ap)
            mv = tpool.tile([128, 2], F32, tag="mv")
            nc.vector.bn_aggr(mv[:], st6[:])
            std = tpool.tile([128, 1], F32, tag="std")
            nc.scalar.activation(std[:], mv[:, 1:2], AF.Sqrt, bias=epsb[:])
            inv = tpool.tile([128, 1], F32, tag="inv")
            nc.vector.reciprocal(inv[:], std[:])
            nmi = tpool.tile([128, 1], F32, tag="nmi")
            nc.vector.tensor_scalar(nmi[:], mv[:, 0:1], inv[:], -1.0,
                                    ALU.mult, ALU.mult)
            nc.scalar.activation(out_ap, h_ap, AF.Relu, bias=nmi[:],
                                 scale=inv[:])

        # ---------------- phase builders ----------------
        def phase_A1b(i):
            with tc.tile_pool(name=f"psAtb{i}", bufs=2, space="PSUM") as trA:
                transpose_into(xbT, xb, NBT, trA)

        def phase_A1t(i):
            with tc.tile_pool(name=f"psAtt{i}", bufs=2, space="PSUM") as trA:
                transpose_into(xtT, xt, NTT, trA)

        def phase_A2b(i):
            jb = wi[("b", i)]
            with tc.tile_pool(name=f"psA{i}", bufs=3, space="PSUM") as mmA:
                for nt in range(NBT):
                    ps = mmA.tile([128, 2, 256], F32, tag="mmb")
                    for kc in range(2):
                        nc.tensor.matmul(ps[:], xbT[:, kc, nt, :],
                                         wc[:, ds(jb + 2 * kc, 2), :],
                                         start=kc == 0, stop=kc == 1)
                    cpy(tl_b[:, nt, 0, :], ps[:, 0, :])
                    scpy(tl_b[:, nt, 1, :], ps[:, 1, :], degb[:, nt:nt + 1])
                    nc.gpsimd.tensor_copy(tb8[:, nt, :, :],
                                          tl_b[:, nt, 0, :])

        def phase_A2t(i):
            jt = wi[("t", i)]
            with tc.tile_pool(name=f"psAt{i}", bufs=3, space="PSUM") as mmA:
                for tt_ in range(NTT):
                    ps = mmA.tile([128, 2, 256], F32, tag="mmt", bufs=2)
                    ps2 = mmA.tile([128, 256], F32, tag="mmt2", bufs=2)
                    for kc in range(2):
                        # [tt|lrt] one 512-wide group; ttl separate bank
                        nc.tensor.matmul(ps[:], xtT[:, kc, tt_, :],
                                         wc[:, ds(jt + 3 * kc, 2), :],
                                         start=kc == 0, stop=kc == 1)
                        nc.tensor.matmul(ps2[:], xtT[:, kc, tt_, :],
                                         wc[:, jt + 3 * kc + 2, :],
                                         start=kc == 0, stop=kc == 1)
                    cpy(tl_t[:, tt_, 0, :], ps[:, 0, :])
                    scpy(tl_t[:, tt_, 1, :], ps[:, 1, :], degt[:, tt_:tt_ + 1])
                    cpy(tt8[:, tt_, :], ps2[:])

        # Abt stream pool: lives only through the layer pipeline, freed
        # before the final-phase pools are first used.
        bpool_cm = tc.tile_pool(name="abt_stream", bufs=3)
        bpool = bpool_cm.__enter__()

        def phase_B(i):
            """partial_t via DoubleRow fp8; trigger AllReduce."""
            pscr = ppool.tile([128, 2, TP], BF, tag="pscr")
            with tc.tile_pool(name=f"psB{i}", bufs=1, space="PSUM") as ptp:
                pt = [ptp.tile([128, TP], F32, tag=f"pt{dh}",
                               name=f"pt{i}_{dh}") for dh in range(2)]
                for cp in range(NBP):
                    for sh in range(2):
                        ab = bpool.tile([128, 2, TP // 2], F8, tag="abt",
                                        bufs=5)
                        (nc.sync if sh == 0 else nc.gpsimd).dma_start(
                            ab[:], Abt_d[cp][:, :, ts(sh, TP // 2)])
                        for dh in range(2):
                            for s in range(2):
                                nc.tensor.matmul(
                                    pt[dh][:, ts(2 * sh + s, 512)],
                                    tb8[:, ds(2 * cp, 2), dh, :],
                                    ab[:, :, ts(s, 512)],
                                    start=cp == 0, stop=cp == NBP - 1,
                                    perf_mode=DR)
                nc.vector.tensor_copy(pscr[:, 0, :], pt[0][:])
                nc.scalar.copy(pscr[:, 1, :], pt[1][:])
            bounce_in = dpool.tile([128, 2, TP], BF, tag="bin")
            bounce_out = dpool.tile([128, 2, TP], BF, tag="bout",
                                    addr_space="Shared")
            nc.scalar.dma_start(bounce_in[:], pscr[:])
            nc.gpsimd.collective_compute(
                "AllReduce", ALU.add, replica_groups=[list(range(NC))],
                ins=[bounce_in.opt()], outs=[bounce_out.opt()])
            return bounce_out

        def phase_C(i, glo=0, ghi=NBG):
            with tc.tile_pool(name=f"psC{i}_{glo}", bufs=6,
                              space="PSUM") as mmC:
                for g in range(glo, ghi):
                    ntiles = min(GRP, NBT - g * GRP)
                    pss = [mmC.tile([128, D], F32, tag="mm",
                                    name=f"cps{i}_{g}_{k}")
                           for k in range(ntiles)]
                    at = spool.tile([128, NTT, GW], F8, tag="at")
                    nc.sync.dma_start(at[:], At_d[g])
                    for t2 in range(NTT // 2):
                        for k in range(ntiles):
                            nc.tensor.matmul(pss[k][:],
                                             at[:, ds(2 * t2, 2), ts(k, 128)],
                                             tt8[:, ds(2 * t2, 2), :],
                                             start=t2 == 0, stop=False,
                                             perf_mode=DR)
                    for k in range(ntiles):
                        nt = g * GRP + k
                        nc.tensor.matmul(pss[k][:], ident[:],
                                         tl_b[:, nt, 1, :],
                                         start=False, stop=True)
                    nts = [g * GRP + k for k in range(ntiles)]
                    sage_epi_group([pss[k][:] for k in range(ntiles)],
                                   [tl_b[:, nt, 0, :] for nt in nts],
                                   [xb[:, nt, :] for nt in nts])

        def phase_D(i, bounce_out):
            jl = wi[("wl", i)]
            pm = ppool.tile([128, 2, TP], BF, tag="pscr")
            nc.sync.dma_start(pm[:], bounce_out[:])
            with tc.tile_pool(name=f"psD{i}", bufs=5, space="PSUM") as mmD:
                for g in range(NTT // GRP):
                    pss = []
                    for k in range(GRP):
                        tt_ = g * GRP + k
                        ps = mmD.tile([128, D], F32, tag="mm")
                        for kc in range(2):
                            nc.tensor.matmul(ps[:], pm[:, kc, ts(tt_, 128)],
                                             wc[:, jl + kc, :],
                                             start=kc == 0, stop=False)
                        nc.tensor.matmul(ps[:], ident[:], tl_t[:, tt_, 1, :],
                                         start=False, stop=True)
                        pss.append(ps)
                    tts = [g * GRP + k for k in range(GRP)]
                    sage_epi_group([p_[:] for p_ in pss],
                                   [tl_t[:, t_, 0, :] for t_ in tts],
                                   [xt[:, t_, :] for t_ in tts])

        # ================= main pipeline =================
        phase_A1b(0)
        phase_A2b(0)
        phase_A1t(0)
        phase_A2t(0)
        bo = phase_B(0)

        # final-phase tiles (tag reuse: layer tiles dead by first use)
        jfb = wi["fb"]
        jft = wi["ft"]
        jbb = wi["p1bb"]
        fl_b = fpool.tile([128, NBT, 2, D], BF, tag="tl_b")   # [lmp|hb1a]
        xtm_bf = fpool.tile([128, NTT, D], BF, tag="xt")
        htr_bf = fpool.tile([128, NTT, D], BF, tag="tl_t")
        mpz_bf = fpool.tile([128, NBT, D], BF, tag="xb")
        hball = fpool.tile([128, NBT, D], BF, tag="xbT")
        hbn_all = fpool.tile([128, NBT, 128], BF, tag="xtT")
        htn_T = fpool.tile([128, TP], BF, tag="tb8")

        def F1b_bacteria():
            with tc.tile_pool(name="psF1b", bufs=4, space="PSUM") as mmF:
                for nt in range(NBT):
                    ps = mmF.tile([128, 2, 256], F32, tag="mmb")
                    for kc in range(2):
                        nc.tensor.matmul(ps[:], xbT[:, kc, nt, :],
                                         wc[:, ds(jfb + 2 * kc, 2), :],
                                         start=kc == 0, stop=kc == 1)
                    cpy(fl_b[:, nt, :, :], ps[:])

        def F1b_traits():
            with tc.tile_pool(name="psF1t", bufs=4, space="PSUM") as mmF:
                for tt_ in range(NTT):
                    ps = mmF.tile([128, 512], F32, tag="mmt")
                    for kc in range(2):
                        nc.tensor.matmul(ps[:], xtT[:, kc, tt_, :],
                                         wc[:, ds(jft + 2 * kc, 2), :],
                                         start=kc == 0, stop=kc == 1)
                    cpy(xtm_bf[:, tt_, :], ps[:, 0:256])
                    ln_relu(htr_bf[:, tt_, :], ps[:, 256:512])

        for i in range(L):
            phase_C(i)
            if i + 1 < L:
                phase_A1b(i + 1)
                phase_A2b(i + 1)
                phase_D(i, bo)
                phase_A1t(i + 1)
                phase_A2t(i + 1)
                bo = phase_B(i + 1)
            else:
                # last layer: overlap D's epilogues with the bacteria-side
                # head matmuls (which only need xb/C(L-1))
                phase_A1b(9)
                F1b_bacteria()
                phase_D(i, bo)
                bpool_cm.__exit__(None, None, None)
                phase_A1t(9)
                F1b_traits()

        def _f3_stage1(g, trS, mmS1):
            """mpz tiles of group g -> transpose -> p1 -> relu-LN -> hball."""
            lo = g * GRP
            hi = min(lo + GRP, NBT)
            for nt0 in range(lo, hi, 2):
                pst = trS.tile([128, 2, 2, 128], BF, tag="tr")
                for k in range(2):
                    for kcc in range(2):
                        nc.tensor.transpose(pst[:, kcc, k, :],
                                            mpz_bf[:, nt0 + k, ts(kcc, 128)],
                                            ident[:])
                mpTp = hpool.tile([128, 2, 2, 128], BF, tag="htT")
                cpy(mpTp[:], pst[:])
                for k in range(2):
                    nt = nt0 + k
                    ps = mmS1.tile([128, D], F32, tag="mm")
                    for kc in range(2):
                        nc.tensor.matmul(ps[:], mpTp[:, kc, k, :],
                                         wc[:, jbb + kc, :],
                                         start=kc == 0, stop=kc == 1)
                    hv = epool.tile([128, D], F32, tag="cbv")
                    nc.vector.scalar_tensor_tensor(hv[:], ps[:], 1.0,
                                                   fl_b[:, nt, 1, :],
                                                   ALU.mult, ALU.add)
                    ln_relu(hball[:, nt, :], hv[:])

        simrec = {}

        def _f3_stage2(p, trP, mmP):
            """hball pair p -> transpose -> p2 -> hbn_all (UNnormalized;
            the 1/||hb|| row scale rides the sim output copy instead,
            keeping the reciprocal off the critical chain)."""
            nt0 = 2 * p
            psh = trP.tile([128, 2, 2, 128], BF, tag="tr")
            for k in range(2):
                for kcc in range(2):
                    nc.tensor.transpose(psh[:, kcc, k, :],
                                        hball[:, nt0 + k, ts(kcc, 128)],
                                        ident[:])
            hbTp = hpool.tile([128, 2, 2, 128], BF, tag="htT")
            cpy(hbTp[:], psh[:])
            hns = []
            for k in range(2):
                ps2 = mmP.tile([128, 128], F32, tag="mm2")
                for kc in range(2):
                    nc.tensor.matmul(ps2[:], hbTp[:, kc, k, :],
                                     w128[:, kc, :],
                                     start=kc == 0, stop=kc == 1)
                simrec[nt0 + k] = l2_rec(ps2[:])
                hn = hpool.tile([128, 128], BF, tag="hn")
                cpy(hn[:], ps2[:])
                hns.append(hn)
            psn = trP.tile([128, 2, 128], BF, tag="trn", bufs=2)
            for k in range(2):
                nc.tensor.transpose(psn[:, k, :], hns[k][:], ident[:])
            cpy(hbn_all[:, nt0:nt0 + 2, :], psn[:])

        def _f1c_pair(p, trp, mmp):
            """trait head pair p: htr -> transpose -> p2 -> l2 -> htn_T."""
            t0 = 2 * p
            pst = trp.tile([128, 2, 2, 128], BF, tag="tr")
            for k in range(2):
                for kcc in range(2):
                    nc.tensor.transpose(pst[:, kcc, k, :],
                                        htr_bf[:, t0 + k, ts(kcc, 128)],
                                        ident[:])
            htTp = hpool.tile([128, 2, 2, 128], BF, tag="htT")
            cpy(htTp[:], pst[:])
            hns = []
            for k in range(2):
                ps2 = mmp.tile([128, 128], F32, tag="mm2")
                for kc in range(2):
                    nc.tensor.matmul(ps2[:], htTp[:, kc, k, :],
                                     w128[:, 2 + kc, :],
                                     start=kc == 0, stop=kc == 1)
                rec = l2_rec(ps2[:], scale=temp)
                hn = hpool.tile([128, 128], BF, tag="hn")
                nc.vector.tensor_scalar_mul(hn[:], ps2[:], rec[:])
                hns.append(hn)
            psn = trp.tile([128, 2, 2, 128], BF, tag="tr")
            for k in range(2):
                nc.tensor.transpose(psn[:, 0, k, :], hns[k][:], ident[:])
            cpy(htn_T[:, ds(t0 * 128, 256)], psn[:, 0, :, :])

        # ---- F2 (metapath agg) fused with F3 stage 1 + F1c trait head ----
        with tc.tile_pool(name="psF2", bufs=5, space="PSUM") as mmZ, \
             tc.tile_pool(name="psS1t", bufs=2, space="PSUM") as trS, \
             tc.tile_pool(name="psF1c", bufs=1, space="PSUM") as mmH:
            for g in range(NBG):
                ntiles = min(GRP, NBT - g * GRP)
                pss = [mmZ.tile([128, D], F32, tag="mm", name=f"zps{g}_{k}")
                       for k in range(ntiles)]
                at = spool.tile([128, NTT, GW], F8, tag="at")
                nc.sync.dma_start(at[:], Amp_d[g])
                for tck in range(NTT):
                    for k in range(ntiles):
                        nc.tensor.matmul(pss[k][:], at[:, tck, ts(k, 128)],
                                         xtm_bf[:, tck, :],
                                         start=tck == 0, stop=tck == NTT - 1)
                zvg = epool.tile([128, GRP, D], F32, tag="s1g", bufs=2)
                for k in range(ntiles):
                    nt = g * GRP + k
                    nc.vector.scalar_tensor_tensor(zvg[:, k, :], pss[k][:],
                                                   1.0, fl_b[:, nt, 0, :],
                                                   ALU.mult, ALU.add)
                ln_z_group(zvg, ntiles,
                           [mpz_bf[:, g * GRP + k, :]
                            for k in range(ntiles)])
                _f1c_pair(g, trS, mmH)
                # stage 1 for the previous group's tiles (mpz ready)
                if g > 0:
                    _f3_stage1(g - 1, trS, mmZ)
            _f3_stage1(NBG - 1, trS, mmZ)

        # ---- F3 stage 2 (p2 head + normalize) fused with sim writeout ----
        with tc.tile_pool(name="psS2", bufs=2, space="PSUM") as mmP, \
             tc.tile_pool(name="psS2t", bufs=2, space="PSUM") as trP, \
             tc.tile_pool(name="psSim", bufs=2, space="PSUM") as mmS:
            def _sim_tile(nt):
                for s2 in range(2):
                    ob = hpool.tile([128, 1024], BF, tag="ob", bufs=2)
                    for s in range(2):
                        pso = mmS.tile([128, 512], F32, tag="sim")
                        nc.tensor.matmul(
                            pso[:], hbn_all[:, nt, :],
                            htn_T[:, ds(s2 * 1024 + s * 512, 512)],
                            start=True, stop=True)
                        scpy(ob[:, ts(s, 512)], pso[:], simrec[nt][:])
                    nc.sync.dma_start(sim_d[nt][:, ts(s2, 1024)], ob[:])

            # sim tiles lag stage-2 by one pair so their PE work fills
            # the gaps left by stage-2's scalar/vector latency chains
            for p in range(NBP):
                _f3_stage2(p, trP, mmP)
                if p > 0:
                    _sim_tile(2 * p - 2)
                    _sim_tile(2 * p - 1)
            _sim_tile(NBT - 2)
            _sim_tile(NBT - 1)

    nc.compile()
    return nc


def kernel(**inputs):
    in_maps, meta = _prep(inputs)
    nc = build_program(meta)
    res = run_bass_kernel_spmd(nc, in_maps, core_ids=list(range(NC)))
    sim = np.empty((N_B, N_T), np.float32)
    for c in range(NC):
        shard = np.asarray(res.results[c]["simO"], np.float32).reshape(BP, TP)
        sim[c * B_SH:(c + 1) * B_SH] = shard[:B_SH, :N_T]
    if meta["simb"] != 0.0:
        sim += np.float32(meta["simb"])
    return sim


# revision 83
# speedup vs baseline: 1.0607x; 1.0228x over previous
"""Trainium2 Bass kernel for nn_HCMGNNBasedMetaPathModel (v4).

Bacteria rows sharded over 8 cores (3750 -> padded 3840); trait side and
weights replicated. Edge segment ops are dense adjacency matmuls with
EXACT fp8e4m3 edge counts; the 1/max(deg,1) mean normalization is
applied post-matmul in f32.

v4 over v3 (957us -> ~805us):
 - phases B and C use fp8 DoubleRow matmuls (both operands fp8): the
   aggregation features (tb, lin_l-transformed tt) are cast to fp8e4m3,
   paired along the contraction dim with the fp8 adjacency tiles.
   Numerically validated: adds <0.1% to the final max-rel error.
 - layer pipeline reordered: C(i), A1b(i+1), A2b(i+1), D(i),
   A1t(i+1), A2t(i+1), B(i+1)+AR trigger.  The AllReduce of layer i
   (~40-55us fixed latency) completes under C(i)+A1b+A2b; D's
   vector/scalar epilogues overlap A1t/A2t and B's DMA-bound stretch.
 - epilogues use LN/l2 per-row scale invariance twice: the deg-scaled
   lin_r term is matmul-accumulated into the aggregation PSUM via the
   identity (so no cbv STT), and LN(u/||u||+res) == LN(u+||u||*res)
   (so no reciprocal).  Per-group batching of the [128,1]-wide stat
   ops (sqrt/recip/mean*inv) cuts small-op overhead ~4x.
 - final phase fully pipelined: F1b bacteria head overlaps D(2); F2
   metapath groups fused with the F3 p1-head stage and the F1c trait
   head; F3 p2-head keeps hb UNnormalized and applies 1/||hb|| on the
   sim output copy; sim matmuls+writeout lag one pair so the 15MB
   output DMA spreads across the whole tail.
 - Abt stream: fp8 pair tiles split into halves on two DMA queues
   (sync+gpsimd) with a 5-deep pool; fp8 casts round-robin over
   gpsimd/vector/scalar.
"""
import contextlib
import sys

for _p in ("/opt/trn_rl_repo",):
    if _p not in sys.path:
        sys.path.insert(0, _p)

import numpy as np
import ml_dtypes

import concourse.bass as bass
import concourse.tile as tile
from concourse import bacc, mybir
from concourse.bass_utils import run_bass_kernel_spmd

BF16 = ml_dtypes.bfloat16
FP8 = ml_dtypes.float8_e4m3
F32 = mybir.dt.float32
BF = mybir.dt.bfloat16
F8 = mybir.dt.float8e4
AF = mybir.ActivationFunctionType
ALU = mybir.AluOpType
DR = mybir.MatmulPerfMode.DoubleRow
ts, ds = bass.ts, bass.ds

N_B, N_T, D, L, M = 30000, 2000, 256, 3, 2
NC = 8
B_SH = 3750          # real bacteria rows per core
BP = 3840            # padded bacteria rows per core
NBT = BP // 128      # 30 node tiles
NBP = NBT // 2       # 15 node-tile pairs
TP = 2048            # padded trait rows
NTT = TP // 128      # 16 trait tiles
GRP = 4              # bacteria tiles per aggregation group
NBG = 8              # ceil(30/4) groups
GW = GRP * 128       # 512 group width (last group: 2 real tiles + pad)
LN_EPS = 1e-5
NW = 12 * 3 + 10     # wc entries

# ---------------------------------------------------------------------------
# Host-side preprocessing
# ---------------------------------------------------------------------------


def _counts(src, dst, n_dst, n_src):
    """A[d, s] = #edges (s->d) as float32; plus per-dst degree."""
    idx = dst.astype(np.int64) * n_src + src.astype(np.int64)
    A = np.bincount(idx, minlength=n_dst * n_src).astype(np.float32)
    A = A.reshape(n_dst, n_src)
    deg = np.bincount(dst.astype(np.int64), minlength=n_dst).astype(np.float32)
    return A, deg


def _to_fp8_exact(A, what):
    A8 = A.astype(FP8)
    assert np.array_equal(A8.astype(np.float32), A), f"{what} not fp8-exact"
    return A8


def _prep(inp):
    f32 = np.float32
    emb_b = np.asarray(inp["emb_b"], f32)
    emb_t = np.asarray(inp["emb_t"], f32)

    A_tb, deg_b = _counts(np.asarray(inp["src_tb"]), np.asarray(inp["dst_tb"]),
                          N_B, N_T)
    A_bt, deg_t = _counts(np.asarray(inp["src_bt"]), np.asarray(inp["dst_bt"]),
                          N_T, N_B)
    mpw = np.asarray(inp["mp_w"], np.float64)
    e = np.exp(mpw - mpw.max())
    w = e / e.sum()
    sw = float(w.sum())
    mp_adj = np.asarray(inp["mp_adj"], f32)
    A_mp = (w[0] * mp_adj[0].astype(np.float64) +
            w[1] * mp_adj[1].astype(np.float64)).astype(f32)

    xb0 = np.zeros((NC, NBT, 128, D), BF16)
    xb0.reshape(NC, BP, D)[:, :B_SH] = emb_b.reshape(NC, B_SH, D).astype(BF16)
    xt0 = np.zeros((NTT, 128, D), BF16)
    xt0.reshape(TP, D)[:N_T] = emb_t.astype(BF16)

    BPG = NBG * GW  # 4096 padded for group layout

    def shard_T(A):  # [N_B, N_T] -> per-core [NBG, 128, NTT, GW] trait-major
        out = np.zeros((NC, NBG, 128, NTT, GW), f32)
        for c in range(NC):
            blk = np.zeros((TP, BPG), f32)
            blk[:N_T, :B_SH] = A[c * B_SH:(c + 1) * B_SH].T
            out[c] = blk.reshape(NTT, 128, NBG, GW).transpose(2, 1, 0, 3)
        return out

    At8 = _to_fp8_exact(shard_T(A_tb), "A_tb counts")
    Amp8 = _to_fp8_exact(shard_T(A_mp), "A_mp")
    # Abt in node-tile PAIRS for DoubleRow: [NBP, 128, 2, TP]
    Abt8 = np.zeros((NC, NBP, 128, 2, TP), FP8)
    for c in range(NC):
        blk = np.zeros((BP, TP), f32)
        blk[:B_SH, :N_T] = A_bt[:, c * B_SH:(c + 1) * B_SH].T
        Abt8[c] = _to_fp8_exact(
            blk.reshape(NBP, 2, 128, TP).transpose(0, 2, 1, 3), "A_bt counts")

    # degree normalizers: deg_real = max(deg, 1) (exact f32 ints).
    # The kernel exploits LN/l2 per-row scale invariance: instead of
    # l2(agg/deg + lr) it computes l2(agg + deg*lr), so only deg_real
    # is needed.
    degR = np.maximum(deg_b, 1.0)
    degbR = np.zeros((NC, 128, NBT), f32)
    for c in range(NC):
        v = np.ones(BP, f32)
        v[:B_SH] = degR[c * B_SH:(c + 1) * B_SH]
        degbR[c] = v.reshape(NBT, 128).T
    dtr = np.ones(TP, f32)
    dtr[:N_T] = np.maximum(deg_t, 1.0)
    degtR = np.ascontiguousarray(dtr.reshape(NTT, 128).T)

    # ---- weights: gamma folds; all biases must be zero ----
    for nm in ("bt_b", "bt_t", "bl_b", "bl_t", "mpb", "bp1b", "bp1t",
               "bp2b", "bp2t", "lnb_b", "lnb_t", "mplnb", "plnbb", "plnbt"):
        assert not np.any(np.asarray(inp[nm])), f"{nm} must be zero"
    plngb = np.asarray(inp["plngb"], f32)
    plngt = np.asarray(inp["plngt"], f32)
    assert (plngb > 0).all() and (plngt > 0).all()

    Wt_b = np.asarray(inp["Wt_b"], f32)
    Wt_t = np.asarray(inp["Wt_t"], f32)
    Wl_b = np.asarray(inp["Wl_b"], f32)
    Wr_b = np.asarray(inp["Wr_b"], f32)
    Wl_t = np.asarray(inp["Wl_t"], f32)
    Wr_t = np.asarray(inp["Wr_t"], f32)
    lng_b = np.asarray(inp["lng_b"], f32)
    lng_t = np.asarray(inp["lng_t"], f32)

    wlist = []

    def addW(*WTs):
        """for kc in (0,1): for each WT: append WT[kc*128:(kc+1)*128]."""
        base = len(wlist)
        for kc in range(2):
            for WT in WTs:
                wlist.append(np.ascontiguousarray(
                    WT[kc * 128:(kc + 1) * 128]).astype(BF16))
        return base

    wi = {}
    g_b = np.ones(D, f32)
    g_t = np.ones(D, f32)
    for i in range(L):
        WtTb = Wt_b[i].T * g_b[:, None]
        WcTb = (Wr_b[i] @ Wt_b[i]).T * g_b[:, None]
        WtTt = Wt_t[i].T * g_t[:, None]
        WctT = (Wl_b[i] @ Wt_t[i]).T * g_t[:, None]
        WcTt = (Wr_t[i] @ Wt_t[i]).T * g_t[:, None]
        wi[("b", i)] = addW(WtTb, WcTb)          # stride 2 per kc
        wi[("t", i)] = addW(WtTt, WcTt, WctT)    # [tt, lrt, ttl] per kc
        wi[("wl", i)] = addW(Wl_t[i].T)          # stride 1 per kc
        g_b, g_t = lng_b[i], lng_t[i]

    mpW = np.asarray(inp["mpW"], f32)
    g_mp = np.asarray(inp["mplng"], f32)
    Wp1b = np.asarray(inp["Wp1b"], f32)
    Wp1t = np.asarray(inp["Wp1t"], f32)
    wi["fb"] = addW(sw * (mpW.T * g_b[:, None]), Wp1b[:, :D].T * g_b[:, None])
    wi["ft"] = addW(mpW.T * g_t[:, None], Wp1t.T * g_t[:, None])
    wi["p1bb"] = addW(Wp1b[:, D:].T * g_mp[:, None])
    Wc = np.stack(wlist)
    assert Wc.shape[0] == NW, Wc.shape

    W2b = (np.asarray(inp["Wp2b"], f32) * plngb).T
    W2t = (np.asarray(inp["Wp2t"], f32) * plngt).T
    w128 = np.stack([
        np.ascontiguousarray(W2b[:128]).astype(BF16),
        np.ascontiguousarray(W2b[128:]).astype(BF16),
        np.ascontiguousarray(W2t[:128]).astype(BF16),
        np.ascontiguousarray(W2t[128:]).astype(BF16),
    ])

    ident = np.eye(128, dtype=f32).astype(BF16)
    temp = float(np.asarray(inp["temperature"]).reshape(-1)[0])
    simb = float(np.asarray(inp["sim_bias"]).reshape(-1)[0])

    shared = dict(xt0=xt0, Wc=Wc, W128=w128, degt=degtR, ident=ident)
    in_maps = []
    for c in range(NC):
        m = dict(shared)
        m["xb0"] = np.ascontiguousarray(xb0[c])
        m["At8"] = np.ascontiguousarray(At8[c])
        m["Abt8"] = np.ascontiguousarray(Abt8[c])
        m["Amp8"] = np.ascontiguousarray(Amp8[c])
        m["degb"] = np.ascontiguousarray(degbR[c])
        in_maps.append(m)
    meta = dict(wi=wi, temp=temp, simb=simb)
    return in_maps, meta


# ---------------------------------------------------------------------------
# Device program
# ---------------------------------------------------------------------------


def build_program(meta):
    nc = bacc.Bacc("TRN2", target_bir_lowering=False, debug=False,
                   num_devices=NC)
    wi = meta["wi"]
    temp = meta["temp"]

    xb0_d = nc.dram_tensor("xb0", [NBT, 128, D], BF, kind="ExternalInput")
    xt0_d = nc.dram_tensor("xt0", [NTT, 128, D], BF, kind="ExternalInput")
    At_d = nc.dram_tensor("At8", [NBG, 128, NTT, GW], F8, kind="ExternalInput")
    Abt_d = nc.dram_tensor("Abt8", [NBP, 128, 2, TP], F8,
                           kind="ExternalInput")
    Amp_d = nc.dram_tensor("Amp8", [NBG, 128, NTT, GW], F8,
                           kind="ExternalInput")
    Wc_d = nc.dram_tensor("Wc", [NW, 128, D], BF, kind="ExternalInput")
    W128_d = nc.dram_tensor("W128", [4, 128, 128], BF, kind="ExternalInput")
    degb_d = nc.dram_tensor("degb", [128, NBT], F32, kind="ExternalInput")
    degt_d = nc.dram_tensor("degt", [128, NTT], F32, kind="ExternalInput")
    id_d = nc.dram_tensor("ident", [128, 128], BF, kind="ExternalInput")
    sim_d = nc.dram_tensor("simO", [NBT, 128, TP], BF, kind="ExternalOutput")

    with tile.TileContext(nc) as tc, contextlib.ExitStack() as ctx:
        cpool = ctx.enter_context(tc.tile_pool(name="const", bufs=1))
        fpool = ctx.enter_context(tc.tile_pool(name="feat", bufs=1))
        spool = ctx.enter_context(tc.tile_pool(name="at_stream", bufs=3))
        epool = ctx.enter_context(tc.tile_pool(name="epi", bufs=3))
        hpool = ctx.enter_context(tc.tile_pool(name="hbf", bufs=3))
        qpool = ctx.enter_context(tc.tile_pool(name="sq", bufs=1))
        tpool = ctx.enter_context(tc.tile_pool(name="tiny", bufs=24))
        ppool = ctx.enter_context(tc.tile_pool(name="pscr", bufs=2))
        dpool = ctx.enter_context(tc.tile_pool(name="dram", bufs=2,
                                               space="DRAM"))

        # ---- persistent features first (layer-0 transposes need them);
        # chunked so the first transposes start almost immediately ----
        ident = cpool.tile([128, 128], BF)
        nc.sync.dma_start(ident[:], id_d[:])
        wc = cpool.tile([128, NW, D], BF)
        nc.sync.dma_start(wc[:, 0:12, :],
                          Wc_d[0:12].rearrange("n p d -> p n d"))
        xb = fpool.tile([128, NBT, D], BF, tag="xb")
        for t0 in range(0, NBT, 2):
            nc.sync.dma_start(xb[:, t0:t0 + 2, :],
                              xb0_d[t0:t0 + 2].rearrange("n p d -> p n d"))
        xt = fpool.tile([128, NTT, D], BF, tag="xt")
        for t0 in range(0, NTT, 4):
            nc.gpsimd.dma_start(xt[:, t0:t0 + 4, :],
                                xt0_d[t0:t0 + 4].rearrange("n p d -> p n d"))

        # ---- constants (stream under the first transposes) ----
        for j0 in range(12, NW, 12):
            j1 = min(j0 + 12, NW)
            nc.sync.dma_start(wc[:, j0:j1, :],
                              Wc_d[j0:j1].rearrange("n p d -> p n d"))
        w128 = cpool.tile([128, 4, 128], BF)
        nc.sync.dma_start(w128[:], W128_d.rearrange("n p d -> p n d"))
        degb = cpool.tile([128, NBT], F32)
        nc.sync.dma_start(degb[:], degb_d[:])
        degt = cpool.tile([128, NTT], F32)
        nc.sync.dma_start(degt[:], degt_d[:])
        epsb = cpool.tile([128, 1], F32, name="epsb")
        nc.gpsimd.memset(epsb[:], LN_EPS)
        eps24 = cpool.tile([128, 1], F32, name="eps24")
        nc.gpsimd.memset(eps24[:], 1e-24)

        # feature tiles: [tb|lrb], [tt|lrt], fp8 copies for DoubleRow
        tl_b = fpool.tile([128, NBT, 2, D], BF, tag="tl_b")
        tl_t = fpool.tile([128, NTT, 2, D], BF, tag="tl_t")
        tb8 = fpool.tile([128, NBT, 2, 128], F8, tag="tb8")
        tt8 = fpool.tile([128, NTT, D], F8, tag="tt8")
        xbT = fpool.tile([128, 2, NBT, 128], BF, tag="xbT")
        xtT = fpool.tile([128, 2, NTT, 128], BF, tag="xtT")

        # engine alternation for plain psum->sbuf copies
        _alt = [0]

        def cpy(dst, src):
            _alt[0] ^= 1
            (nc.vector.tensor_copy if _alt[0] else nc.scalar.copy)(dst, src)

        # 3-way rotation for sbuf->sbuf fp8 casts (gpsimd is slow alone)
        _c3 = [0]

        def cast3(dst, src):
            _c3[0] = (_c3[0] + 1) % 3
            eng = (nc.gpsimd, nc.vector, nc.scalar)[_c3[0]]
            (eng.copy if eng is nc.scalar else eng.tensor_copy)(dst, src)

        def scpy(dst, src, scale_ap):
            """psum->sbuf copy with per-partition scale, alternating."""
            _alt[0] ^= 1
            if _alt[0]:
                nc.vector.tensor_scalar_mul(dst, src, scale_ap)
            else:
                nc.scalar.activation(dst, src, AF.Copy, scale=scale_ap)

        def transpose_into(dst, src_tile, n_tiles, trp):
            """dst [128, 2, n_tiles, 128] <- per-tile transposes of
            src_tile [128, n_tiles, 256]; two node tiles per psum buf,
            one merged copy per pair."""
            for nt0 in range(0, n_tiles, 2):
                ps = trp.tile([128, 2, 2, 128], BF, tag="tr")
                for k in range(2):
                    for kcc in range(2):
                        nc.tensor.transpose(ps[:, kcc, k, :],
                                            src_tile[:, nt0 + k, ts(kcc, 128)],
                                            ident[:])
                cpy(dst[:, :, nt0:nt0 + 2, :], ps[:])

        def ln_z(out_ap, s1_ap):
            """out = normalize(s1) along free dim (gamma folded downstream).
            The 256-wide apply alternates between vector and scalar."""
            st6 = tpool.tile([128, 6], F32, tag="st6")
            nc.vector.bn_stats(st6[:], s1_ap)
            mv = tpool.tile([128, 2], F32, tag="mv")
            nc.vector.bn_aggr(mv[:], st6[:])
            std = tpool.tile([128, 1], F32, tag="std")
            nc.scalar.activation(std[:], mv[:, 1:2], AF.Sqrt, bias=epsb[:])
            inv = tpool.tile([128, 1], F32, tag="inv")
            nc.vector.reciprocal(inv[:], std[:])
            _alt[0] ^= 1
            if _alt[0]:
                mi = tpool.tile([128, 1], F32, tag="mi")
                nc.scalar.activation(mi[:], mv[:, 0:1], AF.Copy, scale=inv[:])
                nc.vector.tensor_scalar(out_ap, s1_ap, inv[:], mi[:],
                                        ALU.mult, ALU.subtract)
            else:
                nmi = tpool.tile([128, 1], F32, tag="mi")
                nc.vector.tensor_scalar(nmi[:], mv[:, 0:1], inv[:], -1.0,
                                        ALU.mult, ALU.mult)
                nc.scalar.activation(out_ap, s1_ap, AF.Identity, bias=nmi[:],
                                     scale=inv[:])

        def l2_rec(v_ap, scale=None):
            """[128,1] 1/sqrt(||v||^2+1e-24) per row, optionally * scale."""
            ssq = tpool.tile([128, 1], F32, tag="ssq")
            scr = qpool.tile([128, D], F32, tag="sq")
            nc.scalar.activation(scr[:, :v_ap.shape[-1]], v_ap, AF.Square,
                                 accum_out=ssq[:])
            nrm = tpool.tile([128, 1], F32, tag="l2n")
            nc.scalar.activation(nrm[:], ssq[:], AF.Sqrt, bias=eps24[:])
            rec = tpool.tile([128, 1], F32, tag="l2r")
            nc.vector.reciprocal(rec[:], nrm[:])
            if scale is not None:
                nc.scalar.mul(rec[:], rec[:], scale)
            return rec

        def ln_z_group(s1g, n, out_aps):
            """Batched LN over n windows s1g[:, k, :]: the [128,1]-ish
            stat ops run once per group instead of once per tile."""
            st6g = tpool.tile([128, GRP, 6], F32, tag="st6g", bufs=4)
            for k in range(n):
                nc.vector.bn_stats(st6g[:, k, :], s1g[:, k, :])
            mv4 = tpool.tile([128, GRP, 2], F32, tag="mv4", bufs=4)
            for k in range(n):
                nc.vector.bn_aggr(mv4[:, k, :], st6g[:, k, :])
            stdg = tpool.tile([128, GRP], F32, tag="stdg", bufs=4)
            nc.scalar.activation(stdg[:, 0:n], mv4[:, 0:n, 1], AF.Sqrt,
                                 bias=epsb[:])
            invg = tpool.tile([128, GRP], F32, tag="invg", bufs=4)
            nc.vector.reciprocal(invg[:, 0:n], stdg[:, 0:n])
            mig = tpool.tile([128, GRP], F32, tag="mig", bufs=4)
            nc.vector.tensor_tensor(mig[:, 0:n], mv4[:, 0:n, 0],
                                    invg[:, 0:n], ALU.mult)
            nmig = tpool.tile([128, GRP], F32, tag="nmig", bufs=4)
            nc.vector.tensor_scalar_mul(nmig[:, 0:n], mig[:, 0:n], -1.0)
            for k in range(n):
                _c3[0] = (_c3[0] + 1) % 3
                if _c3[0] != 0:
                    nc.vector.tensor_scalar(out_aps[k], s1g[:, k, :],
                                            invg[:, k:k + 1], mig[:, k:k + 1],
                                            ALU.mult, ALU.subtract)
                else:
                    nc.scalar.activation(out_aps[k], s1g[:, k, :],
                                         AF.Identity, bias=nmig[:, k:k + 1],
                                         scale=invg[:, k:k + 1])

        def sage_epi_group(pss, res_aps, out_aps):
            """Batched: out = LN(l2(u) + res) per tile, u in PSUM
            (deg-scaled lr was matmul-accumulated via the identity).
            Uses LN's per-row scale invariance: LN(u/||u|| + res) ==
            LN(u + ||u||*res), so no reciprocal on the l2 side."""
            n = len(pss)
            ssqg = tpool.tile([128, GRP], F32, tag="ssqg", bufs=4)
            for k in range(n):
                scr = qpool.tile([128, D], F32, tag="sq")
                nc.scalar.activation(scr[:], pss[k], AF.Square,
                                     accum_out=ssqg[:, k:k + 1])
            nrmg = tpool.tile([128, GRP], F32, tag="nrmg", bufs=4)
            nc.scalar.activation(nrmg[:, 0:n], ssqg[:, 0:n], AF.Sqrt,
                                 bias=eps24[:])
            s1g = epool.tile([128, GRP, D], F32, tag="s1g", bufs=2)
            for k in range(n):
                nc.vector.scalar_tensor_tensor(s1g[:, k, :], res_aps[k],
                                               nrmg[:, k:k + 1], pss[k],
                                               ALU.mult, ALU.add)
            ln_z_group(s1g, n, out_aps)

        def ln_relu(out_ap, h_ap):
            """out = relu(normalize(h)) (gamma>0 folded downstream)."""
            st6 = tpool.tile([128, 6], F32, tag="st6")
            nc.vector.bn_stats(st6[:], h_ap)
            mv = tpool.tile([128, 2], F32, tag="mv")
            nc.vector.bn_aggr(mv[:], st6[:])
            std = tpool.tile([128, 1], F32, tag="std")
            nc.scalar.activation(std[:], mv[:, 1:2], AF.Sqrt, bias=epsb[:])
            inv = tpool.tile([128, 1], F32, tag="inv")
            nc.vector.reciprocal(inv[:], std[:])
            nmi = tpool.tile([128, 1], F32, tag="nmi")
            nc.vector.tensor_scalar(nmi[:], mv[:, 0:1], inv[:], -1.0,
                                    ALU.mult, ALU.mult)
            nc.scalar.activation(out_ap, h_ap, AF.Relu, bias=nmi[:],
                                 scale=inv[:])

        # ---------------- phase builders ----------------
        def phase_A1b(i):
            with tc.tile_pool(name=f"psAtb{i}", bufs=2, space="PSUM") as trA:
                transpose_into(xbT, xb, NBT, trA)

        def phase_A1t(i):
            with tc.tile_pool(name=f"psAtt{i}", bufs=2, space="PSUM") as trA:
                transpose_into(xtT, xt, NTT, trA)

        def phase_A2b(i):
            jb = wi[("b", i)]
            with tc.tile_pool(name=f"psA{i}", bufs=3, space="PSUM") as mmA:
                for nt in range(NBT):
                    ps = mmA.tile([128, 2, 256], F32, tag="mmb")
                    for kc in range(2):
                        nc.tensor.matmul(ps[:], xbT[:, kc, nt, :],
                                         wc[:, ds(jb + 2 * kc, 2), :],
                                         start=kc == 0, stop=kc == 1)
                    cpy(tl_b[:, nt, 0, :], ps[:, 0, :])
                    scpy(tl_b[:, nt, 1, :], ps[:, 1, :], degb[:, nt:nt + 1])
                    nc.gpsimd.tensor_copy(tb8[:, nt, :, :],
                                          tl_b[:, nt, 0, :])

        def phase_A2t(i):
            jt = wi[("t", i)]
            with tc.tile_pool(name=f"psAt{i}", bufs=3, space="PSUM") as mmA:
                for tt_ in range(NTT):
                    ps = mmA.tile([128, 2, 256], F32, tag="mmt", bufs=2)
                    ps2 = mmA.tile([128, 256], F32, tag="mmt2", bufs=2)
                    for kc in range(2):
                        # [tt|lrt] one 512-wide group; ttl separate bank
                        nc.tensor.matmul(ps[:], xtT[:, kc, tt_, :],
                                         wc[:, ds(jt + 3 * kc, 2), :],
                                         start=kc == 0, stop=kc == 1)
                        nc.tensor.matmul(ps2[:], xtT[:, kc, tt_, :],
                                         wc[:, jt + 3 * kc + 2, :],
                                         start=kc == 0, stop=kc == 1)
                    cpy(tl_t[:, tt_, 0, :], ps[:, 0, :])
                    scpy(tl_t[:, tt_, 1, :], ps[:, 1, :], degt[:, tt_:tt_ + 1])
                    cpy(tt8[:, tt_, :], ps2[:])

        # Abt stream pool: lives only through the layer pipeline, freed
        # before the final-phase pools are first used.
        bpool_cm = tc.tile_pool(name="abt_stream", bufs=3)
        bpool = bpool_cm.__enter__()

        def phase_B(i):
            """partial_t via DoubleRow fp8; trigger AllReduce."""
            pscr = ppool.tile([128, 2, TP], F8, tag="pscr")
            with tc.tile_pool(name=f"psB{i}", bufs=1, space="PSUM") as ptp:
                pt = [ptp.tile([128, TP], F32, tag=f"pt{dh}",
                               name=f"pt{i}_{dh}") for dh in range(2)]
                for cp in range(NBP):
                    for sh in range(2):
                        ab = bpool.tile([128, 2, TP // 2], F8, tag="abt",
                                        bufs=8)
                        (nc.sync if sh == 0 else nc.gpsimd).dma_start(
                            ab[:], Abt_d[cp][:, :, ts(sh, TP // 2)])
                        for dh in range(2):
                            for s in range(2):
                                nc.tensor.matmul(
                                    pt[dh][:, ts(2 * sh + s, 512)],
                                    tb8[:, ds(2 * cp, 2), dh, :],
                                    ab[:, :, ts(s, 512)],
                                    start=cp == 0, stop=cp == NBP - 1,
                                    perf_mode=DR)
                nc.vector.tensor_copy(pscr[:, 0, :], pt[0][:])
                nc.scalar.copy(pscr[:, 1, :], pt[1][:])
            bounce_in = dpool.tile([128, 2, TP], F8, tag="bin")
            bounce_out = dpool.tile([128, 2, TP], F8, tag="bout",
                                    addr_space="Shared")
            nc.scalar.dma_start(bounce_in[:], pscr[:])
            nc.gpsimd.collective_compute(
                "AllReduce", ALU.add, replica_groups=[list(range(NC))],
                ins=[bounce_in.opt()], outs=[bounce_out.opt()])
            return bounce_out

        def phase_C(i, glo=0, ghi=NBG):
            with tc.tile_pool(name=f"psC{i}_{glo}", bufs=6,
                              space="PSUM") as mmC:
                for g in range(glo, ghi):
                    ntiles = min(GRP, NBT - g * GRP)
                    pss = [mmC.tile([128, D], F32, tag="mm",
                                    name=f"cps{i}_{g}_{k}")
                           for k in range(ntiles)]
                    at = spool.tile([128, NTT, GW], F8, tag="at")
                    nc.sync.dma_start(at[:], At_d[g])
                    for t2 in range(NTT // 2):
                        for k in range(ntiles):
                            nc.tensor.matmul(pss[k][:],
                                             at[:, ds(2 * t2, 2), ts(k, 128)],
                                             tt8[:, ds(2 * t2, 2), :],
                                             start=t2 == 0, stop=False,
                                             perf_mode=DR)
                    for k in range(ntiles):
                        nt = g * GRP + k
                        nc.tensor.matmul(pss[k][:], ident[:],
                                         tl_b[:, nt, 1, :],
                                         start=False, stop=True)
                    nts = [g * GRP + k for k in range(ntiles)]
                    sage_epi_group([pss[k][:] for k in range(ntiles)],
                                   [tl_b[:, nt, 0, :] for nt in nts],
                                   [xb[:, nt, :] for nt in nts])

        def phase_D(i, bounce_out):
            jl = wi[("wl", i)]
            pm = ppool.tile([128, 2, TP], F8, tag="pscr")
            nc.sync.dma_start(pm[:], bounce_out[:])
            with tc.tile_pool(name=f"psD{i}", bufs=5, space="PSUM") as mmD:
                for g in range(NTT // GRP):
                    pss = []
                    for k in range(GRP):
                        tt_ = g * GRP + k
                        ps = mmD.tile([128, D], F32, tag="mm")
                        for kc in range(2):
                            nc.tensor.matmul(ps[:], pm[:, kc, ts(tt_, 128)],
                                             wc[:, jl + kc, :],
                                             start=kc == 0, stop=False)
                        nc.tensor.matmul(ps[:], ident[:], tl_t[:, tt_, 1, :],
                                         start=False, stop=True)
                        pss.append(ps)
                    tts = [g * GRP + k for k in range(GRP)]
                    sage_epi_group([p_[:] for p_ in pss],
                                   [tl_t[:, t_, 0, :] for t_ in tts],
                                   [xt[:, t_, :] for t_ in tts])

        # ================= main pipeline =================
        phase_A1b(0)
        phase_A2b(0)
        phase_A1t(0)
        phase_A2t(0)
        bo = phase_B(0)

        # final-phase tiles (tag reuse: layer tiles dead by first use)
        jfb = wi["fb"]
        jft = wi["ft"]
        jbb = wi["p1bb"]
        fl_b = fpool.tile([128, NBT, 2, D], BF, tag="tl_b")   # [lmp|hb1a]
        xtm_bf = fpool.tile([128, NTT, D], BF, tag="xt")
        htr_bf = fpool.tile([128, NTT, D], BF, tag="tl_t")
        mpz_bf = fpool.tile([128, NBT, D], BF, tag="xb")
        hball = fpool.tile([128, NBT, D], BF, tag="xbT")
        hbn_all = fpool.tile([128, NBT, 128], BF, tag="xtT")
        htn_T = fpool.tile([128, TP], BF, tag="tb8")

        def F1b_bacteria():
            with tc.tile_pool(name="psF1b", bufs=4, space="PSUM") as mmF:
                for nt in range(NBT):
                    ps = mmF.tile([128, 2, 256], F32, tag="mmb")
                    for kc in range(2):
                        nc.tensor.matmul(ps[:], xbT[:, kc, nt, :],
                                         wc[:, ds(jfb + 2 * kc, 2), :],
                                         start=kc == 0, stop=kc == 1)
                    cpy(fl_b[:, nt, :, :], ps[:])

        def F1b_traits():
            with tc.tile_pool(name="psF1t", bufs=4, space="PSUM") as mmF:
                for tt_ in range(NTT):
                    ps = mmF.tile([128, 512], F32, tag="mmt")
                    for kc in range(2):
                        nc.tensor.matmul(ps[:], xtT[:, kc, tt_, :],
                                         wc[:, ds(jft + 2 * kc, 2), :],
                                         start=kc == 0, stop=kc == 1)
                    cpy(xtm_bf[:, tt_, :], ps[:, 0:256])
                    ln_relu(htr_bf[:, tt_, :], ps[:, 256:512])

        for i in range(L):
            phase_C(i)
            if i + 1 < L:
                phase_A1b(i + 1)
                phase_A2b(i + 1)
                phase_D(i, bo)
                phase_A1t(i + 1)
                phase_A2t(i + 1)
                bo = phase_B(i + 1)
            else:
                # last layer: overlap D's epilogues with the bacteria-side
                # head matmuls (which only need xb/C(L-1))
                phase_A1b(9)
                F1b_bacteria()
                phase_D(i, bo)
                bpool_cm.__exit__(None, None, None)
                phase_A1t(9)
                F1b_traits()

        def _f3_stage1(g, trS, mmS1):
            """mpz tiles of group g -> transpose -> p1 -> relu-LN -> hball."""
            lo = g * GRP
            hi = min(lo + GRP, NBT)
            for nt0 in range(lo, hi, 2):
                pst = trS.tile([128, 2, 2, 128], BF, tag="tr")
                for k in range(2):
                    for kcc in range(2):
                        nc.tensor.transpose(pst[:, kcc, k, :],
                                            mpz_bf[:, nt0 + k, ts(kcc, 128)],
                                            ident[:])
                mpTp = hpool.tile([128, 2, 2, 128], BF, tag="htT")
                cpy(mpTp[:], pst[:])
                for k in range(2):
                    nt = nt0 + k
                    ps = mmS1.tile([128, D], F32, tag="mm")
                    for kc in range(2):
                        nc.tensor.matmul(ps[:], mpTp[:, kc, k, :],
                                         wc[:, jbb + kc, :],
                                         start=kc == 0, stop=kc == 1)
                    hv = epool.tile([128, D], F32, tag="cbv")
                    nc.vector.scalar_tensor_tensor(hv[:], ps[:], 1.0,
                                                   fl_b[:, nt, 1, :],
                                                   ALU.mult, ALU.add)
                    ln_relu(hball[:, nt, :], hv[:])

        simrec = {}

        def _f3_stage2(p, trP, mmP):
            """hball pair p -> transpose -> p2 -> hbn_all (UNnormalized;
            the 1/||hb|| row scale rides the sim output copy instead,
            keeping the reciprocal off the critical chain)."""
            nt0 = 2 * p
            psh = trP.tile([128, 2, 2, 128], BF, tag="tr")
            for k in range(2):
                for kcc in range(2):
                    nc.tensor.transpose(psh[:, kcc, k, :],
                                        hball[:, nt0 + k, ts(kcc, 128)],
                                        ident[:])
            hbTp = hpool.tile([128, 2, 2, 128], BF, tag="htT")
            cpy(hbTp[:], psh[:])
            hns = []
            for k in range(2):
                ps2 = mmP.tile([128, 128], F32, tag="mm2")
                for kc in range(2):
                    nc.tensor.matmul(ps2[:], hbTp[:, kc, k, :],
                                     w128[:, kc, :],
                                     start=kc == 0, stop=kc == 1)
                simrec[nt0 + k] = l2_rec(ps2[:])
                hn = hpool.tile([128, 128], BF, tag="hn")
                cpy(hn[:], ps2[:])
                hns.append(hn)
            psn = trP.tile([128, 2, 128], BF, tag="trn", bufs=2)
            for k in range(2):
                nc.tensor.transpose(psn[:, k, :], hns[k][:], ident[:])
            cpy(hbn_all[:, nt0:nt0 + 2, :], psn[:])

        def _f1c_pair(p, trp, mmp):
            """trait head pair p: htr -> transpose -> p2 -> l2 -> htn_T."""
            t0 = 2 * p
            pst = trp.tile([128, 2, 2, 128], BF, tag="tr")
            for k in range(2):
                for kcc in range(2):
                    nc.tensor.transpose(pst[:, kcc, k, :],
                                        htr_bf[:, t0 + k, ts(kcc, 128)],
                                        ident[:])
            htTp = hpool.tile([128, 2, 2, 128], BF, tag="htT")
            cpy(htTp[:], pst[:])
            hns = []
            for k in range(2):
                ps2 = mmp.tile([128, 128], F32, tag="mm2")
                for kc in range(2):
                    nc.tensor.matmul(ps2[:], htTp[:, kc, k, :],
                                     w128[:, 2 + kc, :],
                                     start=kc == 0, stop=kc == 1)
                rec = l2_rec(ps2[:], scale=temp)
                hn = hpool.tile([128, 128], BF, tag="hn")
                nc.vector.tensor_scalar_mul(hn[:], ps2[:], rec[:])
                hns.append(hn)
            psn = trp.tile([128, 2, 2, 128], BF, tag="tr")
            for k in range(2):
                nc.tensor.transpose(psn[:, 0, k, :], hns[k][:], ident[:])
            cpy(htn_T[:, ds(t0 * 128, 256)], psn[:, 0, :, :])

        # ---- F2 (metapath agg) fused with F3 stage 1 + F1c trait head ----
        with tc.tile_pool(name="psF2", bufs=5, space="PSUM") as mmZ, \
             tc.tile_pool(name="psS1t", bufs=2, space="PSUM") as trS, \
             tc.tile_pool(name="psF1c", bufs=1, space="PSUM") as mmH:
            for g in range(NBG):
                ntiles = min(GRP, NBT - g * GRP)
                pss = [mmZ.tile([128, D], F32, tag="mm", name=f"zps{g}_{k}")
                       for k in range(ntiles)]
                at = spool.tile([128, NTT, GW], F8, tag="at")
                nc.sync.dma_start(at[:], Amp_d[g])
                for tck in range(NTT):
                    for k in range(ntiles):
                        nc.tensor.matmul(pss[k][:], at[:, tck, ts(k, 128)],
                                         xtm_bf[:, tck, :],
                                         start=tck == 0, stop=tck == NTT - 1)
                zvg = epool.tile([128, GRP, D], F32, tag="s1g", bufs=2)
                for k in range(ntiles):
                    nt = g * GRP + k
                    nc.vector.scalar_tensor_tensor(zvg[:, k, :], pss[k][:],
                                                   1.0, fl_b[:, nt, 0, :],
                                                   ALU.mult, ALU.add)
                ln_z_group(zvg, ntiles,
                           [mpz_bf[:, g * GRP + k, :]
                            for k in range(ntiles)])
                _f1c_pair(g, trS, mmH)
                # stage 1 for the previous group's tiles (mpz ready)
                if g > 0:
                    _f3_stage1(g - 1, trS, mmZ)
            _f3_stage1(NBG - 1, trS, mmZ)

        # ---- F3 stage 2 (p2 head + normalize) fused with sim writeout ----
        with tc.tile_pool(name="psS2", bufs=2, space="PSUM") as mmP, \
             tc.tile_pool(name="psS2t", bufs=2, space="PSUM") as trP, \
             tc.tile_pool(name="psSim", bufs=2, space="PSUM") as mmS:
            def _sim_tile(nt):
                for s2 in range(2):
                    ob = hpool.tile([128, 1024], BF, tag="ob", bufs=2)
                    for s in range(2):
                        pso = mmS.tile([128, 512], F32, tag="sim")
                        nc.tensor.matmul(
                            pso[:], hbn_all[:, nt, :],
                            htn_T[:, ds(s2 * 1024 + s * 512, 512)],
                            start=True, stop=True)
                        scpy(ob[:, ts(s, 512)], pso[:], simrec[nt][:])
                    nc.sync.dma_start(sim_d[nt][:, ts(s2, 1024)], ob[:])

            # sim tiles lag stage-2 by one pair so their PE work fills
            # the gaps left by stage-2's scalar/vector latency chains
            for p in range(NBP):
                _f3_stage2(p, trP, mmP)
                if p > 0:
                    _sim_tile(2 * p - 2)
                    _sim_tile(2 * p - 1)
            _sim_tile(NBT - 2)
            _sim_tile(NBT - 1)

    nc.compile()
    return nc


def kernel(**inputs):
    in_maps, meta = _prep(inputs)
    nc = build_program(meta)
    res = run_bass_kernel_spmd(nc, in_maps, core_ids=list(range(NC)))
    sim = np.empty((N_B, N_T), np.float32)
    for c in range(NC):
        shard = np.asarray(res.results[c]["simO"], np.float32).reshape(BP, TP)
        sim[c * B_SH:(c + 1) * B_SH] = shard[:B_SH, :N_T]
    if meta["simb"] != 0.0:
        sim += np.float32(meta["simb"])
    return sim


# revision 84
# speedup vs baseline: 1.0621x; 1.0013x over previous
"""Trainium2 Bass kernel for nn_HCMGNNBasedMetaPathModel (v4).

Bacteria rows sharded over 8 cores (3750 -> padded 3840); trait side and
weights replicated. Edge segment ops are dense adjacency matmuls with
EXACT fp8e4m3 edge counts; the 1/max(deg,1) mean normalization is
applied post-matmul in f32.

v4 over v3 (957us -> ~805us):
 - phases B and C use fp8 DoubleRow matmuls (both operands fp8): the
   aggregation features (tb, lin_l-transformed tt) are cast to fp8e4m3,
   paired along the contraction dim with the fp8 adjacency tiles.
   Numerically validated: adds <0.1% to the final max-rel error.
 - layer pipeline reordered: C(i), A1b(i+1), A2b(i+1), D(i),
   A1t(i+1), A2t(i+1), B(i+1)+AR trigger.  The AllReduce of layer i
   (~40-55us fixed latency) completes under C(i)+A1b+A2b; D's
   vector/scalar epilogues overlap A1t/A2t and B's DMA-bound stretch.
 - epilogues use LN/l2 per-row scale invariance twice: the deg-scaled
   lin_r term is matmul-accumulated into the aggregation PSUM via the
   identity (so no cbv STT), and LN(u/||u||+res) == LN(u+||u||*res)
   (so no reciprocal).  Per-group batching of the [128,1]-wide stat
   ops (sqrt/recip/mean*inv) cuts small-op overhead ~4x.
 - final phase fully pipelined: F1b bacteria head overlaps D(2); F2
   metapath groups fused with the F3 p1-head stage and the F1c trait
   head; F3 p2-head keeps hb UNnormalized and applies 1/||hb|| on the
   sim output copy; sim matmuls+writeout lag one pair so the 15MB
   output DMA spreads across the whole tail.
 - Abt stream: fp8 pair tiles split into halves on two DMA queues
   (sync+gpsimd) with a 5-deep pool; fp8 casts round-robin over
   gpsimd/vector/scalar.
"""
import contextlib
import sys

for _p in ("/opt/trn_rl_repo",):
    if _p not in sys.path:
        sys.path.insert(0, _p)

import numpy as np
import ml_dtypes

import concourse.bass as bass
import concourse.tile as tile
from concourse import bacc, mybir
from concourse.bass_utils import run_bass_kernel_spmd

BF16 = ml_dtypes.bfloat16
FP8 = ml_dtypes.float8_e4m3
F32 = mybir.dt.float32
BF = mybir.dt.bfloat16
F8 = mybir.dt.float8e4
AF = mybir.ActivationFunctionType
ALU = mybir.AluOpType
DR = mybir.MatmulPerfMode.DoubleRow
ts, ds = bass.ts, bass.ds

N_B, N_T, D, L, M = 30000, 2000, 256, 3, 2
NC = 8
B_SH = 3750          # real bacteria rows per core
BP = 3840            # padded bacteria rows per core
NBT = BP // 128      # 30 node tiles
NBP = NBT // 2       # 15 node-tile pairs
TP = 2048            # padded trait rows
NTT = TP // 128      # 16 trait tiles
GRP = 4              # bacteria tiles per aggregation group
NBG = 8              # ceil(30/4) groups
GW = GRP * 128       # 512 group width (last group: 2 real tiles + pad)
LN_EPS = 1e-5
NW = 12 * 3 + 10     # wc entries

# ---------------------------------------------------------------------------
# Host-side preprocessing
# ---------------------------------------------------------------------------


def _counts(src, dst, n_dst, n_src):
    """A[d, s] = #edges (s->d) as float32; plus per-dst degree."""
    idx = dst.astype(np.int64) * n_src + src.astype(np.int64)
    A = np.bincount(idx, minlength=n_dst * n_src).astype(np.float32)
    A = A.reshape(n_dst, n_src)
    deg = np.bincount(dst.astype(np.int64), minlength=n_dst).astype(np.float32)
    return A, deg


def _to_fp8_exact(A, what):
    A8 = A.astype(FP8)
    assert np.array_equal(A8.astype(np.float32), A), f"{what} not fp8-exact"
    return A8


def _prep(inp):
    f32 = np.float32
    emb_b = np.asarray(inp["emb_b"], f32)
    emb_t = np.asarray(inp["emb_t"], f32)

    A_tb, deg_b = _counts(np.asarray(inp["src_tb"]), np.asarray(inp["dst_tb"]),
                          N_B, N_T)
    A_bt, deg_t = _counts(np.asarray(inp["src_bt"]), np.asarray(inp["dst_bt"]),
                          N_T, N_B)
    mpw = np.asarray(inp["mp_w"], np.float64)
    e = np.exp(mpw - mpw.max())
    w = e / e.sum()
    sw = float(w.sum())
    mp_adj = np.asarray(inp["mp_adj"], f32)
    A_mp = (w[0] * mp_adj[0].astype(np.float64) +
            w[1] * mp_adj[1].astype(np.float64)).astype(f32)

    xb0 = np.zeros((NC, NBT, 128, D), BF16)
    xb0.reshape(NC, BP, D)[:, :B_SH] = emb_b.reshape(NC, B_SH, D).astype(BF16)
    xt0 = np.zeros((NTT, 128, D), BF16)
    xt0.reshape(TP, D)[:N_T] = emb_t.astype(BF16)

    BPG = NBG * GW  # 4096 padded for group layout

    def shard_T(A):  # [N_B, N_T] -> per-core [NBG, 128, NTT, GW] trait-major
        out = np.zeros((NC, NBG, 128, NTT, GW), f32)
        for c in range(NC):
            blk = np.zeros((TP, BPG), f32)
            blk[:N_T, :B_SH] = A[c * B_SH:(c + 1) * B_SH].T
            out[c] = blk.reshape(NTT, 128, NBG, GW).transpose(2, 1, 0, 3)
        return out

    At8 = _to_fp8_exact(shard_T(A_tb), "A_tb counts")
    Amp8 = _to_fp8_exact(shard_T(A_mp), "A_mp")
    # Abt in node-tile PAIRS for DoubleRow: [NBP, 128, 2, TP]
    Abt8 = np.zeros((NC, NBP, 128, 2, TP), FP8)
    for c in range(NC):
        blk = np.zeros((BP, TP), f32)
        blk[:B_SH, :N_T] = A_bt[:, c * B_SH:(c + 1) * B_SH].T
        Abt8[c] = _to_fp8_exact(
            blk.reshape(NBP, 2, 128, TP).transpose(0, 2, 1, 3), "A_bt counts")

    # degree normalizers: deg_real = max(deg, 1) (exact f32 ints).
    # The kernel exploits LN/l2 per-row scale invariance: instead of
    # l2(agg/deg + lr) it computes l2(agg + deg*lr), so only deg_real
    # is needed.
    degR = np.maximum(deg_b, 1.0)
    degbR = np.zeros((NC, 128, NBT), f32)
    for c in range(NC):
        v = np.ones(BP, f32)
        v[:B_SH] = degR[c * B_SH:(c + 1) * B_SH]
        degbR[c] = v.reshape(NBT, 128).T
    dtr = np.ones(TP, f32)
    dtr[:N_T] = np.maximum(deg_t, 1.0)
    degtR = np.ascontiguousarray(dtr.reshape(NTT, 128).T)

    # ---- weights: gamma folds; all biases must be zero ----
    for nm in ("bt_b", "bt_t", "bl_b", "bl_t", "mpb", "bp1b", "bp1t",
               "bp2b", "bp2t", "lnb_b", "lnb_t", "mplnb", "plnbb", "plnbt"):
        assert not np.any(np.asarray(inp[nm])), f"{nm} must be zero"
    plngb = np.asarray(inp["plngb"], f32)
    plngt = np.asarray(inp["plngt"], f32)
    assert (plngb > 0).all() and (plngt > 0).all()

    Wt_b = np.asarray(inp["Wt_b"], f32)
    Wt_t = np.asarray(inp["Wt_t"], f32)
    Wl_b = np.asarray(inp["Wl_b"], f32)
    Wr_b = np.asarray(inp["Wr_b"], f32)
    Wl_t = np.asarray(inp["Wl_t"], f32)
    Wr_t = np.asarray(inp["Wr_t"], f32)
    lng_b = np.asarray(inp["lng_b"], f32)
    lng_t = np.asarray(inp["lng_t"], f32)

    wlist = []

    def addW(*WTs):
        """for kc in (0,1): for each WT: append WT[kc*128:(kc+1)*128]."""
        base = len(wlist)
        for kc in range(2):
            for WT in WTs:
                wlist.append(np.ascontiguousarray(
                    WT[kc * 128:(kc + 1) * 128]).astype(BF16))
        return base

    wi = {}
    g_b = np.ones(D, f32)
    g_t = np.ones(D, f32)
    for i in range(L):
        WtTb = Wt_b[i].T * g_b[:, None]
        WcTb = (Wr_b[i] @ Wt_b[i]).T * g_b[:, None]
        WtTt = Wt_t[i].T * g_t[:, None]
        WctT = (Wl_b[i] @ Wt_t[i]).T * g_t[:, None]
        WcTt = (Wr_t[i] @ Wt_t[i]).T * g_t[:, None]
        wi[("b", i)] = addW(WtTb, WcTb)          # stride 2 per kc
        wi[("t", i)] = addW(WtTt, WcTt, WctT)    # [tt, lrt, ttl] per kc
        wi[("wl", i)] = addW(Wl_t[i].T)          # stride 1 per kc
        g_b, g_t = lng_b[i], lng_t[i]

    mpW = np.asarray(inp["mpW"], f32)
    g_mp = np.asarray(inp["mplng"], f32)
    Wp1b = np.asarray(inp["Wp1b"], f32)
    Wp1t = np.asarray(inp["Wp1t"], f32)
    wi["fb"] = addW(sw * (mpW.T * g_b[:, None]), Wp1b[:, :D].T * g_b[:, None])
    wi["ft"] = addW(mpW.T * g_t[:, None], Wp1t.T * g_t[:, None])
    wi["p1bb"] = addW(Wp1b[:, D:].T * g_mp[:, None])
    Wc = np.stack(wlist)
    assert Wc.shape[0] == NW, Wc.shape

    W2b = (np.asarray(inp["Wp2b"], f32) * plngb).T
    W2t = (np.asarray(inp["Wp2t"], f32) * plngt).T
    w128 = np.stack([
        np.ascontiguousarray(W2b[:128]).astype(BF16),
        np.ascontiguousarray(W2b[128:]).astype(BF16),
        np.ascontiguousarray(W2t[:128]).astype(BF16),
        np.ascontiguousarray(W2t[128:]).astype(BF16),
    ])

    ident = np.eye(128, dtype=f32).astype(BF16)
    temp = float(np.asarray(inp["temperature"]).reshape(-1)[0])
    simb = float(np.asarray(inp["sim_bias"]).reshape(-1)[0])

    shared = dict(xt0=xt0, Wc=Wc, W128=w128, degt=degtR, ident=ident)
    in_maps = []
    for c in range(NC):
        m = dict(shared)
        m["xb0"] = np.ascontiguousarray(xb0[c])
        m["At8"] = np.ascontiguousarray(At8[c])
        m["Abt8"] = np.ascontiguousarray(Abt8[c])
        m["Amp8"] = np.ascontiguousarray(Amp8[c])
        m["degb"] = np.ascontiguousarray(degbR[c])
        in_maps.append(m)
    meta = dict(wi=wi, temp=temp, simb=simb)
    return in_maps, meta


# ---------------------------------------------------------------------------
# Device program
# ---------------------------------------------------------------------------


def build_program(meta):
    nc = bacc.Bacc("TRN2", target_bir_lowering=False, debug=False,
                   num_devices=NC)
    wi = meta["wi"]
    temp = meta["temp"]

    xb0_d = nc.dram_tensor("xb0", [NBT, 128, D], BF, kind="ExternalInput")
    xt0_d = nc.dram_tensor("xt0", [NTT, 128, D], BF, kind="ExternalInput")
    At_d = nc.dram_tensor("At8", [NBG, 128, NTT, GW], F8, kind="ExternalInput")
    Abt_d = nc.dram_tensor("Abt8", [NBP, 128, 2, TP], F8,
                           kind="ExternalInput")
    Amp_d = nc.dram_tensor("Amp8", [NBG, 128, NTT, GW], F8,
                           kind="ExternalInput")
    Wc_d = nc.dram_tensor("Wc", [NW, 128, D], BF, kind="ExternalInput")
    W128_d = nc.dram_tensor("W128", [4, 128, 128], BF, kind="ExternalInput")
    degb_d = nc.dram_tensor("degb", [128, NBT], F32, kind="ExternalInput")
    degt_d = nc.dram_tensor("degt", [128, NTT], F32, kind="ExternalInput")
    id_d = nc.dram_tensor("ident", [128, 128], BF, kind="ExternalInput")
    sim_d = nc.dram_tensor("simO", [NBT, 128, TP], BF, kind="ExternalOutput")

    with tile.TileContext(nc) as tc, contextlib.ExitStack() as ctx:
        cpool = ctx.enter_context(tc.tile_pool(name="const", bufs=1))
        fpool = ctx.enter_context(tc.tile_pool(name="feat", bufs=1))
        spool = ctx.enter_context(tc.tile_pool(name="at_stream", bufs=3))
        epool = ctx.enter_context(tc.tile_pool(name="epi", bufs=3))
        hpool = ctx.enter_context(tc.tile_pool(name="hbf", bufs=3))
        qpool = ctx.enter_context(tc.tile_pool(name="sq", bufs=1))
        tpool = ctx.enter_context(tc.tile_pool(name="tiny", bufs=24))
        ppool = ctx.enter_context(tc.tile_pool(name="pscr", bufs=2))
        dpool = ctx.enter_context(tc.tile_pool(name="dram", bufs=2,
                                               space="DRAM"))

        # ---- persistent features first (layer-0 transposes need them);
        # chunked so the first transposes start almost immediately ----
        ident = cpool.tile([128, 128], BF)
        nc.sync.dma_start(ident[:], id_d[:])
        xb = fpool.tile([128, NBT, D], BF, tag="xb")
        for t0 in range(0, NBT, 2):
            nc.sync.dma_start(xb[:, t0:t0 + 2, :],
                              xb0_d[t0:t0 + 2].rearrange("n p d -> p n d"))
        xt = fpool.tile([128, NTT, D], BF, tag="xt")
        for t0 in range(0, NTT, 4):
            nc.gpsimd.dma_start(xt[:, t0:t0 + 4, :],
                                xt0_d[t0:t0 + 4].rearrange("n p d -> p n d"))

        # ---- constants (stream under the first transposes) ----
        wc = cpool.tile([128, NW, D], BF)
        for j0 in range(0, NW, 12):
            j1 = min(j0 + 12, NW)
            nc.sync.dma_start(wc[:, j0:j1, :],
                              Wc_d[j0:j1].rearrange("n p d -> p n d"))
        w128 = cpool.tile([128, 4, 128], BF)
        nc.sync.dma_start(w128[:], W128_d.rearrange("n p d -> p n d"))
        degb = cpool.tile([128, NBT], F32)
        nc.sync.dma_start(degb[:], degb_d[:])
        degt = cpool.tile([128, NTT], F32)
        nc.sync.dma_start(degt[:], degt_d[:])
        epsb = cpool.tile([128, 1], F32, name="epsb")
        nc.gpsimd.memset(epsb[:], LN_EPS)
        eps24 = cpool.tile([128, 1], F32, name="eps24")
        nc.gpsimd.memset(eps24[:], 1e-24)

        # feature tiles: [tb|lrb], [tt|lrt], fp8 copies for DoubleRow
        tl_b = fpool.tile([128, NBT, 2, D], BF, tag="tl_b")
        tl_t = fpool.tile([128, NTT, 2, D], BF, tag="tl_t")
        tb8 = fpool.tile([128, NBT, 2, 128], F8, tag="tb8")
        tt8 = fpool.tile([128, NTT, D], F8, tag="tt8")
        xbT = fpool.tile([128, 2, NBT, 128], BF, tag="xbT")
        xtT = fpool.tile([128, 2, NTT, 128], BF, tag="xtT")

        # engine alternation for plain psum->sbuf copies
        _alt = [0]

        def cpy(dst, src):
            _alt[0] ^= 1
            (nc.vector.tensor_copy if _alt[0] else nc.scalar.copy)(dst, src)

        # 3-way rotation for sbuf->sbuf fp8 casts (gpsimd is slow alone)
        _c3 = [0]

        def cast3(dst, src):
            _c3[0] = (_c3[0] + 1) % 3
            eng = (nc.gpsimd, nc.vector, nc.scalar)[_c3[0]]
            (eng.copy if eng is nc.scalar else eng.tensor_copy)(dst, src)

        def scpy(dst, src, scale_ap):
            """psum->sbuf copy with per-partition scale, alternating."""
            _alt[0] ^= 1
            if _alt[0]:
                nc.vector.tensor_scalar_mul(dst, src, scale_ap)
            else:
                nc.scalar.activation(dst, src, AF.Copy, scale=scale_ap)

        def transpose_into(dst, src_tile, n_tiles, trp):
            """dst [128, 2, n_tiles, 128] <- per-tile transposes of
            src_tile [128, n_tiles, 256]; two node tiles per psum buf,
            one merged copy per pair."""
            for nt0 in range(0, n_tiles, 2):
                ps = trp.tile([128, 2, 2, 128], BF, tag="tr")
                for k in range(2):
                    for kcc in range(2):
                        nc.tensor.transpose(ps[:, kcc, k, :],
                                            src_tile[:, nt0 + k, ts(kcc, 128)],
                                            ident[:])
                cpy(dst[:, :, nt0:nt0 + 2, :], ps[:])

        def ln_z(out_ap, s1_ap):
            """out = normalize(s1) along free dim (gamma folded downstream).
            The 256-wide apply alternates between vector and scalar."""
            st6 = tpool.tile([128, 6], F32, tag="st6")
            nc.vector.bn_stats(st6[:], s1_ap)
            mv = tpool.tile([128, 2], F32, tag="mv")
            nc.vector.bn_aggr(mv[:], st6[:])
            std = tpool.tile([128, 1], F32, tag="std")
            nc.scalar.activation(std[:], mv[:, 1:2], AF.Sqrt, bias=epsb[:])
            inv = tpool.tile([128, 1], F32, tag="inv")
            nc.vector.reciprocal(inv[:], std[:])
            _alt[0] ^= 1
            if _alt[0]:
                mi = tpool.tile([128, 1], F32, tag="mi")
                nc.scalar.activation(mi[:], mv[:, 0:1], AF.Copy, scale=inv[:])
                nc.vector.tensor_scalar(out_ap, s1_ap, inv[:], mi[:],
                                        ALU.mult, ALU.subtract)
            else:
                nmi = tpool.tile([128, 1], F32, tag="mi")
                nc.vector.tensor_scalar(nmi[:], mv[:, 0:1], inv[:], -1.0,
                                        ALU.mult, ALU.mult)
                nc.scalar.activation(out_ap, s1_ap, AF.Identity, bias=nmi[:],
                                     scale=inv[:])

        def l2_rec(v_ap, scale=None):
            """[128,1] 1/sqrt(||v||^2+1e-24) per row, optionally * scale."""
            ssq = tpool.tile([128, 1], F32, tag="ssq")
            scr = qpool.tile([128, D], F32, tag="sq")
            nc.scalar.activation(scr[:, :v_ap.shape[-1]], v_ap, AF.Square,
                                 accum_out=ssq[:])
            nrm = tpool.tile([128, 1], F32, tag="l2n")
            nc.scalar.activation(nrm[:], ssq[:], AF.Sqrt, bias=eps24[:])
            rec = tpool.tile([128, 1], F32, tag="l2r")
            nc.vector.reciprocal(rec[:], nrm[:])
            if scale is not None:
                nc.scalar.mul(rec[:], rec[:], scale)
            return rec

        def ln_z_group(s1g, n, out_aps):
            """Batched LN over n windows s1g[:, k, :]: the [128,1]-ish
            stat ops run once per group instead of once per tile."""
            st6g = tpool.tile([128, GRP, 6], F32, tag="st6g", bufs=4)
            for k in range(n):
                nc.vector.bn_stats(st6g[:, k, :], s1g[:, k, :])
            mv4 = tpool.tile([128, GRP, 2], F32, tag="mv4", bufs=4)
            for k in range(n):
                nc.vector.bn_aggr(mv4[:, k, :], st6g[:, k, :])
            stdg = tpool.tile([128, GRP], F32, tag="stdg", bufs=4)
            nc.scalar.activation(stdg[:, 0:n], mv4[:, 0:n, 1], AF.Sqrt,
                                 bias=epsb[:])
            invg = tpool.tile([128, GRP], F32, tag="invg", bufs=4)
            nc.vector.reciprocal(invg[:, 0:n], stdg[:, 0:n])
            mig = tpool.tile([128, GRP], F32, tag="mig", bufs=4)
            nc.vector.tensor_tensor(mig[:, 0:n], mv4[:, 0:n, 0],
                                    invg[:, 0:n], ALU.mult)
            nmig = tpool.tile([128, GRP], F32, tag="nmig", bufs=4)
            nc.vector.tensor_scalar_mul(nmig[:, 0:n], mig[:, 0:n], -1.0)
            for k in range(n):
                _c3[0] = (_c3[0] + 1) % 3
                if _c3[0] != 0:
                    nc.vector.tensor_scalar(out_aps[k], s1g[:, k, :],
                                            invg[:, k:k + 1], mig[:, k:k + 1],
                                            ALU.mult, ALU.subtract)
                else:
                    nc.scalar.activation(out_aps[k], s1g[:, k, :],
                                         AF.Identity, bias=nmig[:, k:k + 1],
                                         scale=invg[:, k:k + 1])

        def sage_epi_group(pss, res_aps, out_aps):
            """Batched: out = LN(l2(u) + res) per tile, u in PSUM
            (deg-scaled lr was matmul-accumulated via the identity).
            Uses LN's per-row scale invariance: LN(u/||u|| + res) ==
            LN(u + ||u||*res), so no reciprocal on the l2 side."""
            n = len(pss)
            ssqg = tpool.tile([128, GRP], F32, tag="ssqg", bufs=4)
            for k in range(n):
                scr = qpool.tile([128, D], F32, tag="sq")
                nc.scalar.activation(scr[:], pss[k], AF.Square,
                                     accum_out=ssqg[:, k:k + 1])
            nrmg = tpool.tile([128, GRP], F32, tag="nrmg", bufs=4)
            nc.scalar.activation(nrmg[:, 0:n], ssqg[:, 0:n], AF.Sqrt,
                                 bias=eps24[:])
            s1g = epool.tile([128, GRP, D], F32, tag="s1g", bufs=2)
            for k in range(n):
                nc.vector.scalar_tensor_tensor(s1g[:, k, :], res_aps[k],
                                               nrmg[:, k:k + 1], pss[k],
                                               ALU.mult, ALU.add)
            ln_z_group(s1g, n, out_aps)

        def ln_relu(out_ap, h_ap):
            """out = relu(normalize(h)) (gamma>0 folded downstream)."""
            st6 = tpool.tile([128, 6], F32, tag="st6")
            nc.vector.bn_stats(st6[:], h_ap)
            mv = tpool.tile([128, 2], F32, tag="mv")
            nc.vector.bn_aggr(mv[:], st6[:])
            std = tpool.tile([128, 1], F32, tag="std")
            nc.scalar.activation(std[:], mv[:, 1:2], AF.Sqrt, bias=epsb[:])
            inv = tpool.tile([128, 1], F32, tag="inv")
            nc.vector.reciprocal(inv[:], std[:])
            nmi = tpool.tile([128, 1], F32, tag="nmi")
            nc.vector.tensor_scalar(nmi[:], mv[:, 0:1], inv[:], -1.0,
                                    ALU.mult, ALU.mult)
            nc.scalar.activation(out_ap, h_ap, AF.Relu, bias=nmi[:],
                                 scale=inv[:])

        # ---------------- phase builders ----------------
        def phase_A1b(i):
            with tc.tile_pool(name=f"psAtb{i}", bufs=2, space="PSUM") as trA:
                transpose_into(xbT, xb, NBT, trA)

        def phase_A1t(i):
            with tc.tile_pool(name=f"psAtt{i}", bufs=2, space="PSUM") as trA:
                transpose_into(xtT, xt, NTT, trA)

        def phase_A2b(i):
            jb = wi[("b", i)]
            with tc.tile_pool(name=f"psA{i}", bufs=3, space="PSUM") as mmA:
                for nt in range(NBT):
                    ps = mmA.tile([128, 2, 256], F32, tag="mmb")
                    for kc in range(2):
                        nc.tensor.matmul(ps[:], xbT[:, kc, nt, :],
                                         wc[:, ds(jb + 2 * kc, 2), :],
                                         start=kc == 0, stop=kc == 1)
                    cpy(tl_b[:, nt, 0, :], ps[:, 0, :])
                    scpy(tl_b[:, nt, 1, :], ps[:, 1, :], degb[:, nt:nt + 1])
                    nc.gpsimd.tensor_copy(tb8[:, nt, :, :],
                                          tl_b[:, nt, 0, :])

        def phase_A2t(i):
            jt = wi[("t", i)]
            with tc.tile_pool(name=f"psAt{i}", bufs=3, space="PSUM") as mmA:
                for tt_ in range(NTT):
                    ps = mmA.tile([128, 2, 256], F32, tag="mmt", bufs=2)
                    ps2 = mmA.tile([128, 256], F32, tag="mmt2", bufs=2)
                    for kc in range(2):
                        # [tt|lrt] one 512-wide group; ttl separate bank
                        nc.tensor.matmul(ps[:], xtT[:, kc, tt_, :],
                                         wc[:, ds(jt + 3 * kc, 2), :],
                                         start=kc == 0, stop=kc == 1)
                        nc.tensor.matmul(ps2[:], xtT[:, kc, tt_, :],
                                         wc[:, jt + 3 * kc + 2, :],
                                         start=kc == 0, stop=kc == 1)
                    cpy(tl_t[:, tt_, 0, :], ps[:, 0, :])
                    scpy(tl_t[:, tt_, 1, :], ps[:, 1, :], degt[:, tt_:tt_ + 1])
                    cpy(tt8[:, tt_, :], ps2[:])

        # Abt stream pool: lives only through the layer pipeline, freed
        # before the final-phase pools are first used.
        bpool_cm = tc.tile_pool(name="abt_stream", bufs=3)
        bpool = bpool_cm.__enter__()

        def phase_B(i):
            """partial_t via DoubleRow fp8; trigger AllReduce."""
            pscr = ppool.tile([128, 2, TP], F8, tag="pscr")
            with tc.tile_pool(name=f"psB{i}", bufs=1, space="PSUM") as ptp:
                pt = [ptp.tile([128, TP], F32, tag=f"pt{dh}",
                               name=f"pt{i}_{dh}") for dh in range(2)]
                for cp in range(NBP):
                    for sh in range(2):
                        ab = bpool.tile([128, 2, TP // 2], F8, tag="abt",
                                        bufs=9)
                        (nc.sync if sh == 0 else nc.gpsimd).dma_start(
                            ab[:], Abt_d[cp][:, :, ts(sh, TP // 2)])
                        for dh in range(2):
                            for s in range(2):
                                nc.tensor.matmul(
                                    pt[dh][:, ts(2 * sh + s, 512)],
                                    tb8[:, ds(2 * cp, 2), dh, :],
                                    ab[:, :, ts(s, 512)],
                                    start=cp == 0, stop=cp == NBP - 1,
                                    perf_mode=DR)
                nc.vector.tensor_copy(pscr[:, 0, :], pt[0][:])
                nc.scalar.copy(pscr[:, 1, :], pt[1][:])
            bounce_in = dpool.tile([128, 2, TP], F8, tag="bin")
            bounce_out = dpool.tile([128, 2, TP], F8, tag="bout",
                                    addr_space="Shared")
            nc.scalar.dma_start(bounce_in[:], pscr[:])
            nc.gpsimd.collective_compute(
                "AllReduce", ALU.add, replica_groups=[list(range(NC))],
                ins=[bounce_in.opt()], outs=[bounce_out.opt()])
            return bounce_out

        def phase_C(i, glo=0, ghi=NBG):
            with tc.tile_pool(name=f"psC{i}_{glo}", bufs=6,
                              space="PSUM") as mmC:
                for g in range(glo, ghi):
                    ntiles = min(GRP, NBT - g * GRP)
                    pss = [mmC.tile([128, D], F32, tag="mm",
                                    name=f"cps{i}_{g}_{k}")
                           for k in range(ntiles)]
                    at = spool.tile([128, NTT, GW], F8, tag="at")
                    nc.sync.dma_start(at[:], At_d[g])
                    for t2 in range(NTT // 2):
                        for k in range(ntiles):
                            nc.tensor.matmul(pss[k][:],
                                             at[:, ds(2 * t2, 2), ts(k, 128)],
                                             tt8[:, ds(2 * t2, 2), :],
                                             start=t2 == 0, stop=False,
                                             perf_mode=DR)
                    for k in range(ntiles):
                        nt = g * GRP + k
                        nc.tensor.matmul(pss[k][:], ident[:],
                                         tl_b[:, nt, 1, :],
                                         start=False, stop=True)
                    nts = [g * GRP + k for k in range(ntiles)]
                    sage_epi_group([pss[k][:] for k in range(ntiles)],
                                   [tl_b[:, nt, 0, :] for nt in nts],
                                   [xb[:, nt, :] for nt in nts])

        def phase_D(i, bounce_out):
            jl = wi[("wl", i)]
            pm = ppool.tile([128, 2, TP], F8, tag="pscr")
            nc.sync.dma_start(pm[:], bounce_out[:])
            with tc.tile_pool(name=f"psD{i}", bufs=5, space="PSUM") as mmD:
                for g in range(NTT // GRP):
                    pss = []
                    for k in range(GRP):
                        tt_ = g * GRP + k
                        ps = mmD.tile([128, D], F32, tag="mm")
                        for kc in range(2):
                            nc.tensor.matmul(ps[:], pm[:, kc, ts(tt_, 128)],
                                             wc[:, jl + kc, :],
                                             start=kc == 0, stop=False)
                        nc.tensor.matmul(ps[:], ident[:], tl_t[:, tt_, 1, :],
                                         start=False, stop=True)
                        pss.append(ps)
                    tts = [g * GRP + k for k in range(GRP)]
                    sage_epi_group([p_[:] for p_ in pss],
                                   [tl_t[:, t_, 0, :] for t_ in tts],
                                   [xt[:, t_, :] for t_ in tts])

        # ================= main pipeline =================
        phase_A1b(0)
        phase_A2b(0)
        phase_A1t(0)
        phase_A2t(0)
        bo = phase_B(0)

        # final-phase tiles (tag reuse: layer tiles dead by first use)
        jfb = wi["fb"]
        jft = wi["ft"]
        jbb = wi["p1bb"]
        fl_b = fpool.tile([128, NBT, 2, D], BF, tag="tl_b")   # [lmp|hb1a]
        xtm_bf = fpool.tile([128, NTT, D], BF, tag="xt")
        htr_bf = fpool.tile([128, NTT, D], BF, tag="tl_t")
        mpz_bf = fpool.tile([128, NBT, D], BF, tag="xb")
        hball = fpool.tile([128, NBT, D], BF, tag="xbT")
        hbn_all = fpool.tile([128, NBT, 128], BF, tag="xtT")
        htn_T = fpool.tile([128, TP], BF, tag="tb8")

        def F1b_bacteria():
            with tc.tile_pool(name="psF1b", bufs=4, space="PSUM") as mmF:
                for nt in range(NBT):
                    ps = mmF.tile([128, 2, 256], F32, tag="mmb")
                    for kc in range(2):
                        nc.tensor.matmul(ps[:], xbT[:, kc, nt, :],
                                         wc[:, ds(jfb + 2 * kc, 2), :],
                                         start=kc == 0, stop=kc == 1)
                    cpy(fl_b[:, nt, :, :], ps[:])

        def F1b_traits():
            with tc.tile_pool(name="psF1t", bufs=4, space="PSUM") as mmF:
                for tt_ in range(NTT):
                    ps = mmF.tile([128, 512], F32, tag="mmt")
                    for kc in range(2):
                        nc.tensor.matmul(ps[:], xtT[:, kc, tt_, :],
                                         wc[:, ds(jft + 2 * kc, 2), :],
                                         start=kc == 0, stop=kc == 1)
                    cpy(xtm_bf[:, tt_, :], ps[:, 0:256])
                    ln_relu(htr_bf[:, tt_, :], ps[:, 256:512])

        for i in range(L):
            phase_C(i)
            if i + 1 < L:
                phase_A1b(i + 1)
                phase_A2b(i + 1)
                phase_D(i, bo)
                phase_A1t(i + 1)
                phase_A2t(i + 1)
                bo = phase_B(i + 1)
            else:
                # last layer: overlap D's epilogues with the bacteria-side
                # head matmuls (which only need xb/C(L-1))
                phase_A1b(9)
                F1b_bacteria()
                phase_D(i, bo)
                bpool_cm.__exit__(None, None, None)
                phase_A1t(9)
                F1b_traits()

        def _f3_stage1(g, trS, mmS1):
            """mpz tiles of group g -> transpose -> p1 -> relu-LN -> hball."""
            lo = g * GRP
            hi = min(lo + GRP, NBT)
            for nt0 in range(lo, hi, 2):
                pst = trS.tile([128, 2, 2, 128], BF, tag="tr")
                for k in range(2):
                    for kcc in range(2):
                        nc.tensor.transpose(pst[:, kcc, k, :],
                                            mpz_bf[:, nt0 + k, ts(kcc, 128)],
                                            ident[:])
                mpTp = hpool.tile([128, 2, 2, 128], BF, tag="htT")
                cpy(mpTp[:], pst[:])
                for k in range(2):
                    nt = nt0 + k
                    ps = mmS1.tile([128, D], F32, tag="mm")
                    for kc in range(2):
                        nc.tensor.matmul(ps[:], mpTp[:, kc, k, :],
                                         wc[:, jbb + kc, :],
                                         start=kc == 0, stop=kc == 1)
                    hv = epool.tile([128, D], F32, tag="cbv")
                    nc.vector.scalar_tensor_tensor(hv[:], ps[:], 1.0,
                                                   fl_b[:, nt, 1, :],
                                                   ALU.mult, ALU.add)
                    ln_relu(hball[:, nt, :], hv[:])

        simrec = {}

        def _f3_stage2(p, trP, mmP):
            """hball pair p -> transpose -> p2 -> hbn_all (UNnormalized;
            the 1/||hb|| row scale rides the sim output copy instead,
            keeping the reciprocal off the critical chain)."""
            nt0 = 2 * p
            psh = trP.tile([128, 2, 2, 128], BF, tag="tr")
            for k in range(2):
                for kcc in range(2):
                    nc.tensor.transpose(psh[:, kcc, k, :],
                                        hball[:, nt0 + k, ts(kcc, 128)],
                                        ident[:])
            hbTp = hpool.tile([128, 2, 2, 128], BF, tag="htT")
            cpy(hbTp[:], psh[:])
            hns = []
            for k in range(2):
                ps2 = mmP.tile([128, 128], F32, tag="mm2")
                for kc in range(2):
                    nc.tensor.matmul(ps2[:], hbTp[:, kc, k, :],
                                     w128[:, kc, :],
                                     start=kc == 0, stop=kc == 1)
                simrec[nt0 + k] = l2_rec(ps2[:])
                hn = hpool.tile([128, 128], BF, tag="hn")
                cpy(hn[:], ps2[:])
                hns.append(hn)
            psn = trP.tile([128, 2, 128], BF, tag="trn", bufs=2)
            for k in range(2):
                nc.tensor.transpose(psn[:, k, :], hns[k][:], ident[:])
            cpy(hbn_all[:, nt0:nt0 + 2, :], psn[:])

        def _f1c_pair(p, trp, mmp):
            """trait head pair p: htr -> transpose -> p2 -> l2 -> htn_T."""
            t0 = 2 * p
            pst = trp.tile([128, 2, 2, 128], BF, tag="tr")
            for k in range(2):
                for kcc in range(2):
                    nc.tensor.transpose(pst[:, kcc, k, :],
                                        htr_bf[:, t0 + k, ts(kcc, 128)],
                                        ident[:])
            htTp = hpool.tile([128, 2, 2, 128], BF, tag="htT")
            cpy(htTp[:], pst[:])
            hns = []
            for k in range(2):
                ps2 = mmp.tile([128, 128], F32, tag="mm2")
                for kc in range(2):
                    nc.tensor.matmul(ps2[:], htTp[:, kc, k, :],
                                     w128[:, 2 + kc, :],
                                     start=kc == 0, stop=kc == 1)
                rec = l2_rec(ps2[:], scale=temp)
                hn = hpool.tile([128, 128], BF, tag="hn")
                nc.vector.tensor_scalar_mul(hn[:], ps2[:], rec[:])
                hns.append(hn)
            psn = trp.tile([128, 2, 2, 128], BF, tag="tr")
            for k in range(2):
                nc.tensor.transpose(psn[:, 0, k, :], hns[k][:], ident[:])
            cpy(htn_T[:, ds(t0 * 128, 256)], psn[:, 0, :, :])

        # ---- F2 (metapath agg) fused with F3 stage 1 + F1c trait head ----
        with tc.tile_pool(name="psF2", bufs=5, space="PSUM") as mmZ, \
             tc.tile_pool(name="psS1t", bufs=2, space="PSUM") as trS, \
             tc.tile_pool(name="psF1c", bufs=1, space="PSUM") as mmH:
            for g in range(NBG):
                ntiles = min(GRP, NBT - g * GRP)
                pss = [mmZ.tile([128, D], F32, tag="mm", name=f"zps{g}_{k}")
                       for k in range(ntiles)]
                at = spool.tile([128, NTT, GW], F8, tag="at")
                nc.sync.dma_start(at[:], Amp_d[g])
                for tck in range(NTT):
                    for k in range(ntiles):
                        nc.tensor.matmul(pss[k][:], at[:, tck, ts(k, 128)],
                                         xtm_bf[:, tck, :],
                                         start=tck == 0, stop=tck == NTT - 1)
                zvg = epool.tile([128, GRP, D], F32, tag="s1g", bufs=2)
                for k in range(ntiles):
                    nt = g * GRP + k
                    nc.vector.scalar_tensor_tensor(zvg[:, k, :], pss[k][:],
                                                   1.0, fl_b[:, nt, 0, :],
                                                   ALU.mult, ALU.add)
                ln_z_group(zvg, ntiles,
                           [mpz_bf[:, g * GRP + k, :]
                            for k in range(ntiles)])
                _f1c_pair(g, trS, mmH)
                # stage 1 for the previous group's tiles (mpz ready)
                if g > 0:
                    _f3_stage1(g - 1, trS, mmZ)
            _f3_stage1(NBG - 1, trS, mmZ)

        # ---- F3 stage 2 (p2 head + normalize) fused with sim writeout ----
        with tc.tile_pool(name="psS2", bufs=2, space="PSUM") as mmP, \
             tc.tile_pool(name="psS2t", bufs=2, space="PSUM") as trP, \
             tc.tile_pool(name="psSim", bufs=2, space="PSUM") as mmS:
            def _sim_tile(nt):
                for s2 in range(2):
                    ob = hpool.tile([128, 1024], BF, tag="ob", bufs=2)
                    for s in range(2):
                        pso = mmS.tile([128, 512], F32, tag="sim")
                        nc.tensor.matmul(
                            pso[:], hbn_all[:, nt, :],
                            htn_T[:, ds(s2 * 1024 + s * 512, 512)],
                            start=True, stop=True)
                        scpy(ob[:, ts(s, 512)], pso[:], simrec[nt][:])
                    nc.sync.dma_start(sim_d[nt][:, ts(s2, 1024)], ob[:])

            # sim tiles lag stage-2 by one pair so their PE work fills
            # the gaps left by stage-2's scalar/vector latency chains
            for p in range(NBP):
                _f3_stage2(p, trP, mmP)
                if p > 0:
                    _sim_tile(2 * p - 2)
                    _sim_tile(2 * p - 1)
            _sim_tile(NBT - 2)
            _sim_tile(NBT - 1)

    nc.compile()
    return nc


def kernel(**inputs):
    in_maps, meta = _prep(inputs)
    nc = build_program(meta)
    res = run_bass_kernel_spmd(nc, in_maps, core_ids=list(range(NC)))
    sim = np.empty((N_B, N_T), np.float32)
    for c in range(NC):
        shard = np.asarray(res.results[c]["simO"], np.float32).reshape(BP, TP)
        sim[c * B_SH:(c + 1) * B_SH] = shard[:B_SH, :N_T]
    if meta["simb"] != 0.0:
        sim += np.float32(meta["simb"])
    return sim


# revision 85
# speedup vs baseline: 1.0621x; 1.0001x over previous
"""Trainium2 Bass kernel for nn_HCMGNNBasedMetaPathModel (v4).

Bacteria rows sharded over 8 cores (3750 -> padded 3840); trait side and
weights replicated. Edge segment ops are dense adjacency matmuls with
EXACT fp8e4m3 edge counts; the 1/max(deg,1) mean normalization is
applied post-matmul in f32.

v4 over v3 (957us -> ~805us):
 - phases B and C use fp8 DoubleRow matmuls (both operands fp8): the
   aggregation features (tb, lin_l-transformed tt) are cast to fp8e4m3,
   paired along the contraction dim with the fp8 adjacency tiles.
   Numerically validated: adds <0.1% to the final max-rel error.
 - layer pipeline reordered: C(i), A1b(i+1), A2b(i+1), D(i),
   A1t(i+1), A2t(i+1), B(i+1)+AR trigger.  The AllReduce of layer i
   (~40-55us fixed latency) completes under C(i)+A1b+A2b; D's
   vector/scalar epilogues overlap A1t/A2t and B's DMA-bound stretch.
 - epilogues use LN/l2 per-row scale invariance twice: the deg-scaled
   lin_r term is matmul-accumulated into the aggregation PSUM via the
   identity (so no cbv STT), and LN(u/||u||+res) == LN(u+||u||*res)
   (so no reciprocal).  Per-group batching of the [128,1]-wide stat
   ops (sqrt/recip/mean*inv) cuts small-op overhead ~4x.
 - final phase fully pipelined: F1b bacteria head overlaps D(2); F2
   metapath groups fused with the F3 p1-head stage and the F1c trait
   head; F3 p2-head keeps hb UNnormalized and applies 1/||hb|| on the
   sim output copy; sim matmuls+writeout lag one pair so the 15MB
   output DMA spreads across the whole tail.
 - Abt stream: fp8 pair tiles split into halves on two DMA queues
   (sync+gpsimd) with a 5-deep pool; fp8 casts round-robin over
   gpsimd/vector/scalar.
"""
import contextlib
import sys

for _p in ("/opt/trn_rl_repo",):
    if _p not in sys.path:
        sys.path.insert(0, _p)

import numpy as np
import ml_dtypes

import concourse.bass as bass
import concourse.tile as tile
from concourse import bacc, mybir
from concourse.bass_utils import run_bass_kernel_spmd

BF16 = ml_dtypes.bfloat16
FP8 = ml_dtypes.float8_e4m3
F32 = mybir.dt.float32
BF = mybir.dt.bfloat16
F8 = mybir.dt.float8e4
AF = mybir.ActivationFunctionType
ALU = mybir.AluOpType
DR = mybir.MatmulPerfMode.DoubleRow
ts, ds = bass.ts, bass.ds

N_B, N_T, D, L, M = 30000, 2000, 256, 3, 2
NC = 8
B_SH = 3750          # real bacteria rows per core
BP = 3840            # padded bacteria rows per core
NBT = BP // 128      # 30 node tiles
NBP = NBT // 2       # 15 node-tile pairs
TP = 2048            # padded trait rows
NTT = TP // 128      # 16 trait tiles
GRP = 4              # bacteria tiles per aggregation group
NBG = 8              # ceil(30/4) groups
GW = GRP * 128       # 512 group width (last group: 2 real tiles + pad)
LN_EPS = 1e-5
NW = 12 * 3 + 10     # wc entries

# ---------------------------------------------------------------------------
# Host-side preprocessing
# ---------------------------------------------------------------------------


def _counts(src, dst, n_dst, n_src):
    """A[d, s] = #edges (s->d) as float32; plus per-dst degree."""
    idx = dst.astype(np.int64) * n_src + src.astype(np.int64)
    A = np.bincount(idx, minlength=n_dst * n_src).astype(np.float32)
    A = A.reshape(n_dst, n_src)
    deg = np.bincount(dst.astype(np.int64), minlength=n_dst).astype(np.float32)
    return A, deg


def _to_fp8_exact(A, what):
    A8 = A.astype(FP8)
    assert np.array_equal(A8.astype(np.float32), A), f"{what} not fp8-exact"
    return A8


def _prep(inp):
    f32 = np.float32
    emb_b = np.asarray(inp["emb_b"], f32)
    emb_t = np.asarray(inp["emb_t"], f32)

    A_tb, deg_b = _counts(np.asarray(inp["src_tb"]), np.asarray(inp["dst_tb"]),
                          N_B, N_T)
    A_bt, deg_t = _counts(np.asarray(inp["src_bt"]), np.asarray(inp["dst_bt"]),
                          N_T, N_B)
    mpw = np.asarray(inp["mp_w"], np.float64)
    e = np.exp(mpw - mpw.max())
    w = e / e.sum()
    sw = float(w.sum())
    mp_adj = np.asarray(inp["mp_adj"], f32)
    A_mp = (w[0] * mp_adj[0].astype(np.float64) +
            w[1] * mp_adj[1].astype(np.float64)).astype(f32)

    xb0 = np.zeros((NC, NBT, 128, D), BF16)
    xb0.reshape(NC, BP, D)[:, :B_SH] = emb_b.reshape(NC, B_SH, D).astype(BF16)
    xt0 = np.zeros((NTT, 128, D), BF16)
    xt0.reshape(TP, D)[:N_T] = emb_t.astype(BF16)

    BPG = NBG * GW  # 4096 padded for group layout

    def shard_T(A):  # [N_B, N_T] -> per-core [NBG, 128, NTT, GW] trait-major
        out = np.zeros((NC, NBG, 128, NTT, GW), f32)
        for c in range(NC):
            blk = np.zeros((TP, BPG), f32)
            blk[:N_T, :B_SH] = A[c * B_SH:(c + 1) * B_SH].T
            out[c] = blk.reshape(NTT, 128, NBG, GW).transpose(2, 1, 0, 3)
        return out

    At8 = _to_fp8_exact(shard_T(A_tb), "A_tb counts")
    Amp8 = _to_fp8_exact(shard_T(A_mp), "A_mp")
    # Abt in node-tile PAIRS for DoubleRow: [NBP, 128, 2, TP]
    Abt8 = np.zeros((NC, NBP, 128, 2, TP), FP8)
    for c in range(NC):
        blk = np.zeros((BP, TP), f32)
        blk[:B_SH, :N_T] = A_bt[:, c * B_SH:(c + 1) * B_SH].T
        Abt8[c] = _to_fp8_exact(
            blk.reshape(NBP, 2, 128, TP).transpose(0, 2, 1, 3), "A_bt counts")

    # degree normalizers: deg_real = max(deg, 1) (exact f32 ints).
    # The kernel exploits LN/l2 per-row scale invariance: instead of
    # l2(agg/deg + lr) it computes l2(agg + deg*lr), so only deg_real
    # is needed.
    degR = np.maximum(deg_b, 1.0)
    degbR = np.zeros((NC, 128, NBT), f32)
    for c in range(NC):
        v = np.ones(BP, f32)
        v[:B_SH] = degR[c * B_SH:(c + 1) * B_SH]
        degbR[c] = v.reshape(NBT, 128).T
    dtr = np.ones(TP, f32)
    dtr[:N_T] = np.maximum(deg_t, 1.0)
    degtR = np.ascontiguousarray(dtr.reshape(NTT, 128).T)

    # ---- weights: gamma folds; all biases must be zero ----
    for nm in ("bt_b", "bt_t", "bl_b", "bl_t", "mpb", "bp1b", "bp1t",
               "bp2b", "bp2t", "lnb_b", "lnb_t", "mplnb", "plnbb", "plnbt"):
        assert not np.any(np.asarray(inp[nm])), f"{nm} must be zero"
    plngb = np.asarray(inp["plngb"], f32)
    plngt = np.asarray(inp["plngt"], f32)
    assert (plngb > 0).all() and (plngt > 0).all()

    Wt_b = np.asarray(inp["Wt_b"], f32)
    Wt_t = np.asarray(inp["Wt_t"], f32)
    Wl_b = np.asarray(inp["Wl_b"], f32)
    Wr_b = np.asarray(inp["Wr_b"], f32)
    Wl_t = np.asarray(inp["Wl_t"], f32)
    Wr_t = np.asarray(inp["Wr_t"], f32)
    lng_b = np.asarray(inp["lng_b"], f32)
    lng_t = np.asarray(inp["lng_t"], f32)

    wlist = []

    def addW(*WTs):
        """for kc in (0,1): for each WT: append WT[kc*128:(kc+1)*128]."""
        base = len(wlist)
        for kc in range(2):
            for WT in WTs:
                wlist.append(np.ascontiguousarray(
                    WT[kc * 128:(kc + 1) * 128]).astype(BF16))
        return base

    wi = {}
    g_b = np.ones(D, f32)
    g_t = np.ones(D, f32)
    for i in range(L):
        WtTb = Wt_b[i].T * g_b[:, None]
        WcTb = (Wr_b[i] @ Wt_b[i]).T * g_b[:, None]
        WtTt = Wt_t[i].T * g_t[:, None]
        WctT = (Wl_b[i] @ Wt_t[i]).T * g_t[:, None]
        WcTt = (Wr_t[i] @ Wt_t[i]).T * g_t[:, None]
        wi[("b", i)] = addW(WtTb, WcTb)          # stride 2 per kc
        wi[("t", i)] = addW(WtTt, WcTt, WctT)    # [tt, lrt, ttl] per kc
        wi[("wl", i)] = addW(Wl_t[i].T)          # stride 1 per kc
        g_b, g_t = lng_b[i], lng_t[i]

    mpW = np.asarray(inp["mpW"], f32)
    g_mp = np.asarray(inp["mplng"], f32)
    Wp1b = np.asarray(inp["Wp1b"], f32)
    Wp1t = np.asarray(inp["Wp1t"], f32)
    wi["fb"] = addW(sw * (mpW.T * g_b[:, None]), Wp1b[:, :D].T * g_b[:, None])
    wi["ft"] = addW(mpW.T * g_t[:, None], Wp1t.T * g_t[:, None])
    wi["p1bb"] = addW(Wp1b[:, D:].T * g_mp[:, None])
    Wc = np.stack(wlist)
    assert Wc.shape[0] == NW, Wc.shape

    W2b = (np.asarray(inp["Wp2b"], f32) * plngb).T
    W2t = (np.asarray(inp["Wp2t"], f32) * plngt).T
    w128 = np.stack([
        np.ascontiguousarray(W2b[:128]).astype(BF16),
        np.ascontiguousarray(W2b[128:]).astype(BF16),
        np.ascontiguousarray(W2t[:128]).astype(BF16),
        np.ascontiguousarray(W2t[128:]).astype(BF16),
    ])

    ident = np.eye(128, dtype=f32).astype(BF16)
    temp = float(np.asarray(inp["temperature"]).reshape(-1)[0])
    simb = float(np.asarray(inp["sim_bias"]).reshape(-1)[0])

    shared = dict(xt0=xt0, Wc=Wc, W128=w128, degt=degtR, ident=ident)
    in_maps = []
    for c in range(NC):
        m = dict(shared)
        m["xb0"] = np.ascontiguousarray(xb0[c])
        m["At8"] = np.ascontiguousarray(At8[c])
        m["Abt8"] = np.ascontiguousarray(Abt8[c])
        m["Amp8"] = np.ascontiguousarray(Amp8[c])
        m["degb"] = np.ascontiguousarray(degbR[c])
        in_maps.append(m)
    meta = dict(wi=wi, temp=temp, simb=simb)
    return in_maps, meta


# ---------------------------------------------------------------------------
# Device program
# ---------------------------------------------------------------------------


def build_program(meta):
    nc = bacc.Bacc("TRN2", target_bir_lowering=False, debug=False,
                   num_devices=NC)
    wi = meta["wi"]
    temp = meta["temp"]

    xb0_d = nc.dram_tensor("xb0", [NBT, 128, D], BF, kind="ExternalInput")
    xt0_d = nc.dram_tensor("xt0", [NTT, 128, D], BF, kind="ExternalInput")
    At_d = nc.dram_tensor("At8", [NBG, 128, NTT, GW], F8, kind="ExternalInput")
    Abt_d = nc.dram_tensor("Abt8", [NBP, 128, 2, TP], F8,
                           kind="ExternalInput")
    Amp_d = nc.dram_tensor("Amp8", [NBG, 128, NTT, GW], F8,
                           kind="ExternalInput")
    Wc_d = nc.dram_tensor("Wc", [NW, 128, D], BF, kind="ExternalInput")
    W128_d = nc.dram_tensor("W128", [4, 128, 128], BF, kind="ExternalInput")
    degb_d = nc.dram_tensor("degb", [128, NBT], F32, kind="ExternalInput")
    degt_d = nc.dram_tensor("degt", [128, NTT], F32, kind="ExternalInput")
    id_d = nc.dram_tensor("ident", [128, 128], BF, kind="ExternalInput")
    sim_d = nc.dram_tensor("simO", [NBT, 128, TP], BF, kind="ExternalOutput")

    with tile.TileContext(nc) as tc, contextlib.ExitStack() as ctx:
        cpool = ctx.enter_context(tc.tile_pool(name="const", bufs=1))
        fpool = ctx.enter_context(tc.tile_pool(name="feat", bufs=1))
        spool = ctx.enter_context(tc.tile_pool(name="at_stream", bufs=3))
        epool = ctx.enter_context(tc.tile_pool(name="epi", bufs=3))
        hpool = ctx.enter_context(tc.tile_pool(name="hbf", bufs=3))
        qpool = ctx.enter_context(tc.tile_pool(name="sq", bufs=1))
        tpool = ctx.enter_context(tc.tile_pool(name="tiny", bufs=24))
        ppool = ctx.enter_context(tc.tile_pool(name="pscr", bufs=2))
        dpool = ctx.enter_context(tc.tile_pool(name="dram", bufs=2,
                                               space="DRAM"))

        # ---- persistent features first (layer-0 transposes need them);
        # chunked so the first transposes start almost immediately ----
        ident = cpool.tile([128, 128], BF)
        nc.sync.dma_start(ident[:], id_d[:])
        xb = fpool.tile([128, NBT, D], BF, tag="xb")
        for t0 in range(0, NBT, 2):
            nc.sync.dma_start(xb[:, t0:t0 + 2, :],
                              xb0_d[t0:t0 + 2].rearrange("n p d -> p n d"))
        xt = fpool.tile([128, NTT, D], BF, tag="xt")
        for t0 in range(0, NTT, 4):
            nc.gpsimd.dma_start(xt[:, t0:t0 + 4, :],
                                xt0_d[t0:t0 + 4].rearrange("n p d -> p n d"))

        # ---- constants (stream under the first transposes) ----
        wc = cpool.tile([128, NW, D], BF)
        for j0 in range(0, NW, 12):
            j1 = min(j0 + 12, NW)
            nc.sync.dma_start(wc[:, j0:j1, :],
                              Wc_d[j0:j1].rearrange("n p d -> p n d"))
        w128 = cpool.tile([128, 4, 128], BF)
        nc.sync.dma_start(w128[:], W128_d.rearrange("n p d -> p n d"))
        degb = cpool.tile([128, NBT], F32)
        nc.sync.dma_start(degb[:], degb_d[:])
        degt = cpool.tile([128, NTT], F32)
        nc.sync.dma_start(degt[:], degt_d[:])
        epsb = cpool.tile([128, 1], F32, name="epsb")
        nc.gpsimd.memset(epsb[:], LN_EPS)
        eps24 = cpool.tile([128, 1], F32, name="eps24")
        nc.gpsimd.memset(eps24[:], 1e-24)

        # feature tiles: [tb|lrb], [tt|lrt], fp8 copies for DoubleRow
        tl_b = fpool.tile([128, NBT, 2, D], BF, tag="tl_b")
        tl_t = fpool.tile([128, NTT, 2, D], BF, tag="tl_t")
        tb8 = fpool.tile([128, NBT, 2, 128], F8, tag="tb8")
        tt8 = fpool.tile([128, NTT, D], F8, tag="tt8")
        xbT = fpool.tile([128, 2, NBT, 128], BF, tag="xbT")
        xtT = fpool.tile([128, 2, NTT, 128], BF, tag="xtT")

        # engine alternation for plain psum->sbuf copies
        _alt = [0]

        def cpy(dst, src):
            _alt[0] ^= 1
            (nc.vector.tensor_copy if _alt[0] else nc.scalar.copy)(dst, src)

        # 3-way rotation for sbuf->sbuf fp8 casts (gpsimd is slow alone)
        _c3 = [0]

        def cast3(dst, src):
            _c3[0] = (_c3[0] + 1) % 3
            eng = (nc.gpsimd, nc.vector, nc.scalar)[_c3[0]]
            (eng.copy if eng is nc.scalar else eng.tensor_copy)(dst, src)

        def scpy(dst, src, scale_ap):
            """psum->sbuf copy with per-partition scale, alternating."""
            _alt[0] ^= 1
            if _alt[0]:
                nc.vector.tensor_scalar_mul(dst, src, scale_ap)
            else:
                nc.scalar.activation(dst, src, AF.Copy, scale=scale_ap)

        def transpose_into(dst, src_tile, n_tiles, trp):
            """dst [128, 2, n_tiles, 128] <- per-tile transposes of
            src_tile [128, n_tiles, 256]; two node tiles per psum buf,
            one merged copy per pair."""
            for nt0 in range(0, n_tiles, 2):
                ps = trp.tile([128, 2, 2, 128], BF, tag="tr")
                for k in range(2):
                    for kcc in range(2):
                        nc.tensor.transpose(ps[:, kcc, k, :],
                                            src_tile[:, nt0 + k, ts(kcc, 128)],
                                            ident[:])
                cpy(dst[:, :, nt0:nt0 + 2, :], ps[:])

        def ln_z(out_ap, s1_ap):
            """out = normalize(s1) along free dim (gamma folded downstream).
            The 256-wide apply alternates between vector and scalar."""
            st6 = tpool.tile([128, 6], F32, tag="st6")
            nc.vector.bn_stats(st6[:], s1_ap)
            mv = tpool.tile([128, 2], F32, tag="mv")
            nc.vector.bn_aggr(mv[:], st6[:])
            std = tpool.tile([128, 1], F32, tag="std")
            nc.scalar.activation(std[:], mv[:, 1:2], AF.Sqrt, bias=epsb[:])
            inv = tpool.tile([128, 1], F32, tag="inv")
            nc.vector.reciprocal(inv[:], std[:])
            _alt[0] ^= 1
            if _alt[0]:
                mi = tpool.tile([128, 1], F32, tag="mi")
                nc.scalar.activation(mi[:], mv[:, 0:1], AF.Copy, scale=inv[:])
                nc.vector.tensor_scalar(out_ap, s1_ap, inv[:], mi[:],
                                        ALU.mult, ALU.subtract)
            else:
                nmi = tpool.tile([128, 1], F32, tag="mi")
                nc.vector.tensor_scalar(nmi[:], mv[:, 0:1], inv[:], -1.0,
                                        ALU.mult, ALU.mult)
                nc.scalar.activation(out_ap, s1_ap, AF.Identity, bias=nmi[:],
                                     scale=inv[:])

        def l2_rec(v_ap, scale=None):
            """[128,1] 1/sqrt(||v||^2+1e-24) per row, optionally * scale."""
            ssq = tpool.tile([128, 1], F32, tag="ssq")
            scr = qpool.tile([128, D], F32, tag="sq")
            nc.scalar.activation(scr[:, :v_ap.shape[-1]], v_ap, AF.Square,
                                 accum_out=ssq[:])
            nrm = tpool.tile([128, 1], F32, tag="l2n")
            nc.scalar.activation(nrm[:], ssq[:], AF.Sqrt, bias=eps24[:])
            rec = tpool.tile([128, 1], F32, tag="l2r")
            nc.vector.reciprocal(rec[:], nrm[:])
            if scale is not None:
                nc.scalar.mul(rec[:], rec[:], scale)
            return rec

        def ln_z_group(s1g, n, out_aps):
            """Batched LN over n windows s1g[:, k, :]: the [128,1]-ish
            stat ops run once per group instead of once per tile."""
            st6g = tpool.tile([128, GRP, 6], F32, tag="st6g", bufs=4)
            for k in range(n):
                nc.vector.bn_stats(st6g[:, k, :], s1g[:, k, :])
            mv4 = tpool.tile([128, GRP, 2], F32, tag="mv4", bufs=4)
            for k in range(n):
                nc.vector.bn_aggr(mv4[:, k, :], st6g[:, k, :])
            stdg = tpool.tile([128, GRP], F32, tag="stdg", bufs=4)
            nc.scalar.activation(stdg[:, 0:n], mv4[:, 0:n, 1], AF.Sqrt,
                                 bias=epsb[:])
            invg = tpool.tile([128, GRP], F32, tag="invg", bufs=4)
            nc.vector.reciprocal(invg[:, 0:n], stdg[:, 0:n])
            mig = tpool.tile([128, GRP], F32, tag="mig", bufs=4)
            nc.vector.tensor_tensor(mig[:, 0:n], mv4[:, 0:n, 0],
                                    invg[:, 0:n], ALU.mult)
            nmig = tpool.tile([128, GRP], F32, tag="nmig", bufs=4)
            nc.vector.tensor_scalar_mul(nmig[:, 0:n], mig[:, 0:n], -1.0)
            for k in range(n):
                _c3[0] = (_c3[0] + 1) % 3
                if _c3[0] != 0:
                    nc.vector.tensor_scalar(out_aps[k], s1g[:, k, :],
                                            invg[:, k:k + 1], mig[:, k:k + 1],
                                            ALU.mult, ALU.subtract)
                else:
                    nc.scalar.activation(out_aps[k], s1g[:, k, :],
                                         AF.Identity, bias=nmig[:, k:k + 1],
                                         scale=invg[:, k:k + 1])

        def sage_epi_group(pss, res_aps, out_aps):
            """Batched: out = LN(l2(u) + res) per tile, u in PSUM
            (deg-scaled lr was matmul-accumulated via the identity).
            Uses LN's per-row scale invariance: LN(u/||u|| + res) ==
            LN(u + ||u||*res), so no reciprocal on the l2 side."""
            n = len(pss)
            ssqg = tpool.tile([128, GRP], F32, tag="ssqg", bufs=4)
            for k in range(n):
                scr = qpool.tile([128, D], F32, tag="sq")
                nc.scalar.activation(scr[:], pss[k], AF.Square,
                                     accum_out=ssqg[:, k:k + 1])
            nrmg = tpool.tile([128, GRP], F32, tag="nrmg", bufs=4)
            nc.scalar.activation(nrmg[:, 0:n], ssqg[:, 0:n], AF.Sqrt,
                                 bias=eps24[:])
            s1g = epool.tile([128, GRP, D], F32, tag="s1g", bufs=2)
            for k in range(n):
                nc.vector.scalar_tensor_tensor(s1g[:, k, :], res_aps[k],
                                               nrmg[:, k:k + 1], pss[k],
                                               ALU.mult, ALU.add)
            ln_z_group(s1g, n, out_aps)

        def ln_relu(out_ap, h_ap):
            """out = relu(normalize(h)) (gamma>0 folded downstream)."""
            st6 = tpool.tile([128, 6], F32, tag="st6")
            nc.vector.bn_stats(st6[:], h_ap)
            mv = tpool.tile([128, 2], F32, tag="mv")
            nc.vector.bn_aggr(mv[:], st6[:])
            std = tpool.tile([128, 1], F32, tag="std")
            nc.scalar.activation(std[:], mv[:, 1:2], AF.Sqrt, bias=epsb[:])
            inv = tpool.tile([128, 1], F32, tag="inv")
            nc.vector.reciprocal(inv[:], std[:])
            nmi = tpool.tile([128, 1], F32, tag="nmi")
            nc.vector.tensor_scalar(nmi[:], mv[:, 0:1], inv[:], -1.0,
                                    ALU.mult, ALU.mult)
            nc.scalar.activation(out_ap, h_ap, AF.Relu, bias=nmi[:],
                                 scale=inv[:])

        # ---------------- phase builders ----------------
        def phase_A1b(i):
            with tc.tile_pool(name=f"psAtb{i}", bufs=2, space="PSUM") as trA:
                transpose_into(xbT, xb, NBT, trA)

        def phase_A1t(i):
            with tc.tile_pool(name=f"psAtt{i}", bufs=2, space="PSUM") as trA:
                transpose_into(xtT, xt, NTT, trA)

        def phase_A2b(i):
            jb = wi[("b", i)]
            with tc.tile_pool(name=f"psA{i}", bufs=3, space="PSUM") as mmA:
                for nt in range(NBT):
                    ps = mmA.tile([128, 2, 256], F32, tag="mmb")
                    for kc in range(2):
                        nc.tensor.matmul(ps[:], xbT[:, kc, nt, :],
                                         wc[:, ds(jb + 2 * kc, 2), :],
                                         start=kc == 0, stop=kc == 1)
                    cpy(tl_b[:, nt, 0, :], ps[:, 0, :])
                    scpy(tl_b[:, nt, 1, :], ps[:, 1, :], degb[:, nt:nt + 1])
                    nc.gpsimd.tensor_copy(tb8[:, nt, :, :],
                                          tl_b[:, nt, 0, :])

        def phase_A2t(i):
            jt = wi[("t", i)]
            with tc.tile_pool(name=f"psAt{i}", bufs=3, space="PSUM") as mmA:
                for tt_ in range(NTT):
                    ps = mmA.tile([128, 2, 256], F32, tag="mmt", bufs=2)
                    ps2 = mmA.tile([128, 256], F32, tag="mmt2", bufs=2)
                    for kc in range(2):
                        # [tt|lrt] one 512-wide group; ttl separate bank
                        nc.tensor.matmul(ps[:], xtT[:, kc, tt_, :],
                                         wc[:, ds(jt + 3 * kc, 2), :],
                                         start=kc == 0, stop=kc == 1)
                        nc.tensor.matmul(ps2[:], xtT[:, kc, tt_, :],
                                         wc[:, jt + 3 * kc + 2, :],
                                         start=kc == 0, stop=kc == 1)
                    cpy(tl_t[:, tt_, 0, :], ps[:, 0, :])
                    scpy(tl_t[:, tt_, 1, :], ps[:, 1, :], degt[:, tt_:tt_ + 1])
                    cpy(tt8[:, tt_, :], ps2[:])

        # Abt stream pool: lives only through the layer pipeline, freed
        # before the final-phase pools are first used.
        bpool_cm = tc.tile_pool(name="abt_stream", bufs=3)
        bpool = bpool_cm.__enter__()

        def phase_B(i):
            """partial_t via DoubleRow fp8; trigger AllReduce."""
            pscr = ppool.tile([128, 2, TP], F8, tag="pscr")
            with tc.tile_pool(name=f"psB{i}", bufs=1, space="PSUM") as ptp:
                pt = [ptp.tile([128, TP], F32, tag=f"pt{dh}",
                               name=f"pt{i}_{dh}") for dh in range(2)]
                for cp in range(NBP):
                    for sh in range(2):
                        ab = bpool.tile([128, 2, TP // 2], F8, tag="abt",
                                        bufs=9)
                        (nc.sync if sh == 0 else nc.gpsimd).dma_start(
                            ab[:], Abt_d[cp][:, :, ts(sh, TP // 2)])
                        for dh in range(2):
                            for s in range(2):
                                nc.tensor.matmul(
                                    pt[dh][:, ts(2 * sh + s, 512)],
                                    tb8[:, ds(2 * cp, 2), dh, :],
                                    ab[:, :, ts(s, 512)],
                                    start=cp == 0, stop=cp == NBP - 1,
                                    perf_mode=DR)
                nc.vector.tensor_copy(pscr[:, 0, :], pt[0][:])
                nc.scalar.copy(pscr[:, 1, :], pt[1][:])
            bounce_in = dpool.tile([128, 2, TP], F8, tag="bin")
            bounce_out = dpool.tile([128, 2, TP], F8, tag="bout",
                                    addr_space="Shared")
            nc.scalar.dma_start(bounce_in[:], pscr[:])
            nc.gpsimd.collective_compute(
                "AllReduce", ALU.add, replica_groups=[list(range(NC))],
                ins=[bounce_in.opt()], outs=[bounce_out.opt()])
            return bounce_out

        def phase_C(i, glo=0, ghi=NBG):
            with tc.tile_pool(name=f"psC{i}_{glo}", bufs=6,
                              space="PSUM") as mmC:
                for g in range(glo, ghi):
                    ntiles = min(GRP, NBT - g * GRP)
                    pss = [mmC.tile([128, D], F32, tag="mm",
                                    name=f"cps{i}_{g}_{k}")
                           for k in range(ntiles)]
                    at = spool.tile([128, NTT, GW], F8, tag="at")
                    nc.sync.dma_start(at[:], At_d[g])
                    for t2 in range(NTT // 2):
                        for k in range(ntiles):
                            nc.tensor.matmul(pss[k][:],
                                             at[:, ds(2 * t2, 2), ts(k, 128)],
                                             tt8[:, ds(2 * t2, 2), :],
                                             start=t2 == 0, stop=False,
                                             perf_mode=DR)
                    for k in range(ntiles):
                        nt = g * GRP + k
                        nc.tensor.matmul(pss[k][:], ident[:],
                                         tl_b[:, nt, 1, :],
                                         start=False, stop=True)
                    nts = [g * GRP + k for k in range(ntiles)]
                    sage_epi_group([pss[k][:] for k in range(ntiles)],
                                   [tl_b[:, nt, 0, :] for nt in nts],
                                   [xb[:, nt, :] for nt in nts])

        def phase_D(i, bounce_out):
            jl = wi[("wl", i)]
            pm = ppool.tile([128, 2, TP], F8, tag="pscr")
            nc.sync.dma_start(pm[:], bounce_out[:])
            with tc.tile_pool(name=f"psD{i}", bufs=5, space="PSUM") as mmD:
                for g in range(NTT // GRP):
                    pss = []
                    for k in range(GRP):
                        tt_ = g * GRP + k
                        ps = mmD.tile([128, D], F32, tag="mm")
                        for kc in range(2):
                            nc.tensor.matmul(ps[:], pm[:, kc, ts(tt_, 128)],
                                             wc[:, jl + kc, :],
                                             start=kc == 0, stop=False)
                        nc.tensor.matmul(ps[:], ident[:], tl_t[:, tt_, 1, :],
                                         start=False, stop=True)
                        pss.append(ps)
                    tts = [g * GRP + k for k in range(GRP)]
                    sage_epi_group([p_[:] for p_ in pss],
                                   [tl_t[:, t_, 0, :] for t_ in tts],
                                   [xt[:, t_, :] for t_ in tts])

        # ================= main pipeline =================
        phase_A1b(0)
        phase_A2b(0)
        phase_A1t(0)
        phase_A2t(0)
        bo = phase_B(0)

        # final-phase tiles (tag reuse: layer tiles dead by first use)
        jfb = wi["fb"]
        jft = wi["ft"]
        jbb = wi["p1bb"]
        fl_b = fpool.tile([128, NBT, 2, D], BF, tag="tl_b")   # [lmp|hb1a]
        xtm_bf = fpool.tile([128, NTT, D], BF, tag="xt")
        htr_bf = fpool.tile([128, NTT, D], BF, tag="tl_t")
        mpz_bf = fpool.tile([128, NBT, D], BF, tag="xb")
        hball = fpool.tile([128, NBT, D], BF, tag="xbT")
        hbn_all = fpool.tile([128, NBT, 128], BF, tag="xtT")
        htn_T = fpool.tile([128, TP], BF, tag="tb8")

        def F1b_bacteria():
            with tc.tile_pool(name="psF1b", bufs=4, space="PSUM") as mmF:
                for nt in range(NBT):
                    ps = mmF.tile([128, 2, 256], F32, tag="mmb")
                    for kc in range(2):
                        nc.tensor.matmul(ps[:], xbT[:, kc, nt, :],
                                         wc[:, ds(jfb + 2 * kc, 2), :],
                                         start=kc == 0, stop=kc == 1)
                    cpy(fl_b[:, nt, :, :], ps[:])

        def F1b_traits():
            with tc.tile_pool(name="psF1t", bufs=4, space="PSUM") as mmF:
                for tt_ in range(NTT):
                    ps = mmF.tile([128, 512], F32, tag="mmt")
                    for kc in range(2):
                        nc.tensor.matmul(ps[:], xtT[:, kc, tt_, :],
                                         wc[:, ds(jft + 2 * kc, 2), :],
                                         start=kc == 0, stop=kc == 1)
                    cpy(xtm_bf[:, tt_, :], ps[:, 0:256])
                    ln_relu(htr_bf[:, tt_, :], ps[:, 256:512])

        for i in range(L):
            phase_C(i)
            if i + 1 < L:
                phase_A1b(i + 1)
                phase_A2b(i + 1)
                phase_D(i, bo)
                phase_A1t(i + 1)
                phase_A2t(i + 1)
                bo = phase_B(i + 1)
            else:
                # last layer: overlap D's epilogues with the bacteria-side
                # head matmuls (which only need xb/C(L-1))
                phase_A1b(9)
                F1b_bacteria()
                phase_D(i, bo)
                bpool_cm.__exit__(None, None, None)
                phase_A1t(9)
                F1b_traits()

        def _f3_stage1(g, trS, mmS1):
            """mpz tiles of group g -> transpose -> p1 -> relu-LN -> hball."""
            lo = g * GRP
            hi = min(lo + GRP, NBT)
            for nt0 in range(lo, hi, 2):
                pst = trS.tile([128, 2, 2, 128], BF, tag="tr")
                for k in range(2):
                    for kcc in range(2):
                        nc.tensor.transpose(pst[:, kcc, k, :],
                                            mpz_bf[:, nt0 + k, ts(kcc, 128)],
                                            ident[:])
                mpTp = hpool.tile([128, 2, 2, 128], BF, tag="htT")
                cpy(mpTp[:], pst[:])
                for k in range(2):
                    nt = nt0 + k
                    ps = mmS1.tile([128, D], F32, tag="mm")
                    for kc in range(2):
                        nc.tensor.matmul(ps[:], mpTp[:, kc, k, :],
                                         wc[:, jbb + kc, :],
                                         start=kc == 0, stop=kc == 1)
                    hv = epool.tile([128, D], F32, tag="cbv")
                    nc.vector.scalar_tensor_tensor(hv[:], ps[:], 1.0,
                                                   fl_b[:, nt, 1, :],
                                                   ALU.mult, ALU.add)
                    ln_relu(hball[:, nt, :], hv[:])

        simrec = {}

        def _f3_stage2(p, trP, mmP):
            """hball pair p -> transpose -> p2 -> hbn_all (UNnormalized;
            the 1/||hb|| row scale rides the sim output copy instead,
            keeping the reciprocal off the critical chain)."""
            nt0 = 2 * p
            psh = trP.tile([128, 2, 2, 128], BF, tag="tr")
            for k in range(2):
                for kcc in range(2):
                    nc.tensor.transpose(psh[:, kcc, k, :],
                                        hball[:, nt0 + k, ts(kcc, 128)],
                                        ident[:])
            hbTp = hpool.tile([128, 2, 2, 128], BF, tag="htT")
            cpy(hbTp[:], psh[:])
            hns = []
            for k in range(2):
                ps2 = mmP.tile([128, 128], F32, tag="mm2")
                for kc in range(2):
                    nc.tensor.matmul(ps2[:], hbTp[:, kc, k, :],
                                     w128[:, kc, :],
                                     start=kc == 0, stop=kc == 1)
                simrec[nt0 + k] = l2_rec(ps2[:])
                hn = hpool.tile([128, 128], BF, tag="hn")
                cpy(hn[:], ps2[:])
                hns.append(hn)
            psn = trP.tile([128, 2, 128], BF, tag="trn", bufs=2)
            for k in range(2):
                nc.tensor.transpose(psn[:, k, :], hns[k][:], ident[:])
            cpy(hbn_all[:, nt0:nt0 + 2, :], psn[:])

        def _f1c_pair(p, trp, mmp):
            """trait head pair p: htr -> transpose -> p2 -> l2 -> htn_T."""
            t0 = 2 * p
            pst = trp.tile([128, 2, 2, 128], BF, tag="tr")
            for k in range(2):
                for kcc in range(2):
                    nc.tensor.transpose(pst[:, kcc, k, :],
                                        htr_bf[:, t0 + k, ts(kcc, 128)],
                                        ident[:])
            htTp = hpool.tile([128, 2, 2, 128], BF, tag="htT")
            cpy(htTp[:], pst[:])
            hns = []
            for k in range(2):
                ps2 = mmp.tile([128, 128], F32, tag="mm2")
                for kc in range(2):
                    nc.tensor.matmul(ps2[:], htTp[:, kc, k, :],
                                     w128[:, 2 + kc, :],
                                     start=kc == 0, stop=kc == 1)
                rec = l2_rec(ps2[:], scale=temp)
                hn = hpool.tile([128, 128], BF, tag="hn")
                nc.vector.tensor_scalar_mul(hn[:], ps2[:], rec[:])
                hns.append(hn)
            psn = trp.tile([128, 2, 2, 128], BF, tag="tr")
            for k in range(2):
                nc.tensor.transpose(psn[:, 0, k, :], hns[k][:], ident[:])
            cpy(htn_T[:, ds(t0 * 128, 256)], psn[:, 0, :, :])

        # ---- F2 (metapath agg) fused with F3 stage 1 + F1c trait head ----
        with tc.tile_pool(name="psF2", bufs=5, space="PSUM") as mmZ, \
             tc.tile_pool(name="psS1t", bufs=2, space="PSUM") as trS, \
             tc.tile_pool(name="psF1c", bufs=1, space="PSUM") as mmH:
            for g in range(NBG):
                ntiles = min(GRP, NBT - g * GRP)
                pss = [mmZ.tile([128, D], F32, tag="mm", name=f"zps{g}_{k}")
                       for k in range(ntiles)]
                at = spool.tile([128, NTT, GW], F8, tag="at")
                nc.sync.dma_start(at[:], Amp_d[g])
                for tck in range(NTT):
                    for k in range(ntiles):
                        nc.tensor.matmul(pss[k][:], at[:, tck, ts(k, 128)],
                                         xtm_bf[:, tck, :],
                                         start=tck == 0, stop=tck == NTT - 1)
                zvg = epool.tile([128, GRP, D], F32, tag="s1g", bufs=2)
                for k in range(ntiles):
                    nt = g * GRP + k
                    nc.vector.scalar_tensor_tensor(zvg[:, k, :], pss[k][:],
                                                   1.0, fl_b[:, nt, 0, :],
                                                   ALU.mult, ALU.add)
                ln_z_group(zvg, ntiles,
                           [mpz_bf[:, g * GRP + k, :]
                            for k in range(ntiles)])
                _f1c_pair(g, trS, mmH)
                # stage 1 for the previous group's tiles (mpz ready)
                if g > 0:
                    _f3_stage1(g - 1, trS, mmZ)
            _f3_stage1(NBG - 1, trS, mmZ)

        # ---- F3 stage 2 (p2 head + normalize) fused with sim writeout ----
        with tc.tile_pool(name="psS2", bufs=2, space="PSUM") as mmP, \
             tc.tile_pool(name="psS2t", bufs=2, space="PSUM") as trP, \
             tc.tile_pool(name="psSim", bufs=2, space="PSUM") as mmS:
            def _sim_tile(nt):
                for s2 in range(2):
                    ob = hpool.tile([128, 1024], BF, tag="ob", bufs=2)
                    for s in range(2):
                        pso = mmS.tile([128, 512], F32, tag="sim")
                        nc.tensor.matmul(
                            pso[:], hbn_all[:, nt, :],
                            htn_T[:, ds(s2 * 1024 + s * 512, 512)],
                            start=True, stop=True)
                        scpy(ob[:, ts(s, 512)], pso[:], simrec[nt][:])
                    eng = nc.sync if (nt + s2) % 2 == 0 else nc.gpsimd
                    eng.dma_start(sim_d[nt][:, ts(s2, 1024)], ob[:])

            # sim tiles lag stage-2 by one pair so their PE work fills
            # the gaps left by stage-2's scalar/vector latency chains
            for p in range(NBP):
                _f3_stage2(p, trP, mmP)
                if p > 0:
                    _sim_tile(2 * p - 2)
                    _sim_tile(2 * p - 1)
            _sim_tile(NBT - 2)
            _sim_tile(NBT - 1)

    nc.compile()
    return nc


def kernel(**inputs):
    in_maps, meta = _prep(inputs)
    nc = build_program(meta)
    res = run_bass_kernel_spmd(nc, in_maps, core_ids=list(range(NC)))
    sim = np.empty((N_B, N_T), np.float32)
    for c in range(NC):
        shard = np.asarray(res.results[c]["simO"], np.float32).reshape(BP, TP)
        sim[c * B_SH:(c + 1) * B_SH] = shard[:B_SH, :N_T]
    if meta["simb"] != 0.0:
        sim += np.float32(meta["simb"])
    return sim
